# revision 1
# baseline (speedup 1.0000x reference)
"""CapsuleLayer dynamic-routing kernel for 8 TRN2 NeuronCores.

Sharding: in_size (i) is split 8 ways (144 rows/core); every core holds the
full batch.  u_hat (B,1152,10,16 = 189MB) is never materialized: both the
c-weighted sum (s_j) and the agreement update factor through x and W:

    s_un[b, (d,j)]   = sum_{(i,u)} x[b,u,i] * (e[i,j] * W[i,j,d,u])
    A[(i,u), (d,j)]  = sum_b x[b,u,i] * v[b,j,d]
    u_vj1[i,j]       = (1/B) sum_{u,d} W[i,j,d,u] * A[(i,u),(d,j)]

with e = exp(b_ij) unnormalized; the softmax denominator z_j = sum_i e[i,j]
rides inside the per-iteration collective of s_un (the only cross-core
traffic): AllReduce for routing iterations 1-2, ReduceScatter for the final
one (each core then squashes and emits only its own 32-batch output shard,
gathered host-side).  Key layout choices: the contraction index is (i*8+u)
on partitions; the 160-wide capsule axis is d-major (d*10+j) so every
broadcast lands on a step-1 innermost dim; s is produced in (b, (d,j))
orientation so squash needs no cross-partition reduction and v feeds the
A-matmul with no transposes; per-i-block sums/broadcasts (u_vj1, 1/z) are
constant 0/1-pattern matmuls.  All matmuls run in bf16 with fp32 PSUM
accumulation (rel err ~2e-3); exp/ln/copy/square stay in one ACT function
table so only one table load is ever issued.
"""

import os
import sys

import numpy as np

for _p in ("/opt/trn_rl_repo",):
    if _p not in sys.path and os.path.isdir(_p):
        sys.path.insert(0, _p)

import ml_dtypes

NCORES = 8
B, U, I = 256, 8, 1152
J, D = 10, 16
IL = I // NCORES        # 144 in_size rows per core
KL = IL * U             # 1152 local contraction length (i,u)
KT = KL // 128          # 9 partition tiles
JD = J * D              # 160
BF_COLS = KT * B + 2 * KL + KT * JD + 1            # 6049
F32_COLS = 256 + JD                                # m8 | ones | sel10

_CACHE = {}


def _build_module():
    import concourse.bacc as bacc
    import concourse.mybir as mybir
    import concourse.tile as tile

    f32 = mybir.dt.float32
    bf16 = mybir.dt.bfloat16
    AF = mybir.ActivationFunctionType
    ALU = mybir.AluOpType
    AX = mybir.AxisListType

    # Force the act-table pass's first-match lookup to land every function
    # we use (Exp, Ln, Copy, Square) on the one table that covers them all,
    # so only a single LoadActFuncSet is ever emitted.  Table *ids* are
    # positional, so we only hide functions from other tables, never reorder.
    import concourse.hw_specs as hw_specs
    if not hasattr(bacc, "_orig_get_activation_tables"):
        bacc._orig_get_activation_tables = bacc.get_activation_tables

        def _patched_tables(arch):
            tabs = bacc._orig_get_activation_tables(arch)
            AF_ = mybir.ActivationFunctionType
            ours = {AF_.Exp, AF_.Ln, AF_.Copy, AF_.Square, AF_.Identity}
            out = {}
            for name, s in tabs.items():
                if name == "natural_log_exp_and_others":
                    out[name] = s
                else:
                    out[name] = s - ours
            return out

        bacc.get_activation_tables = _patched_tables

    nc = bacc.Bacc(
        "TRN2", target_bir_lowering=False, debug=False, num_devices=NCORES
    )

    # all inputs packed host-side into two (128, N) blobs -> 2 load DMAs
    bf_d = nc.declare_dram_parameter("bfin", [128, BF_COLS], bf16, isOutput=False)
    f32_d = nc.declare_dram_parameter("f32in", [128, F32_COLS], f32, isOutput=False)
    out_d = nc.declare_dram_parameter("out", [B // NCORES, JD], f32, isOutput=True)

    with tile.TileContext(nc) as tc:
        with (
            tc.tile_pool(name="const", bufs=1) as cpool,
            tc.tile_pool(name="work", bufs=2) as wpool,
            tc.tile_pool(name="psum", bufs=1, space="PSUM") as ppool,
            tc.tile_pool(name="apsum", bufs=2, space="PSUM") as apool,
            tc.tile_pool(name="work3", bufs=3) as wpool3,
            tc.tile_pool(name="dram", bufs=3, space="DRAM") as dpool,
        ):
            # ---- persistent loads; separate tiles so dependency tracking
            # is exact: s-matmuls gate only on the slices they read ----
            W_END = KT * JD + 1
            XT_END = W_END + KT * B
            wsb_sb = cpool.tile([128, W_END], bf16)
            nc.sync.dma_start(wsb_sb[:, :], bf_d[:, 0:W_END])
            wsb = wsb_sb[:, 0:KT * JD].rearrange("p (t n) -> p t n", n=JD)
            ones8 = wsb_sb[:, KT * JD:W_END]
            xt_sb = cpool.tile([128, KT * B], bf16)
            nc.scalar.dma_start(xt_sb[:, :], bf_d[:, W_END:XT_END])
            xt = xt_sb[:, :].rearrange("p (t b) -> p t b", b=B)
            xb_sb = cpool.tile([128, 2 * KL], bf16)
            nc.sync.dma_start(xb_sb[:, :], bf_d[:, XT_END:BF_COLS])
            xb0 = xb_sb[:, 0:KL]
            xb1 = xb_sb[:, KL:2 * KL]
            f32_sb = cpool.tile([128, F32_COLS], f32)
            nc.sync.dma_start(f32_sb[:, :], f32_d[:, :])
            m8 = f32_sb[:, 0:128]
            ones10 = f32_sb[0:J, 128:256]     # (10, 128) of ones
            sel10 = f32_sb[0:J, 256:256 + JD]  # sel10[j', d*J+j] = (j==j')

            b_b = cpool.tile([128, KT, J], f32)     # b_ij replicated over u
            nc.vector.memset(b_b[:, :, :], 1.0)

            def wc_group(wc, e_b, g):
                nc.vector.tensor_tensor(
                    wc[:, 3 * g:3 * (g + 1), :].rearrange("p t (d j) -> p t d j", j=J),
                    wsb[:, 3 * g:3 * (g + 1), :].rearrange("p t (d j) -> p t d j", j=J),
                    e_b[:, 3 * g:3 * (g + 1), :].unsqueeze(2).broadcast_to([128, 3, D, J]),
                    ALU.mult,
                )

            def s_mms(s_ps, wc, t):
                # the two b-halves live in separate PSUM banks: a start=True
                # matmul clears its bank, so interleaved accumulation groups
                # must not share one
                s_ps0, s_ps1 = s_ps
                nc.tensor.matmul(
                    s_ps0[:, :], xt[:, t, 0:128], wc[:, t, :],
                    start=(t == 0), stop=(t == KT - 1),
                )
                nc.tensor.matmul(
                    s_ps1[:, :], xt[:, t, 128:B], wc[:, t, :],
                    start=(t == 0), stop=(t == KT - 1),
                )

            def z_mms(e_b):
                # z_loc[j] = sum_i e[i,j] = (1/8)*sum_partitions e_b, as a
                # (J,1) column; also keeps the PE warm between phases
                z_ps = ppool.tile([J, 1], f32, tag="z_ps")
                for t in range(KT):
                    nc.tensor.matmul(
                        z_ps[:, :], e_b[:, t, :], ones8[:, 0:1],
                        start=(t == 0), stop=(t == KT - 1),
                    )
                return z_ps

            def stage_and_collect(s_ps, z_ps, last):
                # stage [s | z] in SBUF: the z column is written into the
                # right partition blocks with tiny DVE copies so the two wide
                # DMAs carry everything; PSUM itself is not DMA-readable
                s_ps0, s_ps1 = s_ps
                # bf16 payload for the AllReduce iterations (the routing is
                # insensitive to s1/s2 rounding: 1.73e-3 vs 1.77e-3 end to
                # end); the final ReduceScatter stays fp32 since it feeds the
                # output directly
                cdt = f32 if last else bf16
                s_sb = wpool.tile([128, 2 * (JD + 1)], cdt, tag="s_sb")
                nc.scalar.copy(s_sb[:, 0:JD], s_ps0[:, :])
                nc.vector.tensor_copy(s_sb[:, JD + 1:2 * JD + 1], s_ps1[:, :])
                for r in range(4) if last else range(1):
                    nc.vector.tensor_copy(
                        s_sb[r * 32:r * 32 + J, JD:JD + 1], z_ps[:, :]
                    )
                    if last:
                        nc.vector.tensor_copy(
                            s_sb[r * 32:r * 32 + J, 2 * JD + 1:2 * JD + 2], z_ps[:, :]
                        )
                cc_in = dpool.tile([B, JD + 1], cdt, tag="cc_in")
                nc.sync.dma_start(
                    cc_in[:, :].rearrange("(c p) n -> p c n", p=128),
                    s_sb[:, :].rearrange("p (c n) -> p c n", n=JD + 1),
                )
                kind = "ReduceScatter" if last else "AllReduce"
                shape = [B // NCORES, JD + 1] if last else [B, JD + 1]
                cc_out = dpool.tile(shape, cdt, tag="cc3_out" if last else "cc_out", name="ccout")
                nc.gpsimd.collective_compute(
                    kind,
                    ALU.add,
                    replica_groups=[list(range(NCORES))],
                    ins=[cc_in.opt()],
                    outs=[cc_out.opt()],
                )
                return cc_out

            # ---- iteration 0 front: b0 == 1 ----
            e_b = wpool.tile([128, KT, J], bf16, tag="e_b")
            nc.scalar.activation(e_b[:, :, :], b_b[:, :, :], AF.Exp)
            wc = wpool.tile([128, KT, JD], bf16, tag="wc")
            for g in range(3):
                wc_group(wc, e_b, g)
            s_ps = (
                ppool.tile([128, JD], f32, tag="s_ps0", name="s_ps0"),
                ppool.tile([128, JD], f32, tag="s_ps1", name="s_ps1"),
            )
            for t in range(KT):
                s_mms(s_ps, wc, t)
            cc_out = stage_and_collect(s_ps, z_mms(e_b), last=False)

            for it in range(2):
                last_cc = it == 1

                # ---- post-AllReduce squash -> v ----
                sgz = wpool.tile([128, 2 * (JD + 1)], bf16, tag="sgz")
                nc.sync.dma_start(
                    sgz[:, :].rearrange("p (c n) -> p c n", n=JD + 1),
                    cc_out[:, :].rearrange("(c p) n -> p c n", p=128),
                )
                sg = sgz[:, :].rearrange("p (c n) -> p c n", n=JD + 1)[:, :, 0:JD]

                # zinv at (d,j) columns on all 128 partitions: recip the z
                # column, scale sel10 by it per-partition, then a (K=10)
                # ones-matmul lifts it to 128 partitions
                s_n = wpool.tile([128, 2 * JD], f32, tag="s_n")
                zinv = wpool.tile([J, 1], f32, tag="zinv")
                nc.vector.reciprocal(zinv[:, :], sgz[0:J, JD:JD + 1])
                zsel = wpool.tile([J, JD], f32, tag="zsel")
                nc.vector.tensor_scalar_mul(zsel[:, :], sel10[:, :], zinv[:, 0:1])
                zbc_ps = ppool.tile([128, JD], f32, tag="zbc_ps")
                nc.tensor.matmul(zbc_ps[:, :], ones10[:, :], zsel[:, :], start=True, stop=True)

                # s = s_un / z[j]
                nc.vector.tensor_tensor(
                    s_n[:, :].rearrange("p (c n) -> p c n", n=JD),
                    sg,
                    zbc_ps[:, :].unsqueeze(1).broadcast_to([128, 2, JD]),
                    ALU.mult,
                )

                # mag_sq[b, d] = sum_j s[b, (d,j)]^2 : plain innermost reduce
                sq = wpool.tile([128, 2 * JD], bf16, tag="sq")
                nc.scalar.square(sq[:, :], s_n[:, :])
                msq = wpool.tile([128, 2 * D], f32, tag="msq")
                nc.vector.tensor_reduce(
                    msq[:, :].rearrange("p (c d) -> p c d", d=D),
                    sq[:, :].rearrange("p (c d j) -> p c d j", d=D, j=J),
                    axis=AX.X,
                    op=ALU.add,
                )

                # F = sqrt(m)/(1+m) on the tiny (128, 32) footprint;
                # sqrt(m) = exp(0.5*ln(m)) keeps every ACT func in one table
                lnm = wpool.tile([128, 2 * D], f32, tag="lnm")
                nc.scalar.activation(lnm[:, :], msq[:, :], AF.Ln)
                rt = wpool.tile([128, 2 * D], f32, tag="rt")
                nc.scalar.activation(rt[:, :], lnm[:, :], AF.Exp, scale=0.5)
                dn = wpool.tile([128, 2 * D], f32, tag="dn")
                nc.vector.tensor_scalar_add(dn[:, :], msq[:, :], 1.0)
                rc = wpool.tile([128, 2 * D], f32, tag="rc")
                nc.vector.reciprocal(rc[:, :], dn[:, :])
                f_t = wpool.tile([128, 2 * D], f32, tag="f_t")
                nc.vector.tensor_mul(f_t[:, :], rt[:, :], rc[:, :])

                # v = s * F (F broadcast over j); v lands directly in the
                # (b, (d,j)) layout the A-matmul needs -- no transposes
                vt = wpool.tile([128, 2 * JD], bf16, tag="vt")
                for ch in range(2):
                    nc.vector.tensor_tensor(
                        vt[:, ch * JD:(ch + 1) * JD].rearrange("p (d j) -> p d j", j=J),
                        s_n[:, ch * JD:(ch + 1) * JD].rearrange("p (d j) -> p d j", j=J),
                        f_t[:, ch * D:(ch + 1) * D].unsqueeze(2).broadcast_to([128, D, J]),
                        ALU.mult,
                    )
                vb0 = vt[:, 0:JD]
                vb1 = vt[:, JD:2 * JD]

                # ---- fused: A-path group g immediately feeds the next
                # iteration's e/Wc/s-matmuls for those k-tiles ----
                e_b = wpool.tile([128, KT, J], bf16, tag="e_b")
                wc = wpool.tile([128, KT, JD], bf16, tag="wc")
                s_ps = (
                    ppool.tile([128, JD], f32, tag="s_ps0", name="s_ps0"),
                    ppool.tile([128, JD], f32, tag="s_ps1", name="s_ps1"),
                )
                r_t = wpool.tile([128, KT, J], f32, tag="r_t")
                uv_ps = ppool.tile([128, KT * J], f32, tag="uv_ps")
                for g in range(3):
                    a_ps = apool.tile([128, 3 * JD], f32, tag="a_ps")
                    for tt in range(3):
                        t = g * 3 + tt
                        nc.tensor.matmul(
                            a_ps[:, tt * JD:(tt + 1) * JD],
                            xb0[:, t * 128:(t + 1) * 128], vb0,
                            start=True, stop=False,
                        )
                        nc.tensor.matmul(
                            a_ps[:, tt * JD:(tt + 1) * JD],
                            xb1[:, t * 128:(t + 1) * 128], vb1,
                            start=False, stop=True,
                        )
                    a_sb = wpool3.tile([128, 3 * JD], bf16, tag="a_sb")
                    nc.scalar.copy(a_sb[:, :], a_ps[:, :])
                    p_t = wpool3.tile([128, 3 * JD], bf16, tag="p_t")
                    nc.vector.tensor_tensor(
                        p_t[:, :],
                        wsb[:, g * 3:(g + 1) * 3, :].rearrange("p a n -> p (a n)"),
                        a_sb[:, :],
                        ALU.mult,
                    )
                    nc.vector.tensor_reduce(
                        r_t[:, g * 3:(g + 1) * 3, :],
                        p_t.rearrange("p (a d j) -> p a j d", d=D, j=J),
                        axis=AX.X,
                        op=ALU.add,
                    )
                for t in range(KT):
                    nc.tensor.matmul(
                        uv_ps[:, t * J:(t + 1) * J], m8[:, :], r_t[:, t, :],
                        start=True, stop=True,
                    )
                nc.vector.tensor_add(
                    b_b[:, :, :], b_b[:, :, :],
                    uv_ps.rearrange("p (t j) -> p t j", j=J),
                )
                nc.scalar.activation(e_b[:, :, :], b_b[:, :, :], AF.Exp)
                for g in range(3):
                    wc_group(wc, e_b, g)
                for t in range(KT):
                    s_mms(s_ps, wc, t)
                cc_out = stage_and_collect(s_ps, z_mms(e_b), last=last_cc)

            # ---- post-ReduceScatter shard squash -> out ----
            sg3z = wpool.tile([32, JD + 1], f32, tag="sg3z")
            nc.sync.dma_start(sg3z[:, :], cc_out[0:32, 0:JD + 1])
            zinv3 = wpool.tile([J, 1], f32, tag="zinv3")
            nc.vector.reciprocal(zinv3[:, :], sg3z[0:J, JD:JD + 1])
            zsel3 = wpool.tile([J, JD], f32, tag="zsel3")
            nc.vector.tensor_scalar_mul(zsel3[:, :], sel10[:, :], zinv3[:, 0:1])
            zbc3 = ppool.tile([32, JD], f32, tag="zbc_ps", name="zbc3")
            nc.tensor.matmul(zbc3[:, :], ones10[:, 0:32], zsel3[:, :], start=True, stop=True)
            sn3 = wpool.tile([32, JD], f32, tag="sn3")
            nc.vector.tensor_mul(sn3[:, :], sg3z[0:32, 0:JD], zbc3[:, :])
            sq3 = wpool.tile([32, JD], bf16, tag="sq3")
            nc.scalar.square(sq3[:, :], sn3[:, :])
            msq3 = wpool.tile([32, D], f32, tag="msq3")
            nc.vector.tensor_reduce(
                msq3[:, :],
                sq3[:, :].rearrange("p (d j) -> p d j", j=J),
                axis=AX.X,
                op=ALU.add,
            )
            ln3 = wpool.tile([32, D], f32, tag="ln3")
            nc.scalar.activation(ln3[:, :], msq3[:, :], AF.Ln)
            rt3 = wpool.tile([32, D], f32, tag="rt3")
            nc.scalar.activation(rt3[:, :], ln3[:, :], AF.Exp, scale=0.5)
            dn3 = wpool.tile([32, D], f32, tag="dn3")
            nc.vector.tensor_scalar_add(dn3[:, :], msq3[:, :], 1.0)
            rc3 = wpool.tile([32, D], f32, tag="rc3")
            nc.vector.reciprocal(rc3[:, :], dn3[:, :])
            f3 = wpool.tile([32, D], f32, tag="f3")
            nc.vector.tensor_mul(f3[:, :], rt3[:, :], rc3[:, :])
            v3 = wpool.tile([32, JD], f32, tag="v3")
            nc.vector.tensor_tensor(
                v3[:, :].rearrange("p (d j) -> p d j", j=J),
                sn3[:, :].rearrange("p (d j) -> p d j", j=J),
                f3[:, :].unsqueeze(2).broadcast_to([32, D, J]),
                ALU.mult,
            )
            nc.sync.dma_start(out_d[:, :], v3[:, :])

    nc.finalize()
    return nc


def _f32_blob():
    blob = np.zeros((128, F32_COLS), np.float32)
    blob[:, 0:128] = np.kron(np.eye(16, dtype=np.float32), np.ones((8, 8), np.float32)) / B
    blob[0:J, 128:256] = 1.0
    blob[0:J, 256:256 + JD] = np.tile(np.eye(J, dtype=np.float32), (1, D))
    return blob


def _prep_in_maps(x, W):
    x = np.asarray(x, np.float32)
    W = np.asarray(W, np.float32)
    Wm = W[0]
    f32_blob = _f32_blob()
    in_maps = []
    for c in range(NCORES):
        sl = slice(c * IL, (c + 1) * IL)
        xs = x[:, :, sl]                                            # (B, U, IL)
        xt = np.ascontiguousarray(xs.transpose(2, 1, 0).reshape(KL, B))
        xb = xt.T
        w = Wm[sl].transpose(0, 3, 2, 1).reshape(KL, JD)   # cols = (d, j)
        bf = np.zeros((128, BF_COLS), np.float32)
        o = 0
        bf[:, o:o + KT * JD] = w.reshape(KT, 128, JD).transpose(1, 0, 2).reshape(128, KT * JD); o += KT * JD
        bf[:, o] = 0.125; o += 1
        bf[:, o:o + KT * B] = xt.reshape(KT, 128, B).transpose(1, 0, 2).reshape(128, KT * B); o += KT * B
        bf[:, o:o + KL] = xb[0:128]; o += KL
        bf[:, o:o + KL] = xb[128:256]; o += KL
        assert o == BF_COLS
        in_maps.append({
            "bfin": bf.astype(ml_dtypes.bfloat16),
            "f32in": f32_blob,
        })
    return in_maps


def run(x, W, trace=False):
    from concourse.bass_utils import run_bass_kernel_spmd

    if "nc" not in _CACHE:
        _CACHE["nc"] = _build_module()
    nc = _CACHE["nc"]
    in_maps = _prep_in_maps(x, W)
    res = run_bass_kernel_spmd(
        nc, in_maps, core_ids=list(range(NCORES)), trace=trace
    )
    v = np.concatenate(
        [np.asarray(res.results[c]["out"], np.float32) for c in range(NCORES)],
        axis=0,
    )                                                               # (B, (d,j))
    out = v.reshape(B, D, J).transpose(0, 2, 1)[..., None]
    return np.ascontiguousarray(out.astype(np.float32)), res


def kernel(x, W):
    out, _ = run(x, W, trace=False)
    return out



# revision 5
# speedup vs baseline: 1.0098x; 1.0098x over previous
"""CapsuleLayer dynamic-routing kernel for 8 TRN2 NeuronCores.

Sharding: in_size (i) is split 8 ways (144 rows/core); every core holds the
full batch.  u_hat (B,1152,10,16 = 189MB) is never materialized: both the
c-weighted sum (s_j) and the agreement update factor through x and W:

    s_un[b, (d,j)]   = sum_{(i,u)} x[b,u,i] * (e[i,j] * W[i,j,d,u])
    A[(i,u), (d,j)]  = sum_b x[b,u,i] * v[b,j,d]
    u_vj1[i,j]       = (1/B) sum_{u,d} W[i,j,d,u] * A[(i,u),(d,j)]

with e unnormalized; the softmax denominator z_j = sum_i e[i,j] rides inside
the per-iteration collective (the only cross-core traffic): AllReduce for
routing iterations 1-2, ReduceScatter for the final one (each core then
squashes and emits only its own 32-batch output shard, gathered host-side).

Structural choices vs the straightforward version:
  * iteration 0 has b=1 (uniform softmax), so s0 is a plain matmul on raw W
    with a compile-time softmax denominator z0=1152 -- no exp, no gating, no
    z column in the first collective;
  * b_ij is never materialized: e is tracked multiplicatively,
    e_{k+1} = e_k * exp(u_vj1), with iteration 1's e = exp(1 + u_vj1)
    produced by a single fused activation (Exp with bias=1);
  * inputs stream in three (W,x) k-groups so the first matmuls start while
    the rest of the load is in flight;
  * the agreement block is pipelined per k-group across four engines
    (PE matmul -> ACT drain -> gate -> DVE d-reduce -> PE u-fold -> ACT exp
    -> DVE gate -> PE s-matmul);
  * collective payloads are [128, W]-shaped so DMAs move 128 fat descriptors
    instead of 256 thin ones.
All matmuls run in bf16 with fp32 PSUM accumulation; exp/ln/copy live in one
ACT function table so only one table load is ever issued.
"""

import os
import sys

import numpy as np

for _p in ("/opt/trn_rl_repo",):
    if _p not in sys.path and os.path.isdir(_p):
        sys.path.insert(0, _p)

import ml_dtypes

NCORES = 8
B, U, I = 256, 8, 1152
J, D = 10, 16
IL = I // NCORES        # 144 in_size rows per core
KL = IL * U             # 1152 local contraction length (i,u)
KT = KL // 128          # 9 partition tiles
NG = 3                  # k-groups of 3 tiles each
GT = KT // NG           # tiles per group
JD = J * D              # 160
GRP = GT * (JD + B)     # bf16 cols per load group
BF_COLS = NG * GRP + 2 * KL + 1                     # groups | xb0 | xb1 | ones8
F32_COLS = 256 + JD                                 # m8 | ones10 | sel10

_CACHE = {}


def _build_module(a_gate_pool=True):
    import concourse.bacc as bacc
    import concourse.mybir as mybir
    import concourse.tile as tile

    f32 = mybir.dt.float32
    bf16 = mybir.dt.bfloat16
    AF = mybir.ActivationFunctionType
    ALU = mybir.AluOpType
    AX = mybir.AxisListType

    # Force the act-table pass's first-match lookup to land every function
    # we use (Exp, Ln, Copy) on the one table that covers them all, so only
    # a single LoadActFuncSet is ever emitted.  Table *ids* are positional,
    # so we only hide functions from other tables, never reorder.
    if not hasattr(bacc, "_orig_get_activation_tables"):
        bacc._orig_get_activation_tables = bacc.get_activation_tables

        def _patched_tables(arch):
            tabs = bacc._orig_get_activation_tables(arch)
            AF_ = mybir.ActivationFunctionType
            ours = {AF_.Exp, AF_.Ln, AF_.Copy, AF_.Square, AF_.Identity}
            out = {}
            for name, s in tabs.items():
                if name == "natural_log_exp_and_others":
                    out[name] = s
                else:
                    out[name] = s - ours
            return out

        bacc.get_activation_tables = _patched_tables

    nc = bacc.Bacc(
        "TRN2", target_bir_lowering=False, debug=False, num_devices=NCORES
    )

    bf_d = nc.declare_dram_parameter("bfin", [128, BF_COLS], bf16, isOutput=False)
    f32_d = nc.declare_dram_parameter("f32in", [128, F32_COLS], f32, isOutput=False)
    out_d = nc.declare_dram_parameter("out", [B // NCORES, JD], f32, isOutput=True)

    a_gate = None  # set below

    with tile.TileContext(nc) as tc:
        with (
            tc.tile_pool(name="const", bufs=1) as cpool,
            tc.tile_pool(name="work", bufs=2) as wpool,
            tc.tile_pool(name="psum", bufs=1, space="PSUM") as ppool,
            tc.tile_pool(name="apsum", bufs=2, space="PSUM") as apool,
            tc.tile_pool(name="work3", bufs=3) as wpool3,
            tc.tile_pool(name="ework", bufs=2) as epool,
            tc.tile_pool(name="dram", bufs=3, space="DRAM") as dpool,
        ):
            a_gate = nc.gpsimd if a_gate_pool else nc.vector

            # ---- streamed loads: 3 (W | xt) k-groups so matmuls start
            # early, then the A-path / normalization constants ----
            grp_sb = []
            for g in range(NG):
                t_ = cpool.tile([128, GRP], bf16, tag=f"grp{g}", name=f"grp{g}")
                nc.sync.dma_start(t_[:, :], bf_d[:, g * GRP:(g + 1) * GRP])
                grp_sb.append(t_)
            wsb = [
                t_[:, 0:GT * JD].rearrange("p (t n) -> p t n", n=JD)
                for t_ in grp_sb
            ]
            xt = [
                t_[:, GT * JD:GRP].rearrange("p (t b) -> p t b", b=B)
                for t_ in grp_sb
            ]
            XB0 = NG * GRP
            xb_sb = cpool.tile([128, 2 * KL + 1], bf16)
            nc.scalar.dma_start(xb_sb[:, :], bf_d[:, XB0:BF_COLS])
            xb0 = xb_sb[:, 0:KL]
            xb1 = xb_sb[:, KL:2 * KL]
            ones8 = xb_sb[:, 2 * KL:2 * KL + 1]
            f32_sb = cpool.tile([128, F32_COLS], f32)
            nc.scalar.dma_start(f32_sb[:, :], f32_d[:, :])
            m8 = f32_sb[:, 0:128]
            ones10 = f32_sb[0:J, 128:256]      # (10, 128) of ones
            sel10 = f32_sb[0:J, 256:256 + JD]  # sel10[j', d*J+j] = (j==j')

            def s_mms(s_ps, rhs_of, first, last):
                # the two b-halves live in separate PSUM banks: a start=True
                # matmul clears its bank, so interleaved accumulation groups
                # must not share one
                s_ps0, s_ps1 = s_ps
                for g in range(NG):
                    for t_ in range(GT):
                        st = first and g == 0 and t_ == 0
                        sp = last and g == NG - 1 and t_ == GT - 1
                        nc.tensor.matmul(
                            s_ps0[:, :], xt[g][:, t_, 0:128], rhs_of(g, t_),
                            start=st, stop=sp,
                        )
                        nc.tensor.matmul(
                            s_ps1[:, :], xt[g][:, t_, 128:B], rhs_of(g, t_),
                            start=st, stop=sp,
                        )

            def stage_and_collect(s_ps, z_ps, last):
                # stage [s | z] in SBUF; PSUM itself is not DMA-readable.
                # Payload stays [128, W]-shaped (fat rows -> 128 descriptors)
                # for the AllReduce iterations; the final ReduceScatter needs
                # batch on the outer axis so each core receives its 32-row
                # output shard.
                s_ps0, s_ps1 = s_ps
                if not last:
                    width = 2 * JD + (1 if z_ps is not None else 0)
                    s_sb = wpool.tile([128, width], bf16, tag="s_sb")
                    nc.scalar.copy(s_sb[:, 0:JD], s_ps0[:, :])
                    nc.vector.tensor_copy(s_sb[:, JD:2 * JD], s_ps1[:, :])
                    if z_ps is not None:
                        nc.vector.tensor_copy(
                            s_sb[0:J, 2 * JD:2 * JD + 1], z_ps[:, :]
                        )
                    cc_in = dpool.tile([128, width], bf16, tag="cc_in")
                    nc.sync.dma_start(cc_in[:, :], s_sb[:, :])
                    cc_out = dpool.tile([128, width], bf16, tag="cc_out", name="ccout")
                    nc.gpsimd.collective_compute(
                        "AllReduce",
                        ALU.add,
                        replica_groups=[list(range(NCORES))],
                        ins=[cc_in.opt()],
                        outs=[cc_out.opt()],
                    )
                    return cc_out
                # final iteration: fp32, feeds the output directly
                s_sb = wpool.tile([128, 2 * (JD + 1)], f32, tag="s_sb3")
                nc.scalar.copy(s_sb[:, 0:JD], s_ps0[:, :])
                nc.vector.tensor_copy(s_sb[:, JD + 1:2 * JD + 1], s_ps1[:, :])
                for r in range(4):
                    nc.vector.tensor_copy(
                        s_sb[r * 32:r * 32 + J, JD:JD + 1], z_ps[:, :]
                    )
                    nc.vector.tensor_copy(
                        s_sb[r * 32:r * 32 + J, 2 * JD + 1:2 * JD + 2], z_ps[:, :]
                    )
                cc_in = dpool.tile([B, JD + 1], f32, tag="cc3_in")
                nc.sync.dma_start(
                    cc_in[:, :].rearrange("(c p) n -> p c n", p=128),
                    s_sb[:, :].rearrange("p (c n) -> p c n", n=JD + 1),
                )
                cc_out = dpool.tile([B // NCORES, JD + 1], f32, tag="cc3_out", name="ccout3")
                nc.gpsimd.collective_compute(
                    "ReduceScatter",
                    ALU.add,
                    replica_groups=[list(range(NCORES))],
                    ins=[cc_in.opt()],
                    outs=[cc_out.opt()],
                )
                return cc_out

            # ---- iteration 0 front: b0 == 1 -> uniform softmax: plain
            # matmul on raw W, denominator is the constant 1152 ----
            s_ps = (
                ppool.tile([128, JD], f32, tag="s_ps0", name="s_ps0"),
                ppool.tile([128, JD], f32, tag="s_ps1", name="s_ps1"),
            )
            s_mms(s_ps, lambda g, t_: wsb[g][:, t_, :], True, True)
            cc_out = stage_and_collect(s_ps, None, last=False)

            e_tiles = [None] * NG
            for it in range(2):
                last_cc = it == 1

                # ---- post-AllReduce squash -> v ----
                width = 2 * JD + (1 if it > 0 else 0)
                sgz = wpool.tile([128, width], bf16, tag="sgz")
                nc.sync.dma_start(sgz[:, :], cc_out[:, :])
                sg = sgz[:, 0:2 * JD]

                s_n = wpool.tile([128, 2 * JD], bf16, tag="s_n")
                if it == 0:
                    # z0 = 1152 exactly (uniform softmax over in_size)
                    nc.vector.tensor_scalar_mul(s_n[:, :], sg[:, :], 1.0 / I)
                else:
                    # zinv at (d,j) columns on all 128 partitions: recip the
                    # z column, scale sel10 by it, lift via a (K=10) matmul
                    zinv = wpool.tile([J, 1], f32, tag="zinv")
                    nc.vector.reciprocal(zinv[:, :], sgz[0:J, 2 * JD:2 * JD + 1])
                    zsel = wpool.tile([J, JD], f32, tag="zsel")
                    nc.vector.tensor_scalar_mul(zsel[:, :], sel10[:, :], zinv[:, 0:1])
                    zbc_ps = ppool.tile([128, JD], f32, tag="zbc_ps")
                    nc.tensor.matmul(
                        zbc_ps[:, :], ones10[:, :], zsel[:, :], start=True, stop=True
                    )
                    nc.vector.tensor_tensor(
                        s_n[:, :].rearrange("p (c n) -> p c n", n=JD),
                        sg.rearrange("p (c n) -> p c n", n=JD),
                        zbc_ps[:, :].unsqueeze(1).broadcast_to([128, 2, JD]),
                        ALU.mult,
                    )

                # mag_sq[b, d] = sum_j s[b, (d,j)]^2 : square then innermost
                # reduce; F = sqrt(m)/(1+m) with the ACT (ln,exp) pair and
                # the DVE (1+m, recip) pair running in parallel off msq
                sq = wpool.tile([128, 2 * JD], bf16, tag="sq")
                nc.vector.tensor_mul(sq[:, :], s_n[:, :], s_n[:, :])
                msq = wpool.tile([128, 2 * D], f32, tag="msq")
                nc.vector.tensor_reduce(
                    msq[:, :].rearrange("p (c d) -> p c d", d=D),
                    sq[:, :].rearrange("p (c d j) -> p c d j", d=D, j=J),
                    axis=AX.X,
                    op=ALU.add,
                )
                lnm = wpool.tile([128, 2 * D], f32, tag="lnm")
                nc.scalar.activation(lnm[:, :], msq[:, :], AF.Ln)
                rt = wpool.tile([128, 2 * D], f32, tag="rt")
                nc.scalar.activation(rt[:, :], lnm[:, :], AF.Exp, scale=0.5)
                dn = wpool.tile([128, 2 * D], f32, tag="dn")
                nc.vector.tensor_scalar_add(dn[:, :], msq[:, :], 1.0)
                rc = wpool.tile([128, 2 * D], f32, tag="rc")
                nc.vector.reciprocal(rc[:, :], dn[:, :])
                f_t = wpool.tile([128, 2 * D], f32, tag="f_t")
                nc.vector.tensor_mul(f_t[:, :], rt[:, :], rc[:, :])

                # v = s * F (F broadcast over j); v lands directly in the
                # (b, (d,j)) layout the A-matmul needs -- no transposes
                vt = wpool.tile([128, 2 * JD], bf16, tag="vt")
                for ch in range(2):
                    nc.vector.tensor_tensor(
                        vt[:, ch * JD:(ch + 1) * JD].rearrange("p (d j) -> p d j", j=J),
                        s_n[:, ch * JD:(ch + 1) * JD].rearrange("p (d j) -> p d j", j=J),
                        f_t[:, ch * D:(ch + 1) * D].unsqueeze(2).broadcast_to([128, D, J]),
                        ALU.mult,
                    )
                vb0 = vt[:, 0:JD]
                vb1 = vt[:, JD:2 * JD]

                # ---- fused per-group pipeline: A-path group g immediately
                # feeds that group's e-update, Wc gate and s-matmuls ----
                s_ps = (
                    ppool.tile([128, JD], f32, tag="s_ps0", name="s_ps0"),
                    ppool.tile([128, JD], f32, tag="s_ps1", name="s_ps1"),
                )
                z_ps = ppool.tile([J, 1], f32, tag="z_ps")
                wc_of = []
                for g in range(NG):
                    a_ps = apool.tile([128, GT * JD], f32, tag="a_ps")
                    for tt in range(GT):
                        t_ = g * GT + tt
                        nc.tensor.matmul(
                            a_ps[:, tt * JD:(tt + 1) * JD],
                            xb0[:, t_ * 128:(t_ + 1) * 128], vb0,
                            start=True, stop=False,
                        )
                        nc.tensor.matmul(
                            a_ps[:, tt * JD:(tt + 1) * JD],
                            xb1[:, t_ * 128:(t_ + 1) * 128], vb1,
                            start=False, stop=True,
                        )
                    a_sb = wpool3.tile([128, GT * JD], bf16, tag="a_sb")
                    nc.scalar.copy(a_sb[:, :], a_ps[:, :])
                    p_t = wpool3.tile([128, GT * JD], bf16, tag="p_t")
                    a_gate.tensor_tensor(
                        p_t[:, :],
                        wsb[g][:, :, :].rearrange("p a n -> p (a n)"),
                        a_sb[:, :],
                        ALU.mult,
                    )
                    r_t = wpool3.tile([128, GT, J], f32, tag="r_t")
                    nc.vector.tensor_reduce(
                        r_t[:, :, :],
                        p_t.rearrange("p (a d j) -> p a j d", d=D, j=J),
                        axis=AX.X,
                        op=ALU.add,
                    )
                    uv_ps = apool.tile([128, GT * J], f32, tag="uv_ps")
                    for tt in range(GT):
                        nc.tensor.matmul(
                            uv_ps[:, tt * J:(tt + 1) * J], m8[:, :], r_t[:, tt, :],
                            start=True, stop=True,
                        )
                    e_new = epool.tile([128, GT, J], bf16, tag=f"e{g}")
                    if it == 0:
                        # e1 = exp(1 + u_vj1): fused add+exp
                        nc.scalar.activation(
                            e_new[:, :, :],
                            uv_ps[:, :].rearrange("p (a j) -> p a j", j=J),
                            AF.Exp,
                            bias=1.0,
                        )
                    else:
                        expuv = wpool3.tile([128, GT, J], bf16, tag="expuv")
                        nc.scalar.activation(
                            expuv[:, :, :],
                            uv_ps[:, :].rearrange("p (a j) -> p a j", j=J),
                            AF.Exp,
                        )
                        nc.vector.tensor_tensor(
                            e_new[:, :, :], e_tiles[g][:, :, :], expuv[:, :, :],
                            ALU.mult,
                        )
                    e_tiles[g] = e_new
                    # z partial: z[j] = sum_i e[i,j] as a (J,1) column
                    for tt in range(GT):
                        nc.tensor.matmul(
                            z_ps[:, :], e_new[:, tt, :], ones8[:, 0:1],
                            start=(g == 0 and tt == 0),
                            stop=(g == NG - 1 and tt == GT - 1),
                        )
                    wc = wpool3.tile([128, GT, JD], bf16, tag="wc")
                    nc.vector.tensor_tensor(
                        wc[:, :, :].rearrange("p t (d j) -> p t d j", j=J),
                        wsb[g][:, :, :].rearrange("p t (d j) -> p t d j", j=J),
                        e_new[:, :, :].unsqueeze(2).broadcast_to([128, GT, D, J]),
                        ALU.mult,
                    )
                    wc_of.append(wc)
                    for tt in range(GT):
                        st = g == 0 and tt == 0
                        sp = g == NG - 1 and tt == GT - 1
                        nc.tensor.matmul(
                            s_ps[0][:, :], xt[g][:, tt, 0:128], wc[:, tt, :],
                            start=st, stop=sp,
                        )
                        nc.tensor.matmul(
                            s_ps[1][:, :], xt[g][:, tt, 128:B], wc[:, tt, :],
                            start=st, stop=sp,
                        )
                cc_out = stage_and_collect(s_ps, z_ps, last=last_cc)

            # ---- post-ReduceScatter shard squash -> out ----
            sg3z = wpool.tile([32, JD + 1], f32, tag="sg3z")
            nc.sync.dma_start(sg3z[:, :], cc_out[0:32, 0:JD + 1])
            zinv3 = wpool.tile([J, 1], f32, tag="zinv3")
            nc.vector.reciprocal(zinv3[:, :], sg3z[0:J, JD:JD + 1])
            zsel3 = wpool.tile([J, JD], f32, tag="zsel3")
            nc.vector.tensor_scalar_mul(zsel3[:, :], sel10[:, :], zinv3[:, 0:1])
            zbc3 = ppool.tile([32, JD], f32, tag="zbc_ps", name="zbc3")
            nc.tensor.matmul(zbc3[:, :], ones10[:, 0:32], zsel3[:, :], start=True, stop=True)
            sn3 = wpool.tile([32, JD], f32, tag="sn3")
            nc.vector.tensor_mul(sn3[:, :], sg3z[0:32, 0:JD], zbc3[:, :])
            sq3 = wpool.tile([32, JD], bf16, tag="sq3")
            nc.vector.tensor_mul(sq3[:, :], sn3[:, :], sn3[:, :])
            msq3 = wpool.tile([32, D], f32, tag="msq3")
            nc.vector.tensor_reduce(
                msq3[:, :],
                sq3[:, :].rearrange("p (d j) -> p d j", j=J),
                axis=AX.X,
                op=ALU.add,
            )
            ln3 = wpool.tile([32, D], f32, tag="ln3")
            nc.scalar.activation(ln3[:, :], msq3[:, :], AF.Ln)
            rt3 = wpool.tile([32, D], f32, tag="rt3")
            nc.scalar.activation(rt3[:, :], ln3[:, :], AF.Exp, scale=0.5)
            dn3 = wpool.tile([32, D], f32, tag="dn3")
            nc.vector.tensor_scalar_add(dn3[:, :], msq3[:, :], 1.0)
            rc3 = wpool.tile([32, D], f32, tag="rc3")
            nc.vector.reciprocal(rc3[:, :], dn3[:, :])
            f3 = wpool.tile([32, D], f32, tag="f3")
            nc.vector.tensor_mul(f3[:, :], rt3[:, :], rc3[:, :])
            v3 = wpool.tile([32, JD], f32, tag="v3")
            nc.vector.tensor_tensor(
                v3[:, :].rearrange("p (d j) -> p d j", j=J),
                sn3[:, :].rearrange("p (d j) -> p d j", j=J),
                f3[:, :].unsqueeze(2).broadcast_to([32, D, J]),
                ALU.mult,
            )
            nc.sync.dma_start(out_d[:, :], v3[:, :])

    nc.finalize()
    return nc


def _f32_blob():
    blob = np.zeros((128, F32_COLS), np.float32)
    blob[:, 0:128] = np.kron(np.eye(16, dtype=np.float32), np.ones((8, 8), np.float32)) / B
    blob[0:J, 128:256] = 1.0
    blob[0:J, 256:256 + JD] = np.tile(np.eye(J, dtype=np.float32), (1, D))
    return blob


def _prep_in_maps(x, W):
    x = np.asarray(x, np.float32)
    W = np.asarray(W, np.float32)
    Wm = W[0]
    f32_blob = _f32_blob()
    in_maps = []
    for c in range(NCORES):
        sl = slice(c * IL, (c + 1) * IL)
        xs = x[:, :, sl]                                            # (B, U, IL)
        xt = np.ascontiguousarray(xs.transpose(2, 1, 0).reshape(KL, B))
        xb = xt.T
        w = Wm[sl].transpose(0, 3, 2, 1).reshape(KL, JD)            # cols = (d, j)
        wt = w.reshape(KT, 128, JD).transpose(1, 0, 2)              # (128, KT, JD)
        xtt = xt.reshape(KT, 128, B).transpose(1, 0, 2)             # (128, KT, B)
        bf = np.zeros((128, BF_COLS), np.float32)
        o = 0
        for g in range(NG):
            bf[:, o:o + GT * JD] = wt[:, g * GT:(g + 1) * GT].reshape(128, GT * JD)
            o += GT * JD
            bf[:, o:o + GT * B] = xtt[:, g * GT:(g + 1) * GT].reshape(128, GT * B)
            o += GT * B
        bf[:, o:o + KL] = xb[0:128]; o += KL
        bf[:, o:o + KL] = xb[128:256]; o += KL
        bf[:, o] = 0.125; o += 1
        assert o == BF_COLS
        in_maps.append({
            "bfin": bf.astype(ml_dtypes.bfloat16),
            "f32in": f32_blob,
        })
    return in_maps


def run(x, W, trace=False):
    from concourse.bass_utils import run_bass_kernel_spmd

    if "nc" not in _CACHE:
        _CACHE["nc"] = _build_module()
    nc = _CACHE["nc"]
    in_maps = _prep_in_maps(x, W)
    res = run_bass_kernel_spmd(
        nc, in_maps, core_ids=list(range(NCORES)), trace=trace
    )
    v = np.concatenate(
        [np.asarray(res.results[c]["out"], np.float32) for c in range(NCORES)],
        axis=0,
    )                                                               # (B, (d,j))
    out = v.reshape(B, D, J).transpose(0, 2, 1)[..., None]
    return np.ascontiguousarray(out.astype(np.float32)), res


def kernel(x, W):
    out, _ = run(x, W, trace=False)
    return out


# revision 31
# speedup vs baseline: 1.0382x; 1.0282x over previous
"""CapsuleLayer dynamic-routing kernel for 8 TRN2 NeuronCores.

Sharding: in_size (i) is split 8 ways (144 rows/core); every core holds the
full batch.  u_hat (B,1152,10,16 = 189MB) is never materialized: both the
c-weighted sum (s_j) and the agreement update factor through x and W:

    s_un[b, (d,j)]   = sum_{(i,u)} x[b,u,i] * (e[i,j] * W[i,j,d,u])
    A[(i,u), (d,j)]  = sum_b x[b,u,i] * v[b,j,d]
    u_vj1[i,j]       = (1/B) sum_{u,d} W[i,j,d,u] * A[(i,u),(d,j)]

with e unnormalized; the softmax denominator z_j = sum_i e[i,j] rides inside
the per-iteration collective (the only cross-core traffic): AllReduce for
routing iterations 1-2, ReduceScatter for the final one (each core then
squashes and emits only its own 32-batch output shard, gathered host-side).

Structural choices vs the straightforward version:
  * iteration 0 has b=1 (uniform softmax), so s0 is a plain matmul on raw W
    with a compile-time softmax denominator z0=1152 -- no exp, no gating, no
    z column in the first collective;
  * b_ij is never materialized: e is tracked multiplicatively,
    e_{k+1} = e_k * exp(u_vj1), with iteration 1's e = exp(1 + u_vj1)
    produced by a single fused activation (Exp with bias=1);
  * inputs stream in three (W,x) k-groups so the first matmuls start while
    the rest of the load is in flight;
  * the agreement block is pipelined per k-group across four engines
    (PE matmul -> ACT drain -> gate -> DVE d-reduce -> PE u-fold -> ACT exp
    -> DVE gate -> PE s-matmul);
  * collective payloads are [128, W]-shaped so DMAs move 128 fat descriptors
    instead of 256 thin ones.
All matmuls run in bf16 with fp32 PSUM accumulation; exp/ln/copy live in one
ACT function table so only one table load is ever issued.
"""

import os
import sys

import numpy as np

for _p in ("/opt/trn_rl_repo",):
    if _p not in sys.path and os.path.isdir(_p):
        sys.path.insert(0, _p)

import ml_dtypes

NCORES = 8
B, U, I = 256, 8, 1152
J, D = 10, 16
IL = I // NCORES        # 144 in_size rows per core
KL = IL * U             # 1152 local contraction length (i,u)
KT = KL // 128          # 9 partition tiles
GROUPS = [3, 3, 3]      # k-tile groups for loads / e / Wc / s-matmuls
GOFF = [0, 3, 6]        # cumulative k-tile offsets
NG = len(GROUPS)
JD = J * D              # 160
BF_COLS = KT * (JD + B) + 2 * KL + 1                # groups | xb0 | xb1 | ones8
F32_COLS = 256 + JD                                 # m8 | ones10 | sel10

_CACHE = {}


def _build_module(a_gate_pool=True):
    import concourse.bacc as bacc
    import concourse.mybir as mybir
    import concourse.tile as tile

    f32 = mybir.dt.float32
    bf16 = mybir.dt.bfloat16
    AF = mybir.ActivationFunctionType
    ALU = mybir.AluOpType
    AX = mybir.AxisListType

    # Force the act-table pass's first-match lookup to land every function
    # we use (Exp, Ln, Copy) on the one table that covers them all, so only
    # a single LoadActFuncSet is ever emitted.  Table *ids* are positional,
    # so we only hide functions from other tables, never reorder.
    if not hasattr(bacc, "_orig_get_activation_tables"):
        bacc._orig_get_activation_tables = bacc.get_activation_tables

        def _patched_tables(arch):
            tabs = bacc._orig_get_activation_tables(arch)
            AF_ = mybir.ActivationFunctionType
            ours = {AF_.Exp, AF_.Ln, AF_.Copy, AF_.Square, AF_.Identity}
            out = {}
            for name, s in tabs.items():
                if name == "natural_log_exp_and_others":
                    out[name] = s
                else:
                    out[name] = s - ours
            return out

        bacc.get_activation_tables = _patched_tables

    nc = bacc.Bacc(
        "TRN2", target_bir_lowering=False, debug=False, num_devices=NCORES
    )

    bf_d = nc.declare_dram_parameter("bfin", [128, BF_COLS], bf16, isOutput=False)
    f32_d = nc.declare_dram_parameter("f32in", [128, F32_COLS], f32, isOutput=False)
    out_d = nc.declare_dram_parameter("out", [B // NCORES, JD], f32, isOutput=True)

    a_gate = None  # set below

    with tile.TileContext(nc) as tc:
        with (
            tc.tile_pool(name="const", bufs=1) as cpool,
            tc.tile_pool(name="work", bufs=2) as wpool,
            tc.tile_pool(name="psum", bufs=1, space="PSUM") as ppool,
            tc.tile_pool(name="apsum", bufs=3, space="PSUM") as apool,
            tc.tile_pool(name="work3", bufs=3) as wpool3,
            tc.tile_pool(name="ework", bufs=2) as epool,
            tc.tile_pool(name="dram", bufs=3, space="DRAM") as dpool,
        ):
            a_gate = nc.gpsimd if a_gate_pool else nc.vector

            # ---- streamed loads: 3 (W | xt) k-groups so matmuls start
            # early, then the A-path / normalization constants ----
            wsb, xt = [], []
            off = 0
            for g, gt in enumerate(GROUPS):
                grp = gt * (JD + B)
                t_ = cpool.tile([128, grp], bf16, tag=f"grp{g}", name=f"grp{g}")
                nc.sync.dma_start(t_[:, :], bf_d[:, off:off + grp])
                off += grp
                wsb.append(t_[:, 0:gt * JD].rearrange("p (t n) -> p t n", n=JD))
                xt.append(t_[:, gt * JD:grp].rearrange("p (t b) -> p t b", b=B))
            XB0 = off
            xb_sb = cpool.tile([128, 2 * KL + 1], bf16)
            xb0 = xb_sb[:, 0:KL]
            xb1 = xb_sb[:, KL:2 * KL]
            ones8 = xb_sb[:, 2 * KL:2 * KL + 1]
            f32_sb = cpool.tile([128, F32_COLS], f32)
            m8 = f32_sb[:, 0:128]
            ones10 = f32_sb[0:J, 128:256]      # (10, 128) of ones
            sel10 = f32_sb[0:J, 256:256 + JD]  # sel10[j', d*J+j] = (j==j')

            def s_mms(s_ps, rhs_of, first, last):
                # the two b-halves live in separate PSUM banks: a start=True
                # matmul clears its bank, so interleaved accumulation groups
                # must not share one
                s_ps0, s_ps1 = s_ps
                for g, gt in enumerate(GROUPS):
                    for t_ in range(gt):
                        st = first and g == 0 and t_ == 0
                        sp = last and g == NG - 1 and t_ == gt - 1
                        nc.tensor.matmul(
                            s_ps0[:, :], xt[g][:, t_, 0:128], rhs_of(g, t_),
                            start=st, stop=sp,
                        )
                        nc.tensor.matmul(
                            s_ps1[:, :], xt[g][:, t_, 128:B], rhs_of(g, t_),
                            start=st, stop=sp,
                        )

            def stage_and_collect(s_ps, z_ps, last):
                # stage [s | z] in SBUF; PSUM itself is not DMA-readable.
                # Payload stays [128, W]-shaped (fat rows -> 128 descriptors)
                # for the AllReduce iterations; the final ReduceScatter needs
                # batch on the outer axis so each core receives its 32-row
                # output shard.
                s_ps0, s_ps1 = s_ps
                if not last:
                    width = 2 * JD + (1 if z_ps is not None else 0)
                    s_sb = wpool.tile([128, width], bf16, tag="s_sb")
                    nc.scalar.copy(s_sb[:, 0:JD], s_ps0[:, :])
                    nc.vector.tensor_copy(s_sb[:, JD:2 * JD], s_ps1[:, :])
                    if z_ps is not None:
                        nc.vector.tensor_copy(
                            s_sb[0:J, 2 * JD:2 * JD + 1], z_ps[:, :]
                        )
                    cc_in = dpool.tile([128, width], bf16, tag="cc_in")
                    nc.sync.dma_start(cc_in[:, :], s_sb[:, :])
                    cc_out = dpool.tile([128, width], bf16, tag="cc_out", name="ccout")
                    nc.gpsimd.collective_compute(
                        "AllReduce",
                        ALU.add,
                        replica_groups=[list(range(NCORES))],
                        ins=[cc_in.opt()],
                        outs=[cc_out.opt()],
                    )
                    return cc_out
                # final iteration: fp32, feeds the output directly
                s_sb = wpool.tile([128, 2 * (JD + 1)], f32, tag="s_sb3")
                nc.scalar.copy(s_sb[:, 0:JD], s_ps0[:, :])
                nc.vector.tensor_copy(s_sb[:, JD + 1:2 * JD + 1], s_ps1[:, :])
                for r in range(4):
                    nc.vector.tensor_copy(
                        s_sb[r * 32:r * 32 + J, JD:JD + 1], z_ps[:, :]
                    )
                    nc.vector.tensor_copy(
                        s_sb[r * 32:r * 32 + J, 2 * JD + 1:2 * JD + 2], z_ps[:, :]
                    )
                cc_in = dpool.tile([B, JD + 1], f32, tag="cc3_in")
                nc.sync.dma_start(
                    cc_in[:, :].rearrange("(c p) n -> p c n", p=128),
                    s_sb[:, :].rearrange("p (c n) -> p c n", n=JD + 1),
                )
                cc_out = dpool.tile([B // NCORES, JD + 1], f32, tag="cc3_out", name="ccout3")
                nc.gpsimd.collective_compute(
                    "ReduceScatter",
                    ALU.add,
                    replica_groups=[list(range(NCORES))],
                    ins=[cc_in.opt()],
                    outs=[cc_out.opt()],
                )
                return cc_out

            # PE warm-up: the cost model's p-state needs ~3us of continuous
            # matmul activity before full rate; burn it on zeros during the
            # input-load wait so the real matmuls start warm
            warm = cpool.tile([128, 256], bf16, name="warm")
            nc.vector.memset(warm[:, :], 0.0)
            # per-partition bias column for the folded-z0 squash constant
            zb0 = cpool.tile([128, 1], f32, name="zb0")
            nc.vector.memset(zb0[:, :], 2.0 * float(np.log(1.0 / I)))
            # warm_ps shares a PSUM bank with zbc/z (all short-lived, strictly
            # ordered through the tag's WAR chain)
            warm_ps = ppool.tile([128, 256], f32, tag="zbc_ps", name="warm_ps")
            for _ in range(6):
                nc.tensor.matmul(
                    warm_ps[:, :], warm[:, 0:128], warm[:, :],
                    start=True, stop=True,
                )

            # ---- iteration 0 front: b0 == 1 -> uniform softmax: plain
            # matmul on raw W, denominator is the constant 1152 ----
            s_ps = (
                ppool.tile([128, JD], f32, tag="s_ps0", name="s_ps0"),
                ppool.tile([128, JD], f32, tag="s_ps1", name="s_ps1"),
            )
            s_mms(s_ps, lambda g, t_: wsb[g][:, t_, :], True, True)
            # A-path / normalization loads go on the same (SP) queue as the
            # three critical (W | xt) groups: DMA arbitration is arrival
            # order, so another queue's DMA would cut ahead of group data
            nc.sync.dma_start(xb_sb[:, :], bf_d[:, XB0:BF_COLS])
            nc.sync.dma_start(f32_sb[:, :], f32_d[:, :])
            cc_out = stage_and_collect(s_ps, None, last=False)

            e_tiles = [None] * NG
            for it in range(2):
                last_cc = it == 1

                # ---- post-AllReduce squash -> v ----
                width = 2 * JD + (1 if it > 0 else 0)
                sgz = wpool.tile([128, width], bf16, tag="sgz")
                nc.sync.dma_start(sgz[:, :], cc_out[:, :])
                sg = sgz[:, 0:2 * JD]

                if it == 0:
                    # z0 = 1152 exactly (uniform softmax over in_size): fold
                    # it into the squash constants instead of scaling s --
                    # the squash then runs directly on the raw AllReduce sum
                    s_n = sg
                else:
                    s_n = wpool.tile([128, 2 * JD], bf16, tag="s_n")
                    # zinv at (d,j) columns on all 128 partitions: recip the
                    # z column, scale sel10 by it, lift via a (K=10) matmul
                    zinv = wpool.tile([J, 1], f32, tag="zinv")
                    nc.vector.reciprocal(zinv[:, :], sgz[0:J, 2 * JD:2 * JD + 1])
                    zsel = wpool.tile([J, JD], f32, tag="zsel")
                    nc.vector.tensor_scalar_mul(zsel[:, :], sel10[:, :], zinv[:, 0:1])
                    zbc_ps = ppool.tile([128, JD], f32, tag="zbc_ps")
                    nc.tensor.matmul(
                        zbc_ps[:, :], ones10[:, :], zsel[:, :], start=True, stop=True
                    )
                    nc.vector.tensor_tensor(
                        s_n[:, :].rearrange("p (c n) -> p c n", n=JD),
                        sg.rearrange("p (c n) -> p c n", n=JD),
                        zbc_ps[:, :].unsqueeze(1).broadcast_to([128, 2, JD]),
                        ALU.mult,
                    )

                # mag_sq[b, d] = sum_j s[b, (d,j)]^2 : square then innermost
                # reduce; F = sqrt(m)/(1+m) with the ACT (ln,exp) pair and
                # the DVE (1+m, recip) pair running in parallel off msq
                sq = wpool.tile([128, 2 * JD], bf16, tag="sq")
                nc.vector.tensor_mul(sq[:, :], s_n[:, :], s_n[:, :])
                msq = wpool.tile([128, 2 * D], f32, tag="msq")
                nc.vector.tensor_reduce(
                    msq[:, :].rearrange("p (c d) -> p c d", d=D),
                    sq[:, :].rearrange("p (c d j) -> p c d j", d=D, j=J),
                    axis=AX.X,
                    op=ALU.add,
                )
                # it==0 carries the constant z0=1152 inside the squash: with
                # c=1/z0, msq here is z0^2-scaled, so F_eff = c*F(c^2*msq) =
                # exp(0.5*ln(msq) + 2*ln(c)) / (1 + c^2*msq), and v = sg*F_eff
                lnm = wpool.tile([128, 2 * D], f32, tag="lnm")
                nc.scalar.activation(lnm[:, :], msq[:, :], AF.Ln)
                rt = wpool.tile([128, 2 * D], f32, tag="rt")
                nc.scalar.activation(
                    rt[:, :], lnm[:, :], AF.Exp, scale=0.5,
                    bias=(zb0[:, 0:1] if it == 0 else 0.0),
                )
                dn = wpool.tile([128, 2 * D], f32, tag="dn")
                if it == 0:
                    nc.vector.tensor_scalar(
                        dn[:, :], msq[:, :], 1.0 / (I * I), 1.0,
                        op0=ALU.mult, op1=ALU.add,
                    )
                else:
                    nc.vector.tensor_scalar_add(dn[:, :], msq[:, :], 1.0)
                rc = wpool.tile([128, 2 * D], f32, tag="rc")
                nc.vector.reciprocal(rc[:, :], dn[:, :])
                f_t = wpool.tile([128, 2 * D], f32, tag="f_t")
                nc.vector.tensor_mul(f_t[:, :], rt[:, :], rc[:, :])

                # v = s * F (F broadcast over j); v lands directly in the
                # (b, (d,j)) layout the A-matmul needs -- no transposes
                vt = wpool.tile([128, 2 * JD], bf16, tag="vt")
                for ch in range(2):
                    nc.vector.tensor_tensor(
                        vt[:, ch * JD:(ch + 1) * JD].rearrange("p (d j) -> p d j", j=J),
                        s_n[:, ch * JD:(ch + 1) * JD].rearrange("p (d j) -> p d j", j=J),
                        f_t[:, ch * D:(ch + 1) * D].unsqueeze(2).broadcast_to([128, D, J]),
                        ALU.mult,
                    )
                vb0 = vt[:, 0:JD]
                vb1 = vt[:, JD:2 * JD]

                # ---- fused per-group pipeline: A-path group g immediately
                # feeds that group's e-update, Wc gate and s-matmuls ----
                s_ps = (
                    ppool.tile([128, JD], f32, tag="s_ps0", name="s_ps0"),
                    ppool.tile([128, JD], f32, tag="s_ps1", name="s_ps1"),
                )
                z_ps = ppool.tile([J, 1], f32, tag="zbc_ps", name="z_ps")
                # A-path at k-tile granularity: per tile, PE matmul -> ACT
                # psum drain -> DVE gate -> DVE d-reduce, so the waves are
                # small and every engine streams; e/Wc/s-matmuls then fire
                # per 3-tile group.  Wc gating for the early groups runs on
                # the otherwise-idle GpSimd; the last group (on the serial
                # chain into the collective) stays on the faster DVE.
                r_g = [None] * NG
                for t_ in range(KT):
                    g, tt = t_ // 3, t_ % 3
                    a_ps = apool.tile([128, JD], f32, tag="a_ps")
                    nc.tensor.matmul(
                        a_ps[:, :],
                        xb0[:, t_ * 128:(t_ + 1) * 128], vb0,
                        start=True, stop=False,
                    )
                    nc.tensor.matmul(
                        a_ps[:, :],
                        xb1[:, t_ * 128:(t_ + 1) * 128], vb1,
                        start=False, stop=True,
                    )
                    a_sb = wpool3.tile([128, JD], bf16, tag="a_sb")
                    nc.scalar.copy(a_sb[:, :], a_ps[:, :])
                    p_t = wpool3.tile([128, JD], bf16, tag="p_t")
                    nc.vector.tensor_tensor(
                        p_t[:, :], wsb[g][:, tt, :], a_sb[:, :], ALU.mult,
                    )
                    if tt == 0:
                        r_g[g] = wpool3.tile(
                            [128, 3, J], f32, tag="r_t", name=f"r_g{g}"
                        )
                    nc.vector.tensor_reduce(
                        r_g[g][:, tt, :],
                        p_t.rearrange("p (d j) -> p j d", d=D, j=J),
                        axis=AX.X,
                        op=ALU.add,
                    )
                    if tt < 2:
                        continue
                    # ---- group complete: u-fold, e-update, z, Wc, s ----
                    uv_ps = ppool.tile([128, 3 * J], f32, tag="uv_ps")
                    for a in range(3):
                        nc.tensor.matmul(
                            uv_ps[:, a * J:(a + 1) * J], m8[:, :], r_g[g][:, a, :],
                            start=True, stop=True,
                        )
                    e_new = epool.tile([128, 3, J], bf16, tag=f"e{g}")
                    if it == 0:
                        # e1 = exp(1 + u_vj1): fused add+exp
                        nc.scalar.activation(
                            e_new[:, :, :],
                            uv_ps[:, :].rearrange("p (a j) -> p a j", j=J),
                            AF.Exp,
                            bias=1.0,
                        )
                    else:
                        expuv = wpool3.tile([128, 3, J], bf16, tag="expuv")
                        nc.scalar.activation(
                            expuv[:, :, :],
                            uv_ps[:, :].rearrange("p (a j) -> p a j", j=J),
                            AF.Exp,
                        )
                        nc.vector.tensor_tensor(
                            e_new[:, :, :], e_tiles[g][:, :, :], expuv[:, :, :],
                            ALU.mult,
                        )
                    e_tiles[g] = e_new
                # ---- tail: z, Wc gates and s-matmuls, emitted after the
                # whole per-ktile pipeline so the (in-order) PE stream never
                # stalls behind a slow gate mid-pipeline ----
                for g in range(NG):
                    wc = wpool3.tile([128, 3, JD], bf16, tag="wc")
                    (a_gate if g == 0 else nc.vector).tensor_tensor(
                        wc[:, :, :].rearrange("p t (d j) -> p t d j", j=J),
                        wsb[g][:, :, :].rearrange("p t (d j) -> p t d j", j=J),
                        e_tiles[g][:, :, :].unsqueeze(2).broadcast_to([128, 3, D, J]),
                        ALU.mult,
                    )
                    for a in range(3):
                        st = g == 0 and a == 0
                        sp = g == NG - 1 and a == 2
                        nc.tensor.matmul(
                            s_ps[0][:, :], xt[g][:, a, 0:128], wc[:, a, :],
                            start=st, stop=sp,
                        )
                        nc.tensor.matmul(
                            s_ps[1][:, :], xt[g][:, a, 128:B], wc[:, a, :],
                            start=st, stop=sp,
                        )
                    # z partial: z[j] = sum_i e[i,j] as a (J,1) column
                    for a in range(3):
                        nc.tensor.matmul(
                            z_ps[:, :], e_tiles[g][:, a, :], ones8[:, 0:1],
                            start=(g == 0 and a == 0),
                            stop=(g == NG - 1 and a == 2),
                        )
                cc_out = stage_and_collect(s_ps, z_ps, last=last_cc)

            # ---- post-ReduceScatter shard squash -> out ----
            sg3z = wpool.tile([32, JD + 1], f32, tag="sg3z")
            nc.sync.dma_start(sg3z[:, :], cc_out[0:32, 0:JD + 1])
            zinv3 = wpool.tile([J, 1], f32, tag="zinv3")
            nc.vector.reciprocal(zinv3[:, :], sg3z[0:J, JD:JD + 1])
            zsel3 = wpool.tile([J, JD], f32, tag="zsel3")
            nc.vector.tensor_scalar_mul(zsel3[:, :], sel10[:, :], zinv3[:, 0:1])
            zbc3 = ppool.tile([32, JD], f32, tag="zbc_ps", name="zbc3")
            nc.tensor.matmul(zbc3[:, :], ones10[:, 0:32], zsel3[:, :], start=True, stop=True)
            sn3 = wpool.tile([32, JD], f32, tag="sn3")
            nc.vector.tensor_mul(sn3[:, :], sg3z[0:32, 0:JD], zbc3[:, :])
            sq3 = wpool.tile([32, JD], bf16, tag="sq3")
            nc.vector.tensor_mul(sq3[:, :], sn3[:, :], sn3[:, :])
            msq3 = wpool.tile([32, D], f32, tag="msq3")
            nc.vector.tensor_reduce(
                msq3[:, :],
                sq3[:, :].rearrange("p (d j) -> p d j", j=J),
                axis=AX.X,
                op=ALU.add,
            )
            ln3 = wpool.tile([32, D], f32, tag="ln3")
            nc.scalar.activation(ln3[:, :], msq3[:, :], AF.Ln)
            rt3 = wpool.tile([32, D], f32, tag="rt3")
            nc.scalar.activation(rt3[:, :], ln3[:, :], AF.Exp, scale=0.5)
            dn3 = wpool.tile([32, D], f32, tag="dn3")
            nc.vector.tensor_scalar_add(dn3[:, :], msq3[:, :], 1.0)
            rc3 = wpool.tile([32, D], f32, tag="rc3")
            nc.vector.reciprocal(rc3[:, :], dn3[:, :])
            f3 = wpool.tile([32, D], f32, tag="f3")
            nc.vector.tensor_mul(f3[:, :], rt3[:, :], rc3[:, :])
            v3 = wpool.tile([32, JD], f32, tag="v3")
            nc.vector.tensor_tensor(
                v3[:, :].rearrange("p (d j) -> p d j", j=J),
                sn3[:, :].rearrange("p (d j) -> p d j", j=J),
                f3[:, :].unsqueeze(2).broadcast_to([32, D, J]),
                ALU.mult,
            )
            nc.sync.dma_start(out_d[:, :], v3[:, :])

    nc.finalize()
    return nc


def _f32_blob():
    blob = np.zeros((128, F32_COLS), np.float32)
    blob[:, 0:128] = np.kron(np.eye(16, dtype=np.float32), np.ones((8, 8), np.float32)) / B
    blob[0:J, 128:256] = 1.0
    blob[0:J, 256:256 + JD] = np.tile(np.eye(J, dtype=np.float32), (1, D))
    return blob


def _prep_in_maps(x, W):
    x = np.asarray(x, np.float32)
    W = np.asarray(W, np.float32)
    Wm = W[0]
    f32_blob = _f32_blob()
    in_maps = []
    for c in range(NCORES):
        sl = slice(c * IL, (c + 1) * IL)
        xs = x[:, :, sl]                                            # (B, U, IL)
        xt = np.ascontiguousarray(xs.transpose(2, 1, 0).reshape(KL, B))
        xb = xt.T
        w = Wm[sl].transpose(0, 3, 2, 1).reshape(KL, JD)            # cols = (d, j)
        wt = w.reshape(KT, 128, JD).transpose(1, 0, 2)              # (128, KT, JD)
        xtt = xt.reshape(KT, 128, B).transpose(1, 0, 2)             # (128, KT, B)
        bf = np.zeros((128, BF_COLS), np.float32)
        o = 0
        for g, gt in enumerate(GROUPS):
            g0 = GOFF[g]
            bf[:, o:o + gt * JD] = wt[:, g0:g0 + gt].reshape(128, gt * JD)
            o += gt * JD
            bf[:, o:o + gt * B] = xtt[:, g0:g0 + gt].reshape(128, gt * B)
            o += gt * B
        bf[:, o:o + KL] = xb[0:128]; o += KL
        bf[:, o:o + KL] = xb[128:256]; o += KL
        bf[:, o] = 0.125; o += 1
        assert o == BF_COLS
        in_maps.append({
            "bfin": bf.astype(ml_dtypes.bfloat16),
            "f32in": f32_blob,
        })
    return in_maps


def run(x, W, trace=False):
    from concourse.bass_utils import run_bass_kernel_spmd

    if "nc" not in _CACHE:
        _CACHE["nc"] = _build_module()
    nc = _CACHE["nc"]
    in_maps = _prep_in_maps(x, W)
    res = run_bass_kernel_spmd(
        nc, in_maps, core_ids=list(range(NCORES)), trace=trace
    )
    v = np.concatenate(
        [np.asarray(res.results[c]["out"], np.float32) for c in range(NCORES)],
        axis=0,
    )                                                               # (B, (d,j))
    out = v.reshape(B, D, J).transpose(0, 2, 1)[..., None]
    return np.ascontiguousarray(out.astype(np.float32)), res


def kernel(x, W):
    out, _ = run(x, W, trace=False)
    return out


# revision 39
# speedup vs baseline: 1.0685x; 1.0292x over previous
"""CapsuleLayer dynamic-routing kernel for 8 TRN2 NeuronCores.

Sharding: in_size (i) is split 8 ways (144 rows/core); every core holds the
full batch.  u_hat (B,1152,10,16 = 189MB) is never materialized: both the
c-weighted sum (s_j) and the agreement update factor through x and W:

    s_un[b, (d,j)]   = sum_{(i,u)} x[b,u,i] * (e[i,j] * W[i,j,d,u])
    A[(i,u), (d,j)]  = sum_b x[b,u,i] * v[b,j,d]
    u_vj1[i,j]       = (1/B) sum_{u,d} W[i,j,d,u] * A[(i,u),(d,j)]

with e unnormalized; the softmax denominator z_j = sum_i e[i,j] rides inside
the per-iteration collective (the only cross-core traffic): AllReduce for
routing iterations 1-2, ReduceScatter for the final one (each core then
squashes and emits only its own 32-batch output shard, gathered host-side).

Structural choices vs the straightforward version:
  * iteration 0 has b=1 (uniform softmax), so s0 is a plain matmul on raw W
    with a compile-time softmax denominator z0=1152 -- no exp, no gating, no
    z column in the first collective;
  * b_ij is never materialized: e is tracked multiplicatively,
    e_{k+1} = e_k * exp(u_vj1), with iteration 1's e = exp(1 + u_vj1)
    produced by a single fused activation (Exp with bias=1);
  * inputs stream in three (W,x) k-groups so the first matmuls start while
    the rest of the load is in flight;
  * the agreement block is pipelined per k-group across four engines
    (PE matmul -> ACT drain -> gate -> DVE d-reduce -> PE u-fold -> ACT exp
    -> DVE gate -> PE s-matmul);
  * collective payloads are [128, W]-shaped so DMAs move 128 fat descriptors
    instead of 256 thin ones.
All matmuls run in bf16 with fp32 PSUM accumulation; exp/ln/copy live in one
ACT function table so only one table load is ever issued.
"""

import os
import sys

import numpy as np

for _p in ("/opt/trn_rl_repo",):
    if _p not in sys.path and os.path.isdir(_p):
        sys.path.insert(0, _p)

import ml_dtypes

NCORES = 8
B, U, I = 256, 8, 1152
J, D = 10, 16
IL = I // NCORES        # 144 in_size rows per core
KL = IL * U             # 1152 local contraction length (i,u)
KT = KL // 128          # 9 partition tiles
GROUPS = [3, 3, 3]      # k-tile groups for loads / e / Wc / s-matmuls
GOFF = [0, 3, 6]        # cumulative k-tile offsets
NG = len(GROUPS)
JD = J * D              # 160
BF_COLS = KT * (JD + B) + 2 * KL + 1                # groups | xb0 | xb1 | ones8
F32_COLS = 256 + JD                                 # m8 | ones10 | sel10

_CACHE = {}


def _build_module(a_gate_pool=True):
    import concourse.bacc as bacc
    import concourse.mybir as mybir
    import concourse.tile as tile

    f32 = mybir.dt.float32
    bf16 = mybir.dt.bfloat16
    fp8 = mybir.dt.float8e4
    CSC = 1.0 / 16.0            # fp8 pre-scale for the AllReduce payload
    CC0 = 1.0 / (I * CSC)       # iteration-0 softmax const with CSC folded
    AF = mybir.ActivationFunctionType
    ALU = mybir.AluOpType
    AX = mybir.AxisListType

    # Force the act-table pass's first-match lookup to land every function
    # we use (Exp, Ln, Copy) on the one table that covers them all, so only
    # a single LoadActFuncSet is ever emitted.  Table *ids* are positional,
    # so we only hide functions from other tables, never reorder.
    if not hasattr(bacc, "_orig_get_activation_tables"):
        bacc._orig_get_activation_tables = bacc.get_activation_tables

        def _patched_tables(arch):
            tabs = bacc._orig_get_activation_tables(arch)
            AF_ = mybir.ActivationFunctionType
            ours = {AF_.Exp, AF_.Ln, AF_.Copy, AF_.Square, AF_.Identity}
            out = {}
            for name, s in tabs.items():
                if name == "natural_log_exp_and_others":
                    out[name] = s
                else:
                    out[name] = s - ours
            return out

        bacc.get_activation_tables = _patched_tables

    nc = bacc.Bacc(
        "TRN2", target_bir_lowering=False, debug=False, num_devices=NCORES
    )

    bf_d = nc.declare_dram_parameter("bfin", [128, BF_COLS], bf16, isOutput=False)
    f32_d = nc.declare_dram_parameter("f32in", [128, F32_COLS], f32, isOutput=False)
    out_d = nc.declare_dram_parameter("out", [B // NCORES, JD], f32, isOutput=True)

    a_gate = None  # set below

    with tile.TileContext(nc) as tc:
        with (
            tc.tile_pool(name="const", bufs=1) as cpool,
            tc.tile_pool(name="work", bufs=2) as wpool,
            tc.tile_pool(name="psum", bufs=1, space="PSUM") as ppool,
            tc.tile_pool(name="apsum", bufs=3, space="PSUM") as apool,
            tc.tile_pool(name="work3", bufs=3) as wpool3,
            tc.tile_pool(name="ework", bufs=2) as epool,
            tc.tile_pool(name="dram", bufs=3, space="DRAM") as dpool,
        ):
            a_gate = nc.gpsimd if a_gate_pool else nc.vector

            # ---- streamed loads: 3 (W | xt) k-groups so matmuls start
            # early, then the A-path / normalization constants ----
            wsb, xt = [], []
            off = 0
            for g, gt in enumerate(GROUPS):
                grp = gt * (JD + B)
                t_ = cpool.tile([128, grp], bf16, tag=f"grp{g}", name=f"grp{g}")
                nc.sync.dma_start(t_[:, :], bf_d[:, off:off + grp])
                off += grp
                wsb.append(t_[:, 0:gt * JD].rearrange("p (t n) -> p t n", n=JD))
                xt.append(t_[:, gt * JD:grp].rearrange("p (t b) -> p t b", b=B))
            XB0 = off
            xb_sb = cpool.tile([128, 2 * KL + 1], bf16)
            xb0 = xb_sb[:, 0:KL]
            xb1 = xb_sb[:, KL:2 * KL]
            ones8 = xb_sb[:, 2 * KL:2 * KL + 1]
            f32_sb = cpool.tile([128, F32_COLS], f32)
            m8 = f32_sb[:, 0:128]
            ones10 = f32_sb[0:J, 128:256]      # (10, 128) of ones
            sel10 = f32_sb[0:J, 256:256 + JD]  # sel10[j', d*J+j] = (j==j')

            def s_mms(s_ps, rhs_of, first, last):
                # the two b-halves live in separate PSUM banks: a start=True
                # matmul clears its bank, so interleaved accumulation groups
                # must not share one
                s_ps0, s_ps1 = s_ps
                for g, gt in enumerate(GROUPS):
                    for t_ in range(gt):
                        st = first and g == 0 and t_ == 0
                        sp = last and g == NG - 1 and t_ == gt - 1
                        nc.tensor.matmul(
                            s_ps0[:, :], xt[g][:, t_, 0:128], rhs_of(g, t_),
                            start=st, stop=sp,
                        )
                        nc.tensor.matmul(
                            s_ps1[:, :], xt[g][:, t_, 128:B], rhs_of(g, t_),
                            start=st, stop=sp,
                        )

            def stage_and_collect(s_ps, z_ps, last):
                # stage [s | z] in SBUF; PSUM itself is not DMA-readable.
                # Payload stays [128, W]-shaped (fat rows -> 128 descriptors)
                # for the AllReduce iterations; the final ReduceScatter needs
                # batch on the outer axis so each core receives its 32-row
                # output shard.
                s_ps0, s_ps1 = s_ps
                if not last:
                    # fp8 payload (half the AllReduce bytes): s_un has sigma
                    # ~260 vs e4m3 max 448, so pre-scale by 1/16 on the way
                    # out; the 16 cancels exactly against z (which rides at
                    # z/16) or folds into the iteration-0 squash constants
                    width = 2 * JD + (1 if z_ps is not None else 0)
                    s_sb = wpool.tile([128, width], fp8, tag="s_sb")
                    nc.scalar.mul(s_sb[:, 0:JD], s_ps0[:, :], CSC)
                    nc.vector.tensor_scalar_mul(
                        s_sb[:, JD:2 * JD], s_ps1[:, :], CSC
                    )
                    if z_ps is not None:
                        nc.vector.tensor_scalar_mul(
                            s_sb[0:J, 2 * JD:2 * JD + 1], z_ps[:, :], CSC
                        )
                    cc_in = dpool.tile([128, width], fp8, tag="cc_in")
                    nc.sync.dma_start(cc_in[:, :], s_sb[:, :])
                    cc_out = dpool.tile([128, width], fp8, tag="cc_out", name="ccout")
                    nc.gpsimd.collective_compute(
                        "AllReduce",
                        ALU.add,
                        replica_groups=[list(range(NCORES))],
                        ins=[cc_in.opt()],
                        outs=[cc_out.opt()],
                    )
                    return cc_out
                # final iteration: fp32, feeds the output directly
                s_sb = wpool.tile([128, 2 * (JD + 1)], f32, tag="s_sb3")
                nc.scalar.copy(s_sb[:, 0:JD], s_ps0[:, :])
                nc.vector.tensor_copy(s_sb[:, JD + 1:2 * JD + 1], s_ps1[:, :])
                for r in range(4):
                    nc.vector.tensor_copy(
                        s_sb[r * 32:r * 32 + J, JD:JD + 1], z_ps[:, :]
                    )
                    nc.vector.tensor_copy(
                        s_sb[r * 32:r * 32 + J, 2 * JD + 1:2 * JD + 2], z_ps[:, :]
                    )
                cc_in = dpool.tile([B, JD + 1], f32, tag="cc3_in")
                nc.sync.dma_start(
                    cc_in[:, :].rearrange("(c p) n -> p c n", p=128),
                    s_sb[:, :].rearrange("p (c n) -> p c n", n=JD + 1),
                )
                cc_out = dpool.tile([B // NCORES, JD + 1], f32, tag="cc3_out", name="ccout3")
                nc.gpsimd.collective_compute(
                    "ReduceScatter",
                    ALU.add,
                    replica_groups=[list(range(NCORES))],
                    ins=[cc_in.opt()],
                    outs=[cc_out.opt()],
                )
                return cc_out

            # PE warm-up: the cost model's p-state needs ~3us of continuous
            # matmul activity before full rate; burn it on zeros during the
            # input-load wait so the real matmuls start warm
            warm = cpool.tile([128, 256], bf16, name="warm")
            nc.vector.memset(warm[:, :], 0.0)
            # per-partition bias column for the folded-z0 squash constant
            zb0 = cpool.tile([128, 1], f32, name="zb0")
            nc.vector.memset(zb0[:, :], 2.0 * float(np.log(CC0)))
            # warm_ps shares a PSUM bank with zbc/z (all short-lived, strictly
            # ordered through the tag's WAR chain)
            warm_ps = ppool.tile([128, 256], f32, tag="zbc_ps", name="warm_ps")
            for _ in range(6):
                nc.tensor.matmul(
                    warm_ps[:, :], warm[:, 0:128], warm[:, :],
                    start=True, stop=True,
                )

            # ---- iteration 0 front: b0 == 1 -> uniform softmax: plain
            # matmul on raw W, denominator is the constant 1152 ----
            s_ps = (
                ppool.tile([128, JD], f32, tag="s_ps0", name="s_ps0"),
                ppool.tile([128, JD], f32, tag="s_ps1", name="s_ps1"),
            )
            s_mms(s_ps, lambda g, t_: wsb[g][:, t_, :], True, True)
            # A-path / normalization loads go on the same (SP) queue as the
            # three critical (W | xt) groups: DMA arbitration is arrival
            # order, so another queue's DMA would cut ahead of group data
            nc.sync.dma_start(xb_sb[:, :], bf_d[:, XB0:BF_COLS])
            nc.sync.dma_start(f32_sb[:, :], f32_d[:, :])
            cc_out = stage_and_collect(s_ps, None, last=False)

            e_tiles = [None] * NG
            for it in range(2):
                last_cc = it == 1

                # ---- post-AllReduce squash -> v ----
                width = 2 * JD + (1 if it > 0 else 0)
                sgz = wpool.tile([128, width], fp8, tag="sgz")
                nc.sync.dma_start(sgz[:, :], cc_out[:, :])
                sg = sgz[:, 0:2 * JD]

                if it == 0:
                    # z0 = 1152 exactly (uniform softmax over in_size): fold
                    # it into the squash constants instead of scaling s --
                    # the squash then runs directly on the raw AllReduce sum
                    s_n = sg
                else:
                    s_n = wpool.tile([128, 2 * JD], bf16, tag="s_n")
                    # zinv at (d,j) columns on all 128 partitions: recip the
                    # z column, scale sel10 by it, lift via a (K=10) matmul
                    zinv = wpool.tile([J, 1], f32, tag="zinv")
                    nc.vector.reciprocal(zinv[:, :], sgz[0:J, 2 * JD:2 * JD + 1])
                    zsel = wpool.tile([J, JD], f32, tag="zsel")
                    nc.vector.tensor_scalar_mul(zsel[:, :], sel10[:, :], zinv[:, 0:1])
                    zbc_ps = ppool.tile([128, JD], f32, tag="zbc_ps")
                    nc.tensor.matmul(
                        zbc_ps[:, :], ones10[:, :], zsel[:, :], start=True, stop=True
                    )
                    nc.vector.tensor_tensor(
                        s_n[:, :].rearrange("p (c n) -> p c n", n=JD),
                        sg.rearrange("p (c n) -> p c n", n=JD),
                        zbc_ps[:, :].unsqueeze(1).broadcast_to([128, 2, JD]),
                        ALU.mult,
                    )

                # mag_sq[b, d] = sum_j s[b, (d,j)]^2 : square then innermost
                # reduce; F = sqrt(m)/(1+m) with the ACT (ln,exp) pair and
                # the DVE (1+m, recip) pair running in parallel off msq
                sq = wpool.tile([128, 2 * JD], bf16, tag="sq")
                nc.vector.tensor_mul(sq[:, :], s_n[:, :], s_n[:, :])
                msq = wpool.tile([128, 2 * D], f32, tag="msq")
                nc.vector.tensor_reduce(
                    msq[:, :].rearrange("p (c d) -> p c d", d=D),
                    sq[:, :].rearrange("p (c d j) -> p c d j", d=D, j=J),
                    axis=AX.X,
                    op=ALU.add,
                )
                # it==0 carries the constant z0=1152 inside the squash: with
                # c=1/z0, msq here is z0^2-scaled, so F_eff = c*F(c^2*msq) =
                # exp(0.5*ln(msq) + 2*ln(c)) / (1 + c^2*msq), and v = sg*F_eff
                lnm = wpool.tile([128, 2 * D], f32, tag="lnm")
                nc.scalar.activation(lnm[:, :], msq[:, :], AF.Ln)
                rt = wpool.tile([128, 2 * D], f32, tag="rt")
                nc.scalar.activation(
                    rt[:, :], lnm[:, :], AF.Exp, scale=0.5,
                    bias=(zb0[:, 0:1] if it == 0 else 0.0),
                )
                dn = wpool.tile([128, 2 * D], f32, tag="dn")
                if it == 0:
                    nc.vector.tensor_scalar(
                        dn[:, :], msq[:, :], CC0 * CC0, 1.0,
                        op0=ALU.mult, op1=ALU.add,
                    )
                else:
                    nc.vector.tensor_scalar_add(dn[:, :], msq[:, :], 1.0)
                rc = wpool.tile([128, 2 * D], f32, tag="rc")
                nc.vector.reciprocal(rc[:, :], dn[:, :])
                f_t = wpool.tile([128, 2 * D], f32, tag="f_t")
                nc.vector.tensor_mul(f_t[:, :], rt[:, :], rc[:, :])

                # v = s * F (F broadcast over j); v lands directly in the
                # (b, (d,j)) layout the A-matmul needs -- no transposes
                vt = wpool.tile([128, 2 * JD], bf16, tag="vt")
                for ch in range(2):
                    nc.vector.tensor_tensor(
                        vt[:, ch * JD:(ch + 1) * JD].rearrange("p (d j) -> p d j", j=J),
                        s_n[:, ch * JD:(ch + 1) * JD].rearrange("p (d j) -> p d j", j=J),
                        f_t[:, ch * D:(ch + 1) * D].unsqueeze(2).broadcast_to([128, D, J]),
                        ALU.mult,
                    )
                vb0 = vt[:, 0:JD]
                vb1 = vt[:, JD:2 * JD]

                # ---- fused per-group pipeline: A-path group g immediately
                # feeds that group's e-update, Wc gate and s-matmuls ----
                s_ps = (
                    ppool.tile([128, JD], f32, tag="s_ps0", name="s_ps0"),
                    ppool.tile([128, JD], f32, tag="s_ps1", name="s_ps1"),
                )
                z_ps = ppool.tile([J, 1], f32, tag="zbc_ps", name="z_ps")
                # A-path at k-tile granularity: per tile, PE matmul -> ACT
                # psum drain -> DVE gate -> DVE d-reduce, so the waves are
                # small and every engine streams; e/Wc/s-matmuls then fire
                # per 3-tile group.  Wc gating for the early groups runs on
                # the otherwise-idle GpSimd; the last group (on the serial
                # chain into the collective) stays on the faster DVE.
                r_g = [None] * NG
                for t_ in range(KT):
                    g, tt = t_ // 3, t_ % 3
                    a_ps = apool.tile([128, JD], f32, tag="a_ps")
                    nc.tensor.matmul(
                        a_ps[:, :],
                        xb0[:, t_ * 128:(t_ + 1) * 128], vb0,
                        start=True, stop=False,
                    )
                    nc.tensor.matmul(
                        a_ps[:, :],
                        xb1[:, t_ * 128:(t_ + 1) * 128], vb1,
                        start=False, stop=True,
                    )
                    a_sb = wpool3.tile([128, JD], bf16, tag="a_sb")
                    nc.scalar.copy(a_sb[:, :], a_ps[:, :])
                    p_t = wpool3.tile([128, JD], bf16, tag="p_t")
                    nc.vector.tensor_tensor(
                        p_t[:, :], wsb[g][:, tt, :], a_sb[:, :], ALU.mult,
                    )
                    if tt == 0:
                        r_g[g] = wpool3.tile(
                            [128, 3, J], f32, tag="r_t", name=f"r_g{g}"
                        )
                    nc.vector.tensor_reduce(
                        r_g[g][:, tt, :],
                        p_t.rearrange("p (d j) -> p j d", d=D, j=J),
                        axis=AX.X,
                        op=ALU.add,
                    )
                    if tt < 2:
                        continue
                    # ---- group complete: u-fold, e-update, z, Wc, s ----
                    uv_ps = ppool.tile([128, 3 * J], f32, tag="uv_ps")
                    for a in range(3):
                        nc.tensor.matmul(
                            uv_ps[:, a * J:(a + 1) * J], m8[:, :], r_g[g][:, a, :],
                            start=True, stop=True,
                        )
                    e_new = epool.tile([128, 3, J], bf16, tag=f"e{g}")
                    if it == 0:
                        # e1 = exp(1 + u_vj1): fused add+exp
                        nc.scalar.activation(
                            e_new[:, :, :],
                            uv_ps[:, :].rearrange("p (a j) -> p a j", j=J),
                            AF.Exp,
                            bias=1.0,
                        )
                    else:
                        expuv = wpool3.tile([128, 3, J], bf16, tag="expuv")
                        nc.scalar.activation(
                            expuv[:, :, :],
                            uv_ps[:, :].rearrange("p (a j) -> p a j", j=J),
                            AF.Exp,
                        )
                        nc.vector.tensor_tensor(
                            e_new[:, :, :], e_tiles[g][:, :, :], expuv[:, :, :],
                            ALU.mult,
                        )
                    e_tiles[g] = e_new
                # ---- tail: z, Wc gates and s-matmuls, emitted after the
                # whole per-ktile pipeline so the (in-order) PE stream never
                # stalls behind a slow gate mid-pipeline ----
                for g in range(NG):
                    wc = wpool3.tile([128, 3, JD], bf16, tag="wc")
                    (a_gate if g == 0 else nc.vector).tensor_tensor(
                        wc[:, :, :].rearrange("p t (d j) -> p t d j", j=J),
                        wsb[g][:, :, :].rearrange("p t (d j) -> p t d j", j=J),
                        e_tiles[g][:, :, :].unsqueeze(2).broadcast_to([128, 3, D, J]),
                        ALU.mult,
                    )
                    for a in range(3):
                        st = g == 0 and a == 0
                        sp = g == NG - 1 and a == 2
                        nc.tensor.matmul(
                            s_ps[0][:, :], xt[g][:, a, 0:128], wc[:, a, :],
                            start=st, stop=sp,
                        )
                        nc.tensor.matmul(
                            s_ps[1][:, :], xt[g][:, a, 128:B], wc[:, a, :],
                            start=st, stop=sp,
                        )
                    # z partial: z[j] = sum_i e[i,j] as a (J,1) column
                    for a in range(3):
                        nc.tensor.matmul(
                            z_ps[:, :], e_tiles[g][:, a, :], ones8[:, 0:1],
                            start=(g == 0 and a == 0),
                            stop=(g == NG - 1 and a == 2),
                        )
                cc_out = stage_and_collect(s_ps, z_ps, last=last_cc)

            # ---- post-ReduceScatter shard squash -> out ----
            sg3z = wpool.tile([32, JD + 1], f32, tag="sg3z")
            nc.sync.dma_start(sg3z[:, :], cc_out[0:32, 0:JD + 1])
            zinv3 = wpool.tile([J, 1], f32, tag="zinv3")
            nc.vector.reciprocal(zinv3[:, :], sg3z[0:J, JD:JD + 1])
            zsel3 = wpool.tile([J, JD], f32, tag="zsel3")
            nc.vector.tensor_scalar_mul(zsel3[:, :], sel10[:, :], zinv3[:, 0:1])
            zbc3 = ppool.tile([32, JD], f32, tag="zbc_ps", name="zbc3")
            nc.tensor.matmul(zbc3[:, :], ones10[:, 0:32], zsel3[:, :], start=True, stop=True)
            sn3 = wpool.tile([32, JD], f32, tag="sn3")
            nc.vector.tensor_mul(sn3[:, :], sg3z[0:32, 0:JD], zbc3[:, :])
            sq3 = wpool.tile([32, JD], bf16, tag="sq3")
            nc.vector.tensor_mul(sq3[:, :], sn3[:, :], sn3[:, :])
            msq3 = wpool.tile([32, D], f32, tag="msq3")
            nc.vector.tensor_reduce(
                msq3[:, :],
                sq3[:, :].rearrange("p (d j) -> p d j", j=J),
                axis=AX.X,
                op=ALU.add,
            )
            ln3 = wpool.tile([32, D], f32, tag="ln3")
            nc.scalar.activation(ln3[:, :], msq3[:, :], AF.Ln)
            rt3 = wpool.tile([32, D], f32, tag="rt3")
            nc.scalar.activation(rt3[:, :], ln3[:, :], AF.Exp, scale=0.5)
            dn3 = wpool.tile([32, D], f32, tag="dn3")
            nc.vector.tensor_scalar_add(dn3[:, :], msq3[:, :], 1.0)
            rc3 = wpool.tile([32, D], f32, tag="rc3")
            nc.vector.reciprocal(rc3[:, :], dn3[:, :])
            f3 = wpool.tile([32, D], f32, tag="f3")
            nc.vector.tensor_mul(f3[:, :], rt3[:, :], rc3[:, :])
            v3 = wpool.tile([32, JD], f32, tag="v3")
            nc.vector.tensor_tensor(
                v3[:, :].rearrange("p (d j) -> p d j", j=J),
                sn3[:, :].rearrange("p (d j) -> p d j", j=J),
                f3[:, :].unsqueeze(2).broadcast_to([32, D, J]),
                ALU.mult,
            )
            nc.sync.dma_start(out_d[:, :], v3[:, :])

    nc.finalize()
    return nc


def _f32_blob():
    blob = np.zeros((128, F32_COLS), np.float32)
    blob[:, 0:128] = np.kron(np.eye(16, dtype=np.float32), np.ones((8, 8), np.float32)) / B
    blob[0:J, 128:256] = 1.0
    blob[0:J, 256:256 + JD] = np.tile(np.eye(J, dtype=np.float32), (1, D))
    return blob


def _prep_in_maps(x, W):
    x = np.asarray(x, np.float32)
    W = np.asarray(W, np.float32)
    Wm = W[0]
    f32_blob = _f32_blob()
    in_maps = []
    for c in range(NCORES):
        sl = slice(c * IL, (c + 1) * IL)
        xs = x[:, :, sl]                                            # (B, U, IL)
        xt = np.ascontiguousarray(xs.transpose(2, 1, 0).reshape(KL, B))
        xb = xt.T
        w = Wm[sl].transpose(0, 3, 2, 1).reshape(KL, JD)            # cols = (d, j)
        wt = w.reshape(KT, 128, JD).transpose(1, 0, 2)              # (128, KT, JD)
        xtt = xt.reshape(KT, 128, B).transpose(1, 0, 2)             # (128, KT, B)
        bf = np.zeros((128, BF_COLS), np.float32)
        o = 0
        for g, gt in enumerate(GROUPS):
            g0 = GOFF[g]
            bf[:, o:o + gt * JD] = wt[:, g0:g0 + gt].reshape(128, gt * JD)
            o += gt * JD
            bf[:, o:o + gt * B] = xtt[:, g0:g0 + gt].reshape(128, gt * B)
            o += gt * B
        bf[:, o:o + KL] = xb[0:128]; o += KL
        bf[:, o:o + KL] = xb[128:256]; o += KL
        bf[:, o] = 0.125; o += 1
        assert o == BF_COLS
        in_maps.append({
            "bfin": bf.astype(ml_dtypes.bfloat16),
            "f32in": f32_blob,
        })
    return in_maps


def run(x, W, trace=False):
    from concourse.bass_utils import run_bass_kernel_spmd

    if "nc" not in _CACHE:
        _CACHE["nc"] = _build_module()
    nc = _CACHE["nc"]
    in_maps = _prep_in_maps(x, W)
    res = run_bass_kernel_spmd(
        nc, in_maps, core_ids=list(range(NCORES)), trace=trace
    )
    v = np.concatenate(
        [np.asarray(res.results[c]["out"], np.float32) for c in range(NCORES)],
        axis=0,
    )                                                               # (B, (d,j))
    out = v.reshape(B, D, J).transpose(0, 2, 1)[..., None]
    return np.ascontiguousarray(out.astype(np.float32)), res


def kernel(x, W):
    out, _ = run(x, W, trace=False)
    return out


# revision 41
# speedup vs baseline: 1.0707x; 1.0020x over previous
"""CapsuleLayer dynamic-routing kernel for 8 TRN2 NeuronCores.

Sharding: in_size (i) is split 8 ways (144 rows/core); every core holds the
full batch.  u_hat (B,1152,10,16 = 189MB) is never materialized: both the
c-weighted sum (s_j) and the agreement update factor through x and W:

    s_un[b, (d,j)]   = sum_{(i,u)} x[b,u,i] * (e[i,j] * W[i,j,d,u])
    A[(i,u), (d,j)]  = sum_b x[b,u,i] * v[b,j,d]
    u_vj1[i,j]       = (1/B) sum_{u,d} W[i,j,d,u] * A[(i,u),(d,j)]

with e unnormalized; the softmax denominator z_j = sum_i e[i,j] rides inside
the per-iteration collective (the only cross-core traffic): AllReduce for
routing iterations 1-2, ReduceScatter for the final one (each core then
squashes and emits only its own 32-batch output shard, gathered host-side).

Structural choices vs the straightforward version:
  * iteration 0 has b=1 (uniform softmax), so s0 is a plain matmul on raw W
    with a compile-time softmax denominator z0=1152 -- no exp, no gating, no
    z column in the first collective;
  * b_ij is never materialized: e is tracked multiplicatively,
    e_{k+1} = e_k * exp(u_vj1), with iteration 1's e = exp(1 + u_vj1)
    produced by a single fused activation (Exp with bias=1);
  * inputs stream in three (W,x) k-groups so the first matmuls start while
    the rest of the load is in flight;
  * the agreement block is pipelined per k-group across four engines
    (PE matmul -> ACT drain -> gate -> DVE d-reduce -> PE u-fold -> ACT exp
    -> DVE gate -> PE s-matmul);
  * collective payloads are [128, W]-shaped so DMAs move 128 fat descriptors
    instead of 256 thin ones.
All matmuls run in bf16 with fp32 PSUM accumulation; exp/ln/copy live in one
ACT function table so only one table load is ever issued.
"""

import os
import sys

import numpy as np

for _p in ("/opt/trn_rl_repo",):
    if _p not in sys.path and os.path.isdir(_p):
        sys.path.insert(0, _p)

import ml_dtypes

NCORES = 8
B, U, I = 256, 8, 1152
J, D = 10, 16
IL = I // NCORES        # 144 in_size rows per core
KL = IL * U             # 1152 local contraction length (i,u)
KT = KL // 128          # 9 partition tiles
GROUPS = [3, 3, 3]      # k-tile groups for loads / e / Wc / s-matmuls
GOFF = [0, 3, 6]        # cumulative k-tile offsets
NG = len(GROUPS)
JD = J * D              # 160
BF_COLS = KT * (JD + B) + 2 * KL + 1                # groups | xb0 | xb1 | ones8
F32_COLS = 256 + JD                                 # m8 | ones10 | sel10

_CACHE = {}


def _build_module(a_gate_pool=True):
    import concourse.bacc as bacc
    import concourse.mybir as mybir
    import concourse.tile as tile

    f32 = mybir.dt.float32
    bf16 = mybir.dt.bfloat16
    fp8 = mybir.dt.float8e4
    CSC = 1.0 / 16.0            # fp8 pre-scale for the AllReduce payload
    CC0 = 1.0 / (I * CSC)       # iteration-0 softmax const with CSC folded
    AF = mybir.ActivationFunctionType
    ALU = mybir.AluOpType
    AX = mybir.AxisListType

    # Force the act-table pass's first-match lookup to land every function
    # we use (Exp, Ln, Copy) on the one table that covers them all, so only
    # a single LoadActFuncSet is ever emitted.  Table *ids* are positional,
    # so we only hide functions from other tables, never reorder.
    if not hasattr(bacc, "_orig_get_activation_tables"):
        bacc._orig_get_activation_tables = bacc.get_activation_tables

        def _patched_tables(arch):
            tabs = bacc._orig_get_activation_tables(arch)
            AF_ = mybir.ActivationFunctionType
            ours = {AF_.Exp, AF_.Ln, AF_.Copy, AF_.Square, AF_.Identity}
            out = {}
            for name, s in tabs.items():
                if name == "natural_log_exp_and_others":
                    out[name] = s
                else:
                    out[name] = s - ours
            return out

        bacc.get_activation_tables = _patched_tables

    nc = bacc.Bacc(
        "TRN2", target_bir_lowering=False, debug=False, num_devices=NCORES
    )

    bf_d = nc.declare_dram_parameter("bfin", [128, BF_COLS], bf16, isOutput=False)
    f32_d = nc.declare_dram_parameter("f32in", [128, F32_COLS], f32, isOutput=False)
    out_d = nc.declare_dram_parameter("out", [B // NCORES, JD], f32, isOutput=True)

    a_gate = None  # set below

    with tile.TileContext(nc) as tc:
        with (
            tc.tile_pool(name="const", bufs=1) as cpool,
            tc.tile_pool(name="work", bufs=2) as wpool,
            tc.tile_pool(name="psum", bufs=1, space="PSUM") as ppool,
            tc.tile_pool(name="apsum", bufs=3, space="PSUM") as apool,
            tc.tile_pool(name="work3", bufs=3) as wpool3,
            tc.tile_pool(name="ework", bufs=2) as epool,
            tc.tile_pool(name="dram", bufs=3, space="DRAM") as dpool,
        ):
            a_gate = nc.gpsimd if a_gate_pool else nc.vector

            # ---- streamed loads: 3 (W | xt) k-groups so matmuls start
            # early, then the A-path / normalization constants ----
            wsb, xt = [], []
            off = 0
            for g, gt in enumerate(GROUPS):
                grp = gt * (JD + B)
                t_ = cpool.tile([128, grp], bf16, tag=f"grp{g}", name=f"grp{g}")
                nc.sync.dma_start(t_[:, :], bf_d[:, off:off + grp])
                off += grp
                wsb.append(t_[:, 0:gt * JD].rearrange("p (t n) -> p t n", n=JD))
                xt.append(t_[:, gt * JD:grp].rearrange("p (t b) -> p t b", b=B))
            XB0 = off
            xb_sb = cpool.tile([128, 2 * KL + 1], bf16)
            xb0 = xb_sb[:, 0:KL]
            xb1 = xb_sb[:, KL:2 * KL]
            ones8 = xb_sb[:, 2 * KL:2 * KL + 1]
            f32_sb = cpool.tile([128, F32_COLS], f32)
            m8 = f32_sb[:, 0:128]
            ones10 = f32_sb[0:J, 128:256]      # (10, 128) of ones
            sel10 = f32_sb[0:J, 256:256 + JD]  # sel10[j', d*J+j] = (j==j')

            def s_mms(s_ps, rhs_of, first, last):
                # the two b-halves live in separate PSUM banks: a start=True
                # matmul clears its bank, so interleaved accumulation groups
                # must not share one
                s_ps0, s_ps1 = s_ps
                for g, gt in enumerate(GROUPS):
                    for t_ in range(gt):
                        st = first and g == 0 and t_ == 0
                        sp = last and g == NG - 1 and t_ == gt - 1
                        nc.tensor.matmul(
                            s_ps0[:, :], xt[g][:, t_, 0:128], rhs_of(g, t_),
                            start=st, stop=sp,
                        )
                        nc.tensor.matmul(
                            s_ps1[:, :], xt[g][:, t_, 128:B], rhs_of(g, t_),
                            start=st, stop=sp,
                        )

            def stage_and_collect(s_ps, z_ps, last):
                # stage [s | z] in SBUF; PSUM itself is not DMA-readable.
                # Payload stays [128, W]-shaped (fat rows -> 128 descriptors)
                # for the AllReduce iterations; the final ReduceScatter needs
                # batch on the outer axis so each core receives its 32-row
                # output shard.
                s_ps0, s_ps1 = s_ps
                if not last:
                    # fp8 payload (half the AllReduce bytes): s_un has sigma
                    # ~260 vs e4m3 max 448, so pre-scale by 1/16 on the way
                    # out; the 16 cancels exactly against z (which rides at
                    # z/16) or folds into the iteration-0 squash constants
                    width = 2 * JD + (1 if z_ps is not None else 0)
                    s_sb = wpool.tile([128, width], fp8, tag="s_sb")
                    nc.scalar.mul(s_sb[:, 0:JD], s_ps0[:, :], CSC)
                    nc.vector.tensor_scalar_mul(
                        s_sb[:, JD:2 * JD], s_ps1[:, :], CSC
                    )
                    if z_ps is not None:
                        nc.vector.tensor_scalar_mul(
                            s_sb[0:J, 2 * JD:2 * JD + 1], z_ps[:, :], CSC
                        )
                    cc_in = dpool.tile([128, width], fp8, tag="cc_in")
                    nc.sync.dma_start(cc_in[:, :], s_sb[:, :])
                    cc_out = dpool.tile([128, width], fp8, tag="cc_out", name="ccout")
                    nc.gpsimd.collective_compute(
                        "AllReduce",
                        ALU.add,
                        replica_groups=[list(range(NCORES))],
                        ins=[cc_in.opt()],
                        outs=[cc_out.opt()],
                    )
                    return cc_out
                # final iteration: bf16 payload, feeds the output directly
                s_sb = wpool.tile([128, 2 * (JD + 1)], bf16, tag="s_sb3")
                nc.scalar.copy(s_sb[:, 0:JD], s_ps0[:, :])
                nc.vector.tensor_copy(s_sb[:, JD + 1:2 * JD + 1], s_ps1[:, :])
                for r in range(4):
                    nc.vector.tensor_copy(
                        s_sb[r * 32:r * 32 + J, JD:JD + 1], z_ps[:, :]
                    )
                    nc.vector.tensor_copy(
                        s_sb[r * 32:r * 32 + J, 2 * JD + 1:2 * JD + 2], z_ps[:, :]
                    )
                cc_in = dpool.tile([B, JD + 1], bf16, tag="cc3_in")
                nc.sync.dma_start(
                    cc_in[:, :].rearrange("(c p) n -> p c n", p=128),
                    s_sb[:, :].rearrange("p (c n) -> p c n", n=JD + 1),
                )
                cc_out = dpool.tile([B // NCORES, JD + 1], bf16, tag="cc3_out", name="ccout3")
                nc.gpsimd.collective_compute(
                    "ReduceScatter",
                    ALU.add,
                    replica_groups=[list(range(NCORES))],
                    ins=[cc_in.opt()],
                    outs=[cc_out.opt()],
                )
                return cc_out

            # PE warm-up: the cost model's p-state needs ~3us of continuous
            # matmul activity before full rate; burn it on zeros during the
            # input-load wait so the real matmuls start warm
            warm = cpool.tile([128, 256], bf16, name="warm")
            nc.vector.memset(warm[:, :], 0.0)
            # per-partition bias column for the folded-z0 squash constant
            zb0 = cpool.tile([128, 1], f32, name="zb0")
            nc.vector.memset(zb0[:, :], 2.0 * float(np.log(CC0)))
            # warm_ps shares a PSUM bank with zbc/z (all short-lived, strictly
            # ordered through the tag's WAR chain)
            warm_ps = ppool.tile([128, 256], f32, tag="zbc_ps", name="warm_ps")
            for _ in range(6):
                nc.tensor.matmul(
                    warm_ps[:, :], warm[:, 0:128], warm[:, :],
                    start=True, stop=True,
                )

            # ---- iteration 0 front: b0 == 1 -> uniform softmax: plain
            # matmul on raw W, denominator is the constant 1152 ----
            s_ps = (
                ppool.tile([128, JD], f32, tag="s_ps0", name="s_ps0"),
                ppool.tile([128, JD], f32, tag="s_ps1", name="s_ps1"),
            )
            s_mms(s_ps, lambda g, t_: wsb[g][:, t_, :], True, True)
            # A-path / normalization loads go on the same (SP) queue as the
            # three critical (W | xt) groups: DMA arbitration is arrival
            # order, so another queue's DMA would cut ahead of group data
            nc.sync.dma_start(xb_sb[:, :], bf_d[:, XB0:BF_COLS])
            nc.sync.dma_start(f32_sb[:, :], f32_d[:, :])
            cc_out = stage_and_collect(s_ps, None, last=False)

            e_tiles = [None] * NG
            for it in range(2):
                last_cc = it == 1

                # ---- post-AllReduce squash -> v ----
                width = 2 * JD + (1 if it > 0 else 0)
                sgz = wpool.tile([128, width], fp8, tag="sgz")
                nc.sync.dma_start(sgz[:, :], cc_out[:, :])
                sg = sgz[:, 0:2 * JD]

                if it == 0:
                    # z0 = 1152 exactly (uniform softmax over in_size): fold
                    # it into the squash constants instead of scaling s --
                    # the squash then runs directly on the raw AllReduce sum
                    s_n = sg
                else:
                    s_n = wpool.tile([128, 2 * JD], bf16, tag="s_n")
                    # zinv at (d,j) columns on all 128 partitions: recip the
                    # z column, scale sel10 by it, lift via a (K=10) matmul
                    zinv = wpool.tile([J, 1], f32, tag="zinv")
                    nc.vector.reciprocal(zinv[:, :], sgz[0:J, 2 * JD:2 * JD + 1])
                    zsel = wpool.tile([J, JD], f32, tag="zsel")
                    nc.vector.tensor_scalar_mul(zsel[:, :], sel10[:, :], zinv[:, 0:1])
                    zbc_ps = ppool.tile([128, JD], f32, tag="zbc_ps")
                    nc.tensor.matmul(
                        zbc_ps[:, :], ones10[:, :], zsel[:, :], start=True, stop=True
                    )
                    nc.vector.tensor_tensor(
                        s_n[:, :].rearrange("p (c n) -> p c n", n=JD),
                        sg.rearrange("p (c n) -> p c n", n=JD),
                        zbc_ps[:, :].unsqueeze(1).broadcast_to([128, 2, JD]),
                        ALU.mult,
                    )

                # mag_sq[b, d] = sum_j s[b, (d,j)]^2 : square then innermost
                # reduce; F = sqrt(m)/(1+m) with the ACT (ln,exp) pair and
                # the DVE (1+m, recip) pair running in parallel off msq
                sq = wpool.tile([128, 2 * JD], bf16, tag="sq")
                nc.vector.tensor_mul(sq[:, :], s_n[:, :], s_n[:, :])
                msq = wpool.tile([128, 2 * D], f32, tag="msq")
                nc.vector.tensor_reduce(
                    msq[:, :].rearrange("p (c d) -> p c d", d=D),
                    sq[:, :].rearrange("p (c d j) -> p c d j", d=D, j=J),
                    axis=AX.X,
                    op=ALU.add,
                )
                # it==0 carries the constant z0=1152 inside the squash: with
                # c=1/z0, msq here is z0^2-scaled, so F_eff = c*F(c^2*msq) =
                # exp(0.5*ln(msq) + 2*ln(c)) / (1 + c^2*msq), and v = sg*F_eff
                lnm = wpool.tile([128, 2 * D], f32, tag="lnm")
                nc.scalar.activation(lnm[:, :], msq[:, :], AF.Ln)
                rt = wpool.tile([128, 2 * D], f32, tag="rt")
                nc.scalar.activation(
                    rt[:, :], lnm[:, :], AF.Exp, scale=0.5,
                    bias=(zb0[:, 0:1] if it == 0 else 0.0),
                )
                dn = wpool.tile([128, 2 * D], f32, tag="dn")
                if it == 0:
                    nc.vector.tensor_scalar(
                        dn[:, :], msq[:, :], CC0 * CC0, 1.0,
                        op0=ALU.mult, op1=ALU.add,
                    )
                else:
                    nc.vector.tensor_scalar_add(dn[:, :], msq[:, :], 1.0)
                rc = wpool.tile([128, 2 * D], f32, tag="rc")
                nc.vector.reciprocal(rc[:, :], dn[:, :])
                f_t = wpool.tile([128, 2 * D], f32, tag="f_t")
                nc.vector.tensor_mul(f_t[:, :], rt[:, :], rc[:, :])

                # v = s * F (F broadcast over j); v lands directly in the
                # (b, (d,j)) layout the A-matmul needs -- no transposes
                vt = wpool.tile([128, 2 * JD], bf16, tag="vt")
                for ch in range(2):
                    nc.vector.tensor_tensor(
                        vt[:, ch * JD:(ch + 1) * JD].rearrange("p (d j) -> p d j", j=J),
                        s_n[:, ch * JD:(ch + 1) * JD].rearrange("p (d j) -> p d j", j=J),
                        f_t[:, ch * D:(ch + 1) * D].unsqueeze(2).broadcast_to([128, D, J]),
                        ALU.mult,
                    )
                vb0 = vt[:, 0:JD]
                vb1 = vt[:, JD:2 * JD]

                # ---- fused per-group pipeline: A-path group g immediately
                # feeds that group's e-update, Wc gate and s-matmuls ----
                s_ps = (
                    ppool.tile([128, JD], f32, tag="s_ps0", name="s_ps0"),
                    ppool.tile([128, JD], f32, tag="s_ps1", name="s_ps1"),
                )
                z_ps = ppool.tile([J, 1], f32, tag="zbc_ps", name="z_ps")
                # A-path at k-tile granularity: per tile, PE matmul -> ACT
                # psum drain -> DVE gate -> DVE d-reduce, so the waves are
                # small and every engine streams; e/Wc/s-matmuls then fire
                # per 3-tile group.  Wc gating for the early groups runs on
                # the otherwise-idle GpSimd; the last group (on the serial
                # chain into the collective) stays on the faster DVE.
                r_g = [None] * NG
                for t_ in range(KT):
                    g, tt = t_ // 3, t_ % 3
                    a_ps = apool.tile([128, JD], f32, tag="a_ps")
                    nc.tensor.matmul(
                        a_ps[:, :],
                        xb0[:, t_ * 128:(t_ + 1) * 128], vb0,
                        start=True, stop=False,
                    )
                    nc.tensor.matmul(
                        a_ps[:, :],
                        xb1[:, t_ * 128:(t_ + 1) * 128], vb1,
                        start=False, stop=True,
                    )
                    a_sb = wpool3.tile([128, JD], bf16, tag="a_sb")
                    nc.scalar.copy(a_sb[:, :], a_ps[:, :])
                    p_t = wpool3.tile([128, JD], bf16, tag="p_t")
                    nc.vector.tensor_tensor(
                        p_t[:, :], wsb[g][:, tt, :], a_sb[:, :], ALU.mult,
                    )
                    if tt == 0:
                        r_g[g] = wpool3.tile(
                            [128, 3, J], f32, tag="r_t", name=f"r_g{g}"
                        )
                    nc.vector.tensor_reduce(
                        r_g[g][:, tt, :],
                        p_t.rearrange("p (d j) -> p j d", d=D, j=J),
                        axis=AX.X,
                        op=ALU.add,
                    )
                    if tt < 2:
                        continue
                    # ---- group complete: u-fold, e-update, z, Wc, s ----
                    uv_ps = ppool.tile([128, 3 * J], f32, tag="uv_ps")
                    for a in range(3):
                        nc.tensor.matmul(
                            uv_ps[:, a * J:(a + 1) * J], m8[:, :], r_g[g][:, a, :],
                            start=True, stop=True,
                        )
                    e_new = epool.tile([128, 3, J], bf16, tag=f"e{g}")
                    if it == 0:
                        # e1 = exp(1 + u_vj1): fused add+exp
                        nc.scalar.activation(
                            e_new[:, :, :],
                            uv_ps[:, :].rearrange("p (a j) -> p a j", j=J),
                            AF.Exp,
                            bias=1.0,
                        )
                    else:
                        expuv = wpool3.tile([128, 3, J], bf16, tag="expuv")
                        nc.scalar.activation(
                            expuv[:, :, :],
                            uv_ps[:, :].rearrange("p (a j) -> p a j", j=J),
                            AF.Exp,
                        )
                        nc.vector.tensor_tensor(
                            e_new[:, :, :], e_tiles[g][:, :, :], expuv[:, :, :],
                            ALU.mult,
                        )
                    e_tiles[g] = e_new
                # ---- tail: z, Wc gates and s-matmuls, emitted after the
                # whole per-ktile pipeline so the (in-order) PE stream never
                # stalls behind a slow gate mid-pipeline ----
                for g in range(NG):
                    wc = wpool3.tile([128, 3, JD], bf16, tag="wc")
                    (a_gate if g == 0 else nc.vector).tensor_tensor(
                        wc[:, :, :].rearrange("p t (d j) -> p t d j", j=J),
                        wsb[g][:, :, :].rearrange("p t (d j) -> p t d j", j=J),
                        e_tiles[g][:, :, :].unsqueeze(2).broadcast_to([128, 3, D, J]),
                        ALU.mult,
                    )
                    for a in range(3):
                        st = g == 0 and a == 0
                        sp = g == NG - 1 and a == 2
                        nc.tensor.matmul(
                            s_ps[0][:, :], xt[g][:, a, 0:128], wc[:, a, :],
                            start=st, stop=sp,
                        )
                        nc.tensor.matmul(
                            s_ps[1][:, :], xt[g][:, a, 128:B], wc[:, a, :],
                            start=st, stop=sp,
                        )
                    # z partial: z[j] = sum_i e[i,j] as a (J,1) column
                    for a in range(3):
                        nc.tensor.matmul(
                            z_ps[:, :], e_tiles[g][:, a, :], ones8[:, 0:1],
                            start=(g == 0 and a == 0),
                            stop=(g == NG - 1 and a == 2),
                        )
                cc_out = stage_and_collect(s_ps, z_ps, last=last_cc)

            # ---- post-ReduceScatter shard squash -> out ----
            sg3z = wpool.tile([32, JD + 1], bf16, tag="sg3z")
            nc.sync.dma_start(sg3z[:, :], cc_out[0:32, 0:JD + 1])
            zinv3 = wpool.tile([J, 1], f32, tag="zinv3")
            nc.vector.reciprocal(zinv3[:, :], sg3z[0:J, JD:JD + 1])
            zsel3 = wpool.tile([J, JD], f32, tag="zsel3")
            nc.vector.tensor_scalar_mul(zsel3[:, :], sel10[:, :], zinv3[:, 0:1])
            zbc3 = ppool.tile([32, JD], f32, tag="zbc_ps", name="zbc3")
            nc.tensor.matmul(zbc3[:, :], ones10[:, 0:32], zsel3[:, :], start=True, stop=True)
            sn3 = wpool.tile([32, JD], f32, tag="sn3")
            nc.vector.tensor_mul(sn3[:, :], sg3z[0:32, 0:JD], zbc3[:, :])
            sq3 = wpool.tile([32, JD], bf16, tag="sq3")
            nc.vector.tensor_mul(sq3[:, :], sn3[:, :], sn3[:, :])
            msq3 = wpool.tile([32, D], f32, tag="msq3")
            nc.vector.tensor_reduce(
                msq3[:, :],
                sq3[:, :].rearrange("p (d j) -> p d j", j=J),
                axis=AX.X,
                op=ALU.add,
            )
            ln3 = wpool.tile([32, D], f32, tag="ln3")
            nc.scalar.activation(ln3[:, :], msq3[:, :], AF.Ln)
            rt3 = wpool.tile([32, D], f32, tag="rt3")
            nc.scalar.activation(rt3[:, :], ln3[:, :], AF.Exp, scale=0.5)
            dn3 = wpool.tile([32, D], f32, tag="dn3")
            nc.vector.tensor_scalar_add(dn3[:, :], msq3[:, :], 1.0)
            rc3 = wpool.tile([32, D], f32, tag="rc3")
            nc.vector.reciprocal(rc3[:, :], dn3[:, :])
            f3 = wpool.tile([32, D], f32, tag="f3")
            nc.vector.tensor_mul(f3[:, :], rt3[:, :], rc3[:, :])
            v3 = wpool.tile([32, JD], f32, tag="v3")
            nc.vector.tensor_tensor(
                v3[:, :].rearrange("p (d j) -> p d j", j=J),
                sn3[:, :].rearrange("p (d j) -> p d j", j=J),
                f3[:, :].unsqueeze(2).broadcast_to([32, D, J]),
                ALU.mult,
            )
            nc.sync.dma_start(out_d[:, :], v3[:, :])

    nc.finalize()
    return nc


def _f32_blob():
    blob = np.zeros((128, F32_COLS), np.float32)
    blob[:, 0:128] = np.kron(np.eye(16, dtype=np.float32), np.ones((8, 8), np.float32)) / B
    blob[0:J, 128:256] = 1.0
    blob[0:J, 256:256 + JD] = np.tile(np.eye(J, dtype=np.float32), (1, D))
    return blob


def _prep_in_maps(x, W):
    x = np.asarray(x, np.float32)
    W = np.asarray(W, np.float32)
    Wm = W[0]
    f32_blob = _f32_blob()
    in_maps = []
    for c in range(NCORES):
        sl = slice(c * IL, (c + 1) * IL)
        xs = x[:, :, sl]                                            # (B, U, IL)
        xt = np.ascontiguousarray(xs.transpose(2, 1, 0).reshape(KL, B))
        xb = xt.T
        w = Wm[sl].transpose(0, 3, 2, 1).reshape(KL, JD)            # cols = (d, j)
        wt = w.reshape(KT, 128, JD).transpose(1, 0, 2)              # (128, KT, JD)
        xtt = xt.reshape(KT, 128, B).transpose(1, 0, 2)             # (128, KT, B)
        bf = np.zeros((128, BF_COLS), np.float32)
        o = 0
        for g, gt in enumerate(GROUPS):
            g0 = GOFF[g]
            bf[:, o:o + gt * JD] = wt[:, g0:g0 + gt].reshape(128, gt * JD)
            o += gt * JD
            bf[:, o:o + gt * B] = xtt[:, g0:g0 + gt].reshape(128, gt * B)
            o += gt * B
        bf[:, o:o + KL] = xb[0:128]; o += KL
        bf[:, o:o + KL] = xb[128:256]; o += KL
        bf[:, o] = 0.125; o += 1
        assert o == BF_COLS
        in_maps.append({
            "bfin": bf.astype(ml_dtypes.bfloat16),
            "f32in": f32_blob,
        })
    return in_maps


def run(x, W, trace=False):
    from concourse.bass_utils import run_bass_kernel_spmd

    if "nc" not in _CACHE:
        _CACHE["nc"] = _build_module()
    nc = _CACHE["nc"]
    in_maps = _prep_in_maps(x, W)
    res = run_bass_kernel_spmd(
        nc, in_maps, core_ids=list(range(NCORES)), trace=trace
    )
    v = np.concatenate(
        [np.asarray(res.results[c]["out"], np.float32) for c in range(NCORES)],
        axis=0,
    )                                                               # (B, (d,j))
    out = v.reshape(B, D, J).transpose(0, 2, 1)[..., None]
    return np.ascontiguousarray(out.astype(np.float32)), res


def kernel(x, W):
    out, _ = run(x, W, trace=False)
    return out


# revision 47
# speedup vs baseline: 1.0711x; 1.0004x over previous
"""CapsuleLayer dynamic-routing kernel for 8 TRN2 NeuronCores.

Sharding: in_size (i) is split 8 ways (144 rows/core); every core holds the
full batch.  u_hat (B,1152,10,16 = 189MB) is never materialized: both the
c-weighted sum (s_j) and the agreement update factor through x and W:

    s_un[b, (d,j)]   = sum_{(i,u)} x[b,u,i] * (e[i,j] * W[i,j,d,u])
    A[(i,u), (d,j)]  = sum_b x[b,u,i] * v[b,j,d]
    u_vj1[i,j]       = (1/B) sum_{u,d} W[i,j,d,u] * A[(i,u),(d,j)]

with e unnormalized; the softmax denominator z_j = sum_i e[i,j] rides inside
the per-iteration collective (the only cross-core traffic): AllReduce for
routing iterations 1-2, ReduceScatter for the final one (each core then
squashes and emits only its own 32-batch output shard, gathered host-side).

Structural choices vs the straightforward version:
  * iteration 0 has b=1 (uniform softmax), so s0 is a plain matmul on raw W
    with a compile-time softmax denominator z0=1152 -- no exp, no gating, no
    z column in the first collective;
  * b_ij is never materialized: e is tracked multiplicatively,
    e_{k+1} = e_k * exp(u_vj1), with iteration 1's e = exp(1 + u_vj1)
    produced by a single fused activation (Exp with bias=1);
  * inputs stream in three (W,x) k-groups so the first matmuls start while
    the rest of the load is in flight;
  * the agreement block is pipelined per k-group across four engines
    (PE matmul -> ACT drain -> gate -> DVE d-reduce -> PE u-fold -> ACT exp
    -> DVE gate -> PE s-matmul);
  * collective payloads are [128, W]-shaped so DMAs move 128 fat descriptors
    instead of 256 thin ones.
All matmuls run in bf16 with fp32 PSUM accumulation; exp/ln/copy live in one
ACT function table so only one table load is ever issued.
"""

import os
import sys

import numpy as np

for _p in ("/opt/trn_rl_repo",):
    if _p not in sys.path and os.path.isdir(_p):
        sys.path.insert(0, _p)

import ml_dtypes

NCORES = 8
B, U, I = 256, 8, 1152
J, D = 10, 16
IL = I // NCORES        # 144 in_size rows per core
KL = IL * U             # 1152 local contraction length (i,u)
KT = KL // 128          # 9 partition tiles
GROUPS = [3, 3, 3]      # k-tile groups for loads / e / Wc / s-matmuls
GOFF = [0, 3, 6]        # cumulative k-tile offsets
NG = len(GROUPS)
JD = J * D              # 160
BF_COLS = KT * (JD + B) + 2 * KL + 1 + 128 + JD     # groups | xb | ones8 | ones10b | sel10b
F32_COLS = 256 + JD                                 # m8 | ones10 | sel10

_CACHE = {}


def _build_module(a_gate_pool=True):
    import concourse.bacc as bacc
    import concourse.mybir as mybir
    import concourse.tile as tile

    f32 = mybir.dt.float32
    bf16 = mybir.dt.bfloat16
    fp8 = mybir.dt.float8e4
    CSC = 1.0 / 16.0            # fp8 pre-scale for the AllReduce payload
    CC0 = 1.0 / (I * CSC)       # iteration-0 softmax const with CSC folded
    AF = mybir.ActivationFunctionType
    ALU = mybir.AluOpType
    AX = mybir.AxisListType

    # Force the act-table pass's first-match lookup to land every function
    # we use (Exp, Ln, Copy) on the one table that covers them all, so only
    # a single LoadActFuncSet is ever emitted.  Table *ids* are positional,
    # so we only hide functions from other tables, never reorder.
    if not hasattr(bacc, "_orig_get_activation_tables"):
        bacc._orig_get_activation_tables = bacc.get_activation_tables

        def _patched_tables(arch):
            tabs = bacc._orig_get_activation_tables(arch)
            AF_ = mybir.ActivationFunctionType
            ours = {AF_.Exp, AF_.Ln, AF_.Copy, AF_.Square, AF_.Identity}
            out = {}
            for name, s in tabs.items():
                if name == "natural_log_exp_and_others":
                    out[name] = s
                else:
                    out[name] = s - ours
            return out

        bacc.get_activation_tables = _patched_tables

    nc = bacc.Bacc(
        "TRN2", target_bir_lowering=False, debug=False, num_devices=NCORES
    )

    bf_d = nc.declare_dram_parameter("bfin", [128, BF_COLS], bf16, isOutput=False)
    f32_d = nc.declare_dram_parameter("f32in", [128, F32_COLS], f32, isOutput=False)
    out_d = nc.declare_dram_parameter("out", [B // NCORES, JD], f32, isOutput=True)

    a_gate = None  # set below

    with tile.TileContext(nc) as tc:
        with (
            tc.tile_pool(name="const", bufs=1) as cpool,
            tc.tile_pool(name="work", bufs=2) as wpool,
            tc.tile_pool(name="psum", bufs=1, space="PSUM") as ppool,
            tc.tile_pool(name="apsum", bufs=3, space="PSUM") as apool,
            tc.tile_pool(name="work3", bufs=3) as wpool3,
            tc.tile_pool(name="ework", bufs=2) as epool,
            tc.tile_pool(name="dram", bufs=3, space="DRAM") as dpool,
        ):
            a_gate = nc.gpsimd if a_gate_pool else nc.vector

            # ---- streamed loads: 3 (W | xt) k-groups so matmuls start
            # early, then the A-path / normalization constants ----
            wsb, xt = [], []
            off = 0
            for g, gt in enumerate(GROUPS):
                grp = gt * (JD + B)
                t_ = cpool.tile([128, grp], bf16, tag=f"grp{g}", name=f"grp{g}")
                nc.sync.dma_start(t_[:, :], bf_d[:, off:off + grp])
                off += grp
                wsb.append(t_[:, 0:gt * JD].rearrange("p (t n) -> p t n", n=JD))
                xt.append(t_[:, gt * JD:grp].rearrange("p (t b) -> p t b", b=B))
            XB0 = off
            xb_sb = cpool.tile([128, 2 * KL + 1 + 128 + JD], bf16)
            xb0 = xb_sb[:, 0:KL]
            xb1 = xb_sb[:, KL:2 * KL]
            ones8 = xb_sb[:, 2 * KL:2 * KL + 1]
            ones10b = xb_sb[0:J, 2 * KL + 1:2 * KL + 1 + 128]
            sel10b = xb_sb[0:J, 2 * KL + 1 + 128:2 * KL + 1 + 128 + JD]
            f32_sb = cpool.tile([128, F32_COLS], f32)
            m8 = f32_sb[:, 0:128]
            ones10 = f32_sb[0:J, 128:256]      # (10, 128) of ones
            sel10 = f32_sb[0:J, 256:256 + JD]  # sel10[j', d*J+j] = (j==j')

            def s_mms(s_ps, rhs_of, first, last):
                # the two b-halves live in separate PSUM banks: a start=True
                # matmul clears its bank, so interleaved accumulation groups
                # must not share one
                s_ps0, s_ps1 = s_ps
                for g, gt in enumerate(GROUPS):
                    for t_ in range(gt):
                        st = first and g == 0 and t_ == 0
                        sp = last and g == NG - 1 and t_ == gt - 1
                        nc.tensor.matmul(
                            s_ps0[:, :], xt[g][:, t_, 0:128], rhs_of(g, t_),
                            start=st, stop=sp,
                        )
                        nc.tensor.matmul(
                            s_ps1[:, :], xt[g][:, t_, 128:B], rhs_of(g, t_),
                            start=st, stop=sp,
                        )

            def stage_and_collect(s_ps, z_ps, last):
                # stage [s | z] in SBUF; PSUM itself is not DMA-readable.
                # Payload stays [128, W]-shaped (fat rows -> 128 descriptors)
                # for the AllReduce iterations; the final ReduceScatter needs
                # batch on the outer axis so each core receives its 32-row
                # output shard.
                s_ps0, s_ps1 = s_ps
                if not last:
                    # fp8 payload (half the AllReduce bytes): s_un has sigma
                    # ~260 vs e4m3 max 448, so pre-scale by 1/16 on the way
                    # out; the 16 cancels exactly against z (which rides at
                    # z/16) or folds into the iteration-0 squash constants
                    width = 2 * JD + (1 if z_ps is not None else 0)
                    s_sb = wpool.tile([128, width], fp8, tag="s_sb")
                    nc.scalar.mul(s_sb[:, 0:JD], s_ps0[:, :], CSC)
                    nc.vector.tensor_scalar_mul(
                        s_sb[:, JD:2 * JD], s_ps1[:, :], CSC
                    )
                    if z_ps is not None:
                        nc.vector.tensor_scalar_mul(
                            s_sb[0:J, 2 * JD:2 * JD + 1], z_ps[:, :], CSC
                        )
                    cc_in = dpool.tile([128, width], fp8, tag="cc_in")
                    nc.sync.dma_start(cc_in[:, :], s_sb[:, :])
                    cc_out = dpool.tile([128, width], fp8, tag="cc_out", name="ccout")
                    nc.gpsimd.collective_compute(
                        "AllReduce",
                        ALU.add,
                        replica_groups=[list(range(NCORES))],
                        ins=[cc_in.opt()],
                        outs=[cc_out.opt()],
                    )
                    return cc_out
                # final iteration: bf16 payload, feeds the output directly
                s_sb = wpool.tile([128, 2 * (JD + 1)], bf16, tag="s_sb3")
                nc.scalar.copy(s_sb[:, 0:JD], s_ps0[:, :])
                nc.vector.tensor_copy(s_sb[:, JD + 1:2 * JD + 1], s_ps1[:, :])
                for r in range(4):
                    nc.vector.tensor_copy(
                        s_sb[r * 32:r * 32 + J, JD:JD + 1], z_ps[:, :]
                    )
                    nc.vector.tensor_copy(
                        s_sb[r * 32:r * 32 + J, 2 * JD + 1:2 * JD + 2], z_ps[:, :]
                    )
                cc_in = dpool.tile([B, JD + 1], bf16, tag="cc3_in")
                nc.sync.dma_start(
                    cc_in[:, :].rearrange("(c p) n -> p c n", p=128),
                    s_sb[:, :].rearrange("p (c n) -> p c n", n=JD + 1),
                )
                cc_out = dpool.tile([B // NCORES, JD + 1], bf16, tag="cc3_out", name="ccout3")
                nc.gpsimd.collective_compute(
                    "ReduceScatter",
                    ALU.add,
                    replica_groups=[list(range(NCORES))],
                    ins=[cc_in.opt()],
                    outs=[cc_out.opt()],
                )
                return cc_out

            # PE warm-up: the cost model's p-state needs ~3us of continuous
            # matmul activity before full rate; burn it on zeros during the
            # input-load wait so the real matmuls start warm
            warm = cpool.tile([128, 256], bf16, name="warm")
            nc.vector.memset(warm[:, :], 0.0)
            # per-partition bias column for the folded-z0 squash constant
            zb0 = cpool.tile([128, 1], f32, name="zb0")
            nc.vector.memset(zb0[:, :], 2.0 * float(np.log(CC0)))
            # warm_ps shares a PSUM bank with zbc/z (all short-lived, strictly
            # ordered through the tag's WAR chain)
            warm_ps = ppool.tile([128, 256], f32, tag="zbc_ps", name="warm_ps")
            for _ in range(6):
                nc.tensor.matmul(
                    warm_ps[:, :], warm[:, 0:128], warm[:, :],
                    start=True, stop=True,
                )

            # ---- iteration 0 front: b0 == 1 -> uniform softmax: plain
            # matmul on raw W, denominator is the constant 1152 ----
            s_ps = (
                ppool.tile([128, JD], f32, tag="s_ps0", name="s_ps0"),
                ppool.tile([128, JD], f32, tag="s_ps1", name="s_ps1"),
            )
            s_mms(s_ps, lambda g, t_: wsb[g][:, t_, :], True, True)
            # A-path / normalization loads go on the same (SP) queue as the
            # three critical (W | xt) groups: DMA arbitration is arrival
            # order, so another queue's DMA would cut ahead of group data
            nc.sync.dma_start(xb_sb[:, :], bf_d[:, XB0:BF_COLS])
            nc.sync.dma_start(f32_sb[:, :], f32_d[:, :])
            cc_out = stage_and_collect(s_ps, None, last=False)

            e_tiles = [None] * NG
            xp_tiles = [None] * NG
            wc_prev = [None] * NG
            for it in range(2):
                last_cc = it == 1

                # ---- post-AllReduce squash -> v ----
                width = 2 * JD + (1 if it > 0 else 0)
                sgz = wpool.tile([128, width], fp8, tag="sgz")
                nc.sync.dma_start(sgz[:, :], cc_out[:, :])
                sg = sgz[:, 0:2 * JD]

                if it == 0:
                    # z0 = 1152 exactly (uniform softmax over in_size): fold
                    # it into the squash constants instead of scaling s --
                    # the squash then runs directly on the raw AllReduce sum
                    s_n = sg
                else:
                    s_n = wpool.tile([128, 2 * JD], bf16, tag="s_n")
                    # zinv at (d,j) columns on all 128 partitions: recip the
                    # z column, scale sel10 by it, lift via a (K=10) matmul
                    # (bf16: this z only steers routing iteration 2, and a
                    # bf16 matmul is 4x cheaper than fp32)
                    zinv = wpool.tile([J, 1], f32, tag="zinv")
                    nc.vector.reciprocal(zinv[:, :], sgz[0:J, 2 * JD:2 * JD + 1])
                    zsel = wpool.tile([J, JD], bf16, tag="zsel")
                    nc.vector.tensor_scalar_mul(zsel[:, :], sel10b[:, :], zinv[:, 0:1])
                    zbc_ps = ppool.tile([128, JD], f32, tag="zbc_ps")
                    nc.tensor.matmul(
                        zbc_ps[:, :], ones10b[:, :], zsel[:, :], start=True, stop=True
                    )
                    nc.vector.tensor_tensor(
                        s_n[:, :].rearrange("p (c n) -> p c n", n=JD),
                        sg.rearrange("p (c n) -> p c n", n=JD),
                        zbc_ps[:, :].unsqueeze(1).broadcast_to([128, 2, JD]),
                        ALU.mult,
                    )

                # mag_sq[b, d] = sum_j s[b, (d,j)]^2 : square then innermost
                # reduce; F = sqrt(m)/(1+m) with the ACT (ln,exp) pair and
                # the DVE (1+m, recip) pair running in parallel off msq
                sq = wpool.tile([128, 2 * JD], bf16, tag="sq")
                nc.vector.tensor_mul(sq[:, :], s_n[:, :], s_n[:, :])
                msq = wpool.tile([128, 2 * D], f32, tag="msq")
                nc.vector.tensor_reduce(
                    msq[:, :].rearrange("p (c d) -> p c d", d=D),
                    sq[:, :].rearrange("p (c d j) -> p c d j", d=D, j=J),
                    axis=AX.X,
                    op=ALU.add,
                )
                # it==0 carries the constant z0=1152 inside the squash: with
                # c=1/z0, msq here is z0^2-scaled, so F_eff = c*F(c^2*msq) =
                # exp(0.5*ln(msq) + 2*ln(c)) / (1 + c^2*msq), and v = sg*F_eff
                lnm = wpool.tile([128, 2 * D], f32, tag="lnm")
                nc.scalar.activation(lnm[:, :], msq[:, :], AF.Ln)
                rt = wpool.tile([128, 2 * D], f32, tag="rt")
                nc.scalar.activation(
                    rt[:, :], lnm[:, :], AF.Exp, scale=0.5,
                    bias=(zb0[:, 0:1] if it == 0 else 0.0),
                )
                dn = wpool.tile([128, 2 * D], f32, tag="dn")
                if it == 0:
                    nc.vector.tensor_scalar(
                        dn[:, :], msq[:, :], CC0 * CC0, 1.0,
                        op0=ALU.mult, op1=ALU.add,
                    )
                else:
                    nc.vector.tensor_scalar_add(dn[:, :], msq[:, :], 1.0)
                rc = wpool.tile([128, 2 * D], f32, tag="rc")
                nc.vector.reciprocal(rc[:, :], dn[:, :])
                f_t = wpool.tile([128, 2 * D], f32, tag="f_t")
                nc.vector.tensor_mul(f_t[:, :], rt[:, :], rc[:, :])

                # v = s * F (F broadcast over j); v lands directly in the
                # (b, (d,j)) layout the A-matmul needs -- no transposes
                vt = wpool.tile([128, 2 * JD], bf16, tag="vt")
                for ch in range(2):
                    nc.vector.tensor_tensor(
                        vt[:, ch * JD:(ch + 1) * JD].rearrange("p (d j) -> p d j", j=J),
                        s_n[:, ch * JD:(ch + 1) * JD].rearrange("p (d j) -> p d j", j=J),
                        f_t[:, ch * D:(ch + 1) * D].unsqueeze(2).broadcast_to([128, D, J]),
                        ALU.mult,
                    )
                vb0 = vt[:, 0:JD]
                vb1 = vt[:, JD:2 * JD]

                # ---- fused per-group pipeline: A-path group g immediately
                # feeds that group's e-update, Wc gate and s-matmuls ----
                s_ps = (
                    ppool.tile([128, JD], f32, tag="s_ps0", name="s_ps0"),
                    ppool.tile([128, JD], f32, tag="s_ps1", name="s_ps1"),
                )
                z_ps = ppool.tile([J, 1], f32, tag="zbc_ps", name="z_ps")
                # A-path at k-tile granularity: per tile, PE matmul -> ACT
                # psum drain -> DVE gate -> DVE d-reduce, so the waves are
                # small and every engine streams; e/Wc/s-matmuls then fire
                # per 3-tile group.  Wc gating for the early groups runs on
                # the otherwise-idle GpSimd; the last group (on the serial
                # chain into the collective) stays on the faster DVE.
                r_g = [None] * NG
                for t_ in range(KT):
                    g, tt = t_ // 3, t_ % 3
                    a_ps = apool.tile([128, JD], f32, tag="a_ps")
                    nc.tensor.matmul(
                        a_ps[:, :],
                        xb0[:, t_ * 128:(t_ + 1) * 128], vb0,
                        start=True, stop=False,
                    )
                    nc.tensor.matmul(
                        a_ps[:, :],
                        xb1[:, t_ * 128:(t_ + 1) * 128], vb1,
                        start=False, stop=True,
                    )
                    a_sb = wpool3.tile([128, JD], bf16, tag="a_sb")
                    nc.scalar.copy(a_sb[:, :], a_ps[:, :])
                    p_t = wpool3.tile([128, JD], bf16, tag="p_t")
                    nc.vector.tensor_tensor(
                        p_t[:, :], wsb[g][:, tt, :], a_sb[:, :], ALU.mult,
                    )
                    if tt == 0:
                        r_g[g] = wpool3.tile(
                            [128, 3, J], f32, tag="r_t", name=f"r_g{g}"
                        )
                    nc.vector.tensor_reduce(
                        r_g[g][:, tt, :],
                        p_t.rearrange("p (d j) -> p j d", d=D, j=J),
                        axis=AX.X,
                        op=ALU.add,
                    )
                    if tt < 2:
                        continue
                    # ---- group complete: u-fold, e-update, z, Wc, s ----
                    uv_ps = ppool.tile([128, 3 * J], f32, tag="uv_ps")
                    for a in range(3):
                        nc.tensor.matmul(
                            uv_ps[:, a * J:(a + 1) * J], m8[:, :], r_g[g][:, a, :],
                            start=True, stop=True,
                        )
                    # e is exp(cumulative u_vj1): the +1 in b never matters
                    # (softmax is shift-invariant), so no bias anywhere
                    expuv = epool.tile(
                        [128, 3, J], bf16, tag=f"x{g}", name=f"expuv{g}"
                    )
                    nc.scalar.activation(
                        expuv[:, :, :],
                        uv_ps[:, :].rearrange("p (a j) -> p a j", j=J),
                        AF.Exp,
                    )
                    if it == 0:
                        e_tiles[g] = expuv
                    else:
                        # full e needed only for the z column; Wc chains off
                        # the previous iteration's Wc directly (below), so
                        # this multiply sits off the critical path
                        e_new = epool.tile([128, 3, J], bf16, tag=f"e{g}")
                        nc.vector.tensor_tensor(
                            e_new[:, :, :], e_tiles[g][:, :, :], expuv[:, :, :],
                            ALU.mult,
                        )
                        e_tiles[g] = e_new
                    xp_tiles[g] = expuv
                # ---- tail: z, Wc gates and s-matmuls, emitted after the
                # whole per-ktile pipeline so the (in-order) PE stream never
                # stalls behind a slow gate mid-pipeline ----
                for g in range(NG):
                    wc = epool.tile([128, 3, JD], bf16, tag=f"wc{g}")
                    (a_gate if g == 0 else nc.vector).tensor_tensor(
                        wc[:, :, :].rearrange("p t (d j) -> p t d j", j=J),
                        (wsb[g] if it == 0 else wc_prev[g])[:, :, :]
                        .rearrange("p t (d j) -> p t d j", j=J),
                        xp_tiles[g][:, :, :].unsqueeze(2).broadcast_to([128, 3, D, J]),
                        ALU.mult,
                    )
                    wc_prev[g] = wc
                    for a in range(3):
                        st = g == 0 and a == 0
                        sp = g == NG - 1 and a == 2
                        nc.tensor.matmul(
                            s_ps[0][:, :], xt[g][:, a, 0:128], wc[:, a, :],
                            start=st, stop=sp,
                        )
                        nc.tensor.matmul(
                            s_ps[1][:, :], xt[g][:, a, 128:B], wc[:, a, :],
                            start=st, stop=sp,
                        )
                    # z partial: z[j] = sum_i e[i,j] as a (J,1) column
                    for a in range(3):
                        nc.tensor.matmul(
                            z_ps[:, :], e_tiles[g][:, a, :], ones8[:, 0:1],
                            start=(g == 0 and a == 0),
                            stop=(g == NG - 1 and a == 2),
                        )
                cc_out = stage_and_collect(s_ps, z_ps, last=last_cc)

            # ---- post-ReduceScatter shard squash -> out ----
            sg3z = wpool.tile([32, JD + 1], bf16, tag="sg3z")
            nc.sync.dma_start(sg3z[:, :], cc_out[0:32, 0:JD + 1])
            zinv3 = wpool.tile([J, 1], f32, tag="zinv3")
            nc.vector.reciprocal(zinv3[:, :], sg3z[0:J, JD:JD + 1])
            zsel3 = wpool.tile([J, JD], f32, tag="zsel3")
            nc.vector.tensor_scalar_mul(zsel3[:, :], sel10[:, :], zinv3[:, 0:1])
            zbc3 = ppool.tile([32, JD], f32, tag="zbc_ps", name="zbc3")
            nc.tensor.matmul(zbc3[:, :], ones10[:, 0:32], zsel3[:, :], start=True, stop=True)
            sn3 = wpool.tile([32, JD], f32, tag="sn3")
            nc.vector.tensor_mul(sn3[:, :], sg3z[0:32, 0:JD], zbc3[:, :])
            sq3 = wpool.tile([32, JD], bf16, tag="sq3")
            nc.vector.tensor_mul(sq3[:, :], sn3[:, :], sn3[:, :])
            msq3 = wpool.tile([32, D], f32, tag="msq3")
            nc.vector.tensor_reduce(
                msq3[:, :],
                sq3[:, :].rearrange("p (d j) -> p d j", j=J),
                axis=AX.X,
                op=ALU.add,
            )
            ln3 = wpool.tile([32, D], f32, tag="ln3")
            nc.scalar.activation(ln3[:, :], msq3[:, :], AF.Ln)
            rt3 = wpool.tile([32, D], f32, tag="rt3")
            nc.scalar.activation(rt3[:, :], ln3[:, :], AF.Exp, scale=0.5)
            dn3 = wpool.tile([32, D], f32, tag="dn3")
            nc.vector.tensor_scalar_add(dn3[:, :], msq3[:, :], 1.0)
            rc3 = wpool.tile([32, D], f32, tag="rc3")
            nc.vector.reciprocal(rc3[:, :], dn3[:, :])
            f3 = wpool.tile([32, D], f32, tag="f3")
            nc.vector.tensor_mul(f3[:, :], rt3[:, :], rc3[:, :])
            v3 = wpool.tile([32, JD], f32, tag="v3")
            nc.vector.tensor_tensor(
                v3[:, :].rearrange("p (d j) -> p d j", j=J),
                sn3[:, :].rearrange("p (d j) -> p d j", j=J),
                f3[:, :].unsqueeze(2).broadcast_to([32, D, J]),
                ALU.mult,
            )
            nc.sync.dma_start(out_d[:, :], v3[:, :])

    nc.finalize()
    return nc


def _f32_blob():
    blob = np.zeros((128, F32_COLS), np.float32)
    blob[:, 0:128] = np.kron(np.eye(16, dtype=np.float32), np.ones((8, 8), np.float32)) / B
    blob[0:J, 128:256] = 1.0
    blob[0:J, 256:256 + JD] = np.tile(np.eye(J, dtype=np.float32), (1, D))
    return blob


def _prep_in_maps(x, W):
    x = np.asarray(x, np.float32)
    W = np.asarray(W, np.float32)
    Wm = W[0]
    f32_blob = _f32_blob()
    in_maps = []
    for c in range(NCORES):
        sl = slice(c * IL, (c + 1) * IL)
        xs = x[:, :, sl]                                            # (B, U, IL)
        xt = np.ascontiguousarray(xs.transpose(2, 1, 0).reshape(KL, B))
        xb = xt.T
        w = Wm[sl].transpose(0, 3, 2, 1).reshape(KL, JD)            # cols = (d, j)
        wt = w.reshape(KT, 128, JD).transpose(1, 0, 2)              # (128, KT, JD)
        xtt = xt.reshape(KT, 128, B).transpose(1, 0, 2)             # (128, KT, B)
        bf = np.zeros((128, BF_COLS), np.float32)
        o = 0
        for g, gt in enumerate(GROUPS):
            g0 = GOFF[g]
            bf[:, o:o + gt * JD] = wt[:, g0:g0 + gt].reshape(128, gt * JD)
            o += gt * JD
            bf[:, o:o + gt * B] = xtt[:, g0:g0 + gt].reshape(128, gt * B)
            o += gt * B
        bf[:, o:o + KL] = xb[0:128]; o += KL
        bf[:, o:o + KL] = xb[128:256]; o += KL
        bf[:, o] = 0.125; o += 1
        bf[0:J, o:o + 128] = 1.0; o += 128
        bf[0:J, o:o + JD] = np.tile(np.eye(J, dtype=np.float32), (1, D)); o += JD
        assert o == BF_COLS
        in_maps.append({
            "bfin": bf.astype(ml_dtypes.bfloat16),
            "f32in": f32_blob,
        })
    return in_maps


def run(x, W, trace=False):
    from concourse.bass_utils import run_bass_kernel_spmd

    if "nc" not in _CACHE:
        _CACHE["nc"] = _build_module()
    nc = _CACHE["nc"]
    in_maps = _prep_in_maps(x, W)
    res = run_bass_kernel_spmd(
        nc, in_maps, core_ids=list(range(NCORES)), trace=trace
    )
    v = np.concatenate(
        [np.asarray(res.results[c]["out"], np.float32) for c in range(NCORES)],
        axis=0,
    )                                                               # (B, (d,j))
    out = v.reshape(B, D, J).transpose(0, 2, 1)[..., None]
    return np.ascontiguousarray(out.astype(np.float32)), res


def kernel(x, W):
    out, _ = run(x, W, trace=False)
    return out


# revision 48
# speedup vs baseline: 1.0724x; 1.0012x over previous
"""CapsuleLayer dynamic-routing kernel for 8 TRN2 NeuronCores.

Sharding: in_size (i) is split 8 ways (144 rows/core); every core holds the
full batch.  u_hat (B,1152,10,16 = 189MB) is never materialized: both the
c-weighted sum (s_j) and the agreement update factor through x and W:

    s_un[b, (d,j)]   = sum_{(i,u)} x[b,u,i] * (e[i,j] * W[i,j,d,u])
    A[(i,u), (d,j)]  = sum_b x[b,u,i] * v[b,j,d]
    u_vj1[i,j]       = (1/B) sum_{u,d} W[i,j,d,u] * A[(i,u),(d,j)]

with e unnormalized; the softmax denominator z_j = sum_i e[i,j] rides inside
the per-iteration collective (the only cross-core traffic): AllReduce for
routing iterations 1-2, ReduceScatter for the final one (each core then
squashes and emits only its own 32-batch output shard, gathered host-side).

Structural choices vs the straightforward version:
  * iteration 0 has b=1 (uniform softmax), so s0 is a plain matmul on raw W
    with a compile-time softmax denominator z0=1152 -- no exp, no gating, no
    z column in the first collective;
  * b_ij is never materialized: e is tracked multiplicatively,
    e_{k+1} = e_k * exp(u_vj1), with iteration 1's e = exp(1 + u_vj1)
    produced by a single fused activation (Exp with bias=1);
  * inputs stream in three (W,x) k-groups so the first matmuls start while
    the rest of the load is in flight;
  * the agreement block is pipelined per k-group across four engines
    (PE matmul -> ACT drain -> gate -> DVE d-reduce -> PE u-fold -> ACT exp
    -> DVE gate -> PE s-matmul);
  * collective payloads are [128, W]-shaped so DMAs move 128 fat descriptors
    instead of 256 thin ones.
All matmuls run in bf16 with fp32 PSUM accumulation; exp/ln/copy live in one
ACT function table so only one table load is ever issued.
"""

import os
import sys

import numpy as np

for _p in ("/opt/trn_rl_repo",):
    if _p not in sys.path and os.path.isdir(_p):
        sys.path.insert(0, _p)

import ml_dtypes

NCORES = 8
B, U, I = 256, 8, 1152
J, D = 10, 16
IL = I // NCORES        # 144 in_size rows per core
KL = IL * U             # 1152 local contraction length (i,u)
KT = KL // 128          # 9 partition tiles
GROUPS = [3, 3, 3]      # k-tile groups for loads / e / Wc / s-matmuls
GOFF = [0, 3, 6]        # cumulative k-tile offsets
NG = len(GROUPS)
JD = J * D              # 160
BF_COLS = KT * (JD + B) + 2 * KL + 1 + 128 + JD     # groups | xb | ones8 | ones10b | sel10b
F32_COLS = 256 + JD                                 # m8 | ones10 | sel10

_CACHE = {}


def _build_module(a_gate_pool=True):
    import concourse.bacc as bacc
    import concourse.mybir as mybir
    import concourse.tile as tile

    f32 = mybir.dt.float32
    bf16 = mybir.dt.bfloat16
    fp8 = mybir.dt.float8e4
    CSC = 1.0 / 16.0            # fp8 pre-scale for the AllReduce payload
    CC0 = 1.0 / (I * CSC)       # iteration-0 softmax const with CSC folded
    AF = mybir.ActivationFunctionType
    ALU = mybir.AluOpType
    AX = mybir.AxisListType

    # Force the act-table pass's first-match lookup to land every function
    # we use (Exp, Ln, Copy) on the one table that covers them all, so only
    # a single LoadActFuncSet is ever emitted.  Table *ids* are positional,
    # so we only hide functions from other tables, never reorder.
    if not hasattr(bacc, "_orig_get_activation_tables"):
        bacc._orig_get_activation_tables = bacc.get_activation_tables

        def _patched_tables(arch):
            tabs = bacc._orig_get_activation_tables(arch)
            AF_ = mybir.ActivationFunctionType
            ours = {AF_.Exp, AF_.Ln, AF_.Copy, AF_.Square, AF_.Identity}
            out = {}
            for name, s in tabs.items():
                if name == "natural_log_exp_and_others":
                    out[name] = s
                else:
                    out[name] = s - ours
            return out

        bacc.get_activation_tables = _patched_tables

    nc = bacc.Bacc(
        "TRN2", target_bir_lowering=False, debug=False, num_devices=NCORES
    )

    bf_d = nc.declare_dram_parameter("bfin", [128, BF_COLS], bf16, isOutput=False)
    f32_d = nc.declare_dram_parameter("f32in", [128, F32_COLS], f32, isOutput=False)
    out_d = nc.declare_dram_parameter("out", [B // NCORES, JD], f32, isOutput=True)

    a_gate = None  # set below

    with tile.TileContext(nc) as tc:
        with (
            tc.tile_pool(name="const", bufs=1) as cpool,
            tc.tile_pool(name="work", bufs=2) as wpool,
            tc.tile_pool(name="psum", bufs=1, space="PSUM") as ppool,
            tc.tile_pool(name="apsum", bufs=3, space="PSUM") as apool,
            tc.tile_pool(name="work3", bufs=3) as wpool3,
            tc.tile_pool(name="ework", bufs=2) as epool,
            tc.tile_pool(name="dram", bufs=3, space="DRAM") as dpool,
        ):
            a_gate = nc.gpsimd if a_gate_pool else nc.vector

            # ---- streamed loads: 3 (W | xt) k-groups so matmuls start
            # early, then the A-path / normalization constants ----
            wsb, xt = [], []
            off = 0
            for g, gt in enumerate(GROUPS):
                grp = gt * (JD + B)
                t_ = cpool.tile([128, grp], bf16, tag=f"grp{g}", name=f"grp{g}")
                nc.sync.dma_start(t_[:, :], bf_d[:, off:off + grp])
                off += grp
                wsb.append(t_[:, 0:gt * JD].rearrange("p (t n) -> p t n", n=JD))
                xt.append(t_[:, gt * JD:grp].rearrange("p (t b) -> p t b", b=B))
            XB0 = off
            xb_sb = cpool.tile([128, 2 * KL + 1 + 128 + JD], bf16)
            xb0 = xb_sb[:, 0:KL]
            xb1 = xb_sb[:, KL:2 * KL]
            ones8 = xb_sb[:, 2 * KL:2 * KL + 1]
            ones10b = xb_sb[0:J, 2 * KL + 1:2 * KL + 1 + 128]
            sel10b = xb_sb[0:J, 2 * KL + 1 + 128:2 * KL + 1 + 128 + JD]
            f32_sb = cpool.tile([128, F32_COLS], f32)
            m8 = f32_sb[:, 0:128]
            ones10 = f32_sb[0:J, 128:256]      # (10, 128) of ones
            sel10 = f32_sb[0:J, 256:256 + JD]  # sel10[j', d*J+j] = (j==j')

            def s_mms(s_ps, rhs_of, first, last):
                # the two b-halves live in separate PSUM banks: a start=True
                # matmul clears its bank, so interleaved accumulation groups
                # must not share one
                s_ps0, s_ps1 = s_ps
                for g, gt in enumerate(GROUPS):
                    for t_ in range(gt):
                        st = first and g == 0 and t_ == 0
                        sp = last and g == NG - 1 and t_ == gt - 1
                        nc.tensor.matmul(
                            s_ps0[:, :], xt[g][:, t_, 0:128], rhs_of(g, t_),
                            start=st, stop=sp,
                        )
                        nc.tensor.matmul(
                            s_ps1[:, :], xt[g][:, t_, 128:B], rhs_of(g, t_),
                            start=st, stop=sp,
                        )

            def stage_and_collect(s_ps, z_ps, last):
                # stage [s | z] in SBUF; PSUM itself is not DMA-readable.
                # Payload stays [128, W]-shaped (fat rows -> 128 descriptors)
                # for the AllReduce iterations; the final ReduceScatter needs
                # batch on the outer axis so each core receives its 32-row
                # output shard.
                s_ps0, s_ps1 = s_ps
                if not last:
                    # fp8 payload (half the AllReduce bytes): s_un has sigma
                    # ~260 vs e4m3 max 448, so pre-scale by 1/16 on the way
                    # out; the 16 cancels exactly against z (which rides at
                    # z/16) or folds into the iteration-0 squash constants
                    width = 2 * JD + (1 if z_ps is not None else 0)
                    s_sb = wpool.tile([128, width], fp8, tag="s_sb")
                    nc.scalar.mul(s_sb[:, 0:JD], s_ps0[:, :], CSC)
                    nc.vector.tensor_scalar_mul(
                        s_sb[:, JD:2 * JD], s_ps1[:, :], CSC
                    )
                    if z_ps is not None:
                        nc.vector.tensor_scalar_mul(
                            s_sb[0:J, 2 * JD:2 * JD + 1], z_ps[:, :], CSC
                        )
                    cc_in = dpool.tile([128, width], fp8, tag="cc_in")
                    nc.sync.dma_start(cc_in[:, :], s_sb[:, :])
                    cc_out = dpool.tile([128, width], fp8, tag="cc_out", name="ccout")
                    nc.gpsimd.collective_compute(
                        "AllReduce",
                        ALU.add,
                        replica_groups=[list(range(NCORES))],
                        ins=[cc_in.opt()],
                        outs=[cc_out.opt()],
                    )
                    return cc_out
                # final iteration: bf16 payload, feeds the output directly
                s_sb = wpool.tile([128, 2 * (JD + 1)], bf16, tag="s_sb3")
                nc.scalar.copy(s_sb[:, 0:JD], s_ps0[:, :])
                nc.vector.tensor_copy(s_sb[:, JD + 1:2 * JD + 1], s_ps1[:, :])
                for r in range(4):
                    nc.vector.tensor_copy(
                        s_sb[r * 32:r * 32 + J, JD:JD + 1], z_ps[:, :]
                    )
                    nc.vector.tensor_copy(
                        s_sb[r * 32:r * 32 + J, 2 * JD + 1:2 * JD + 2], z_ps[:, :]
                    )
                cc_in = dpool.tile([B, JD + 1], bf16, tag="cc3_in")
                nc.sync.dma_start(
                    cc_in[:, :].rearrange("(c p) n -> p c n", p=128),
                    s_sb[:, :].rearrange("p (c n) -> p c n", n=JD + 1),
                )
                cc_out = dpool.tile([B // NCORES, JD + 1], bf16, tag="cc3_out", name="ccout3")
                nc.gpsimd.collective_compute(
                    "ReduceScatter",
                    ALU.add,
                    replica_groups=[list(range(NCORES))],
                    ins=[cc_in.opt()],
                    outs=[cc_out.opt()],
                )
                return cc_out

            # PE warm-up: the cost model's p-state needs ~3us of continuous
            # matmul activity before full rate; burn it on zeros during the
            # input-load wait so the real matmuls start warm
            warm = cpool.tile([128, 256], bf16, name="warm")
            nc.vector.memset(warm[:, :], 0.0)
            # per-partition bias column for the folded-z0 squash constant
            zb0 = cpool.tile([128, 1], f32, name="zb0")
            nc.vector.memset(zb0[:, :], 2.0 * float(np.log(CC0)))
            # warm_ps shares a PSUM bank with zbc/z (all short-lived, strictly
            # ordered through the tag's WAR chain)
            warm_ps = ppool.tile([128, 256], f32, tag="zbc_ps", name="warm_ps")
            for _ in range(6):
                nc.tensor.matmul(
                    warm_ps[:, :], warm[:, 0:128], warm[:, :],
                    start=True, stop=True,
                )

            # ---- iteration 0 front: b0 == 1 -> uniform softmax: plain
            # matmul on raw W, denominator is the constant 1152 ----
            s_ps = (
                ppool.tile([128, JD], f32, tag="s_ps0", name="s_ps0"),
                ppool.tile([128, JD], f32, tag="s_ps1", name="s_ps1"),
            )
            s_mms(s_ps, lambda g, t_: wsb[g][:, t_, :], True, True)
            # A-path / normalization loads go on the same (SP) queue as the
            # three critical (W | xt) groups: DMA arbitration is arrival
            # order, so another queue's DMA would cut ahead of group data
            nc.sync.dma_start(xb_sb[:, :], bf_d[:, XB0:BF_COLS])
            nc.sync.dma_start(f32_sb[:, :], f32_d[:, :])
            cc_out = stage_and_collect(s_ps, None, last=False)

            e_tiles = [None] * NG
            xp_tiles = [None] * NG
            wc_prev = [None] * NG
            for it in range(2):
                last_cc = it == 1

                # ---- post-AllReduce squash -> v ----
                width = 2 * JD + (1 if it > 0 else 0)
                sgz = wpool.tile([128, width], fp8, tag="sgz")
                nc.sync.dma_start(sgz[:, :], cc_out[:, :])
                sg = sgz[:, 0:2 * JD]

                if it == 0:
                    # z0 = 1152 exactly (uniform softmax over in_size): fold
                    # it into the squash constants instead of scaling s --
                    # the squash then runs directly on the raw AllReduce sum
                    s_n = sg
                else:
                    s_n = wpool.tile([128, 2 * JD], bf16, tag="s_n")
                    # zinv at (d,j) columns on all 128 partitions: recip the
                    # z column, scale sel10 by it, lift via a (K=10) matmul
                    # (bf16: this z only steers routing iteration 2, and a
                    # bf16 matmul is 4x cheaper than fp32)
                    zinv = wpool.tile([J, 1], f32, tag="zinv")
                    nc.vector.reciprocal(zinv[:, :], sgz[0:J, 2 * JD:2 * JD + 1])
                    zsel = wpool.tile([J, JD], bf16, tag="zsel")
                    nc.vector.tensor_scalar_mul(zsel[:, :], sel10b[:, :], zinv[:, 0:1])
                    zbc_ps = ppool.tile([128, JD], f32, tag="zbc_ps")
                    nc.tensor.matmul(
                        zbc_ps[:, :], ones10b[:, :], zsel[:, :], start=True, stop=True
                    )
                    nc.vector.tensor_tensor(
                        s_n[:, :].rearrange("p (c n) -> p c n", n=JD),
                        sg.rearrange("p (c n) -> p c n", n=JD),
                        zbc_ps[:, :].unsqueeze(1).broadcast_to([128, 2, JD]),
                        ALU.mult,
                    )

                # mag_sq[b, d] = sum_j s[b, (d,j)]^2 : square then innermost
                # reduce; F = sqrt(m)/(1+m) with the ACT (ln,exp) pair and
                # the DVE (1+m, recip) pair running in parallel off msq
                sq = wpool.tile([128, 2 * JD], bf16, tag="sq")
                nc.vector.tensor_mul(sq[:, :], s_n[:, :], s_n[:, :])
                msq = wpool.tile([128, 2 * D], f32, tag="msq")
                nc.vector.tensor_reduce(
                    msq[:, :].rearrange("p (c d) -> p c d", d=D),
                    sq[:, :].rearrange("p (c d j) -> p c d j", d=D, j=J),
                    axis=AX.X,
                    op=ALU.add,
                )
                # it==0 carries the constant z0=1152 inside the squash: with
                # c=1/z0, msq here is z0^2-scaled, so F_eff = c*F(c^2*msq) =
                # exp(0.5*ln(msq) + 2*ln(c)) / (1 + c^2*msq), and v = sg*F_eff
                lnm = wpool.tile([128, 2 * D], f32, tag="lnm")
                nc.scalar.activation(lnm[:, :], msq[:, :], AF.Ln)
                rt = wpool.tile([128, 2 * D], f32, tag="rt")
                nc.scalar.activation(
                    rt[:, :], lnm[:, :], AF.Exp, scale=0.5,
                    bias=(zb0[:, 0:1] if it == 0 else 0.0),
                )
                dn = wpool.tile([128, 2 * D], f32, tag="dn")
                if it == 0:
                    nc.vector.tensor_scalar(
                        dn[:, :], msq[:, :], CC0 * CC0, 1.0,
                        op0=ALU.mult, op1=ALU.add,
                    )
                else:
                    nc.vector.tensor_scalar_add(dn[:, :], msq[:, :], 1.0)
                rc = wpool.tile([128, 2 * D], f32, tag="rc")
                nc.vector.reciprocal(rc[:, :], dn[:, :])
                f_t = wpool.tile([128, 2 * D], f32, tag="f_t")
                nc.vector.tensor_mul(f_t[:, :], rt[:, :], rc[:, :])

                # v = s * F (F broadcast over j); v lands directly in the
                # (b, (d,j)) layout the A-matmul needs -- no transposes
                vt = wpool.tile([128, 2 * JD], bf16, tag="vt")
                for ch in range(2):
                    nc.vector.tensor_tensor(
                        vt[:, ch * JD:(ch + 1) * JD].rearrange("p (d j) -> p d j", j=J),
                        s_n[:, ch * JD:(ch + 1) * JD].rearrange("p (d j) -> p d j", j=J),
                        f_t[:, ch * D:(ch + 1) * D].unsqueeze(2).broadcast_to([128, D, J]),
                        ALU.mult,
                    )
                vb0 = vt[:, 0:JD]
                vb1 = vt[:, JD:2 * JD]

                # ---- fused per-group pipeline: A-path group g immediately
                # feeds that group's e-update, Wc gate and s-matmuls ----
                s_ps = (
                    ppool.tile([128, JD], f32, tag="s_ps0", name="s_ps0"),
                    ppool.tile([128, JD], f32, tag="s_ps1", name="s_ps1"),
                )
                z_ps = ppool.tile([J, 1], f32, tag="zbc_ps", name="z_ps")
                # A-path at k-tile granularity: per tile, PE matmul -> ACT
                # psum drain -> DVE gate -> DVE d-reduce, so the waves are
                # small and every engine streams; e/Wc/s-matmuls then fire
                # per 3-tile group.  Wc gating for the early groups runs on
                # the otherwise-idle GpSimd; the last group (on the serial
                # chain into the collective) stays on the faster DVE.
                r_g = [None] * NG
                for t_ in range(KT):
                    g, tt = t_ // 3, t_ % 3
                    a_ps = apool.tile([128, JD], f32, tag="a_ps")
                    nc.tensor.matmul(
                        a_ps[:, :],
                        xb0[:, t_ * 128:(t_ + 1) * 128], vb0,
                        start=True, stop=False,
                    )
                    nc.tensor.matmul(
                        a_ps[:, :],
                        xb1[:, t_ * 128:(t_ + 1) * 128], vb1,
                        start=False, stop=True,
                    )
                    a_sb = wpool3.tile([128, JD], bf16, tag="a_sb")
                    nc.scalar.copy(a_sb[:, :], a_ps[:, :])
                    p_t = wpool3.tile([128, JD], bf16, tag="p_t")
                    nc.vector.tensor_tensor(
                        p_t[:, :], wsb[g][:, tt, :], a_sb[:, :], ALU.mult,
                    )
                    if tt == 0:
                        r_g[g] = wpool3.tile(
                            [128, 3, J], f32, tag="r_t", name=f"r_g{g}"
                        )
                    nc.vector.tensor_reduce(
                        r_g[g][:, tt, :],
                        p_t.rearrange("p (d j) -> p j d", d=D, j=J),
                        axis=AX.X,
                        op=ALU.add,
                    )
                    if tt < 2:
                        continue
                    # ---- group complete: u-fold, e-update, z, Wc, s ----
                    uv_ps = ppool.tile([128, 3 * J], f32, tag="uv_ps")
                    for a in range(3):
                        nc.tensor.matmul(
                            uv_ps[:, a * J:(a + 1) * J], m8[:, :], r_g[g][:, a, :],
                            start=True, stop=True,
                        )
                    # e is exp(cumulative u_vj1): the +1 in b never matters
                    # (softmax is shift-invariant), so no bias anywhere
                    expuv = epool.tile(
                        [128, 3, J], bf16, tag=f"x{g}", name=f"expuv{g}"
                    )
                    nc.scalar.activation(
                        expuv[:, :, :],
                        uv_ps[:, :].rearrange("p (a j) -> p a j", j=J),
                        AF.Exp,
                    )
                    if it == 0:
                        e_tiles[g] = expuv
                    else:
                        # full e needed only for the z column; Wc chains off
                        # the previous iteration's Wc directly (below), so
                        # this multiply sits off the critical path
                        e_new = epool.tile([128, 3, J], bf16, tag=f"e{g}")
                        nc.vector.tensor_tensor(
                            e_new[:, :, :], e_tiles[g][:, :, :], expuv[:, :, :],
                            ALU.mult,
                        )
                        e_tiles[g] = e_new
                    xp_tiles[g] = expuv
                # ---- tail: z, Wc gates and s-matmuls, emitted after the
                # whole per-ktile pipeline so the (in-order) PE stream never
                # stalls behind a slow gate mid-pipeline ----
                for g in range(NG):
                    wc = epool.tile([128, 3, JD], bf16, tag=f"wc{g}")
                    (a_gate if g == 0 else nc.vector).tensor_tensor(
                        wc[:, :, :].rearrange("p t (d j) -> p t d j", j=J),
                        wsb[g][:, :, :].rearrange("p t (d j) -> p t d j", j=J),
                        e_tiles[g][:, :, :].unsqueeze(2).broadcast_to([128, 3, D, J]),
                        ALU.mult,
                    )
                    for a in range(3):
                        st = g == 0 and a == 0
                        sp = g == NG - 1 and a == 2
                        nc.tensor.matmul(
                            s_ps[0][:, :], xt[g][:, a, 0:128], wc[:, a, :],
                            start=st, stop=sp,
                        )
                        nc.tensor.matmul(
                            s_ps[1][:, :], xt[g][:, a, 128:B], wc[:, a, :],
                            start=st, stop=sp,
                        )
                    # z partial: z[j] = sum_i e[i,j] as a (J,1) column
                    for a in range(3):
                        nc.tensor.matmul(
                            z_ps[:, :], e_tiles[g][:, a, :], ones8[:, 0:1],
                            start=(g == 0 and a == 0),
                            stop=(g == NG - 1 and a == 2),
                        )
                cc_out = stage_and_collect(s_ps, z_ps, last=last_cc)

            # ---- post-ReduceScatter shard squash -> out ----
            sg3z = wpool.tile([32, JD + 1], bf16, tag="sg3z")
            nc.sync.dma_start(sg3z[:, :], cc_out[0:32, 0:JD + 1])
            zinv3 = wpool.tile([J, 1], f32, tag="zinv3")
            nc.vector.reciprocal(zinv3[:, :], sg3z[0:J, JD:JD + 1])
            zsel3 = wpool.tile([J, JD], f32, tag="zsel3")
            nc.vector.tensor_scalar_mul(zsel3[:, :], sel10[:, :], zinv3[:, 0:1])
            zbc3 = ppool.tile([32, JD], f32, tag="zbc_ps", name="zbc3")
            nc.tensor.matmul(zbc3[:, :], ones10[:, 0:32], zsel3[:, :], start=True, stop=True)
            sn3 = wpool.tile([32, JD], f32, tag="sn3")
            nc.vector.tensor_mul(sn3[:, :], sg3z[0:32, 0:JD], zbc3[:, :])
            sq3 = wpool.tile([32, JD], bf16, tag="sq3")
            nc.vector.tensor_mul(sq3[:, :], sn3[:, :], sn3[:, :])
            msq3 = wpool.tile([32, D], f32, tag="msq3")
            nc.vector.tensor_reduce(
                msq3[:, :],
                sq3[:, :].rearrange("p (d j) -> p d j", j=J),
                axis=AX.X,
                op=ALU.add,
            )
            ln3 = wpool.tile([32, D], f32, tag="ln3")
            nc.scalar.activation(ln3[:, :], msq3[:, :], AF.Ln)
            rt3 = wpool.tile([32, D], f32, tag="rt3")
            nc.scalar.activation(rt3[:, :], ln3[:, :], AF.Exp, scale=0.5)
            dn3 = wpool.tile([32, D], f32, tag="dn3")
            nc.vector.tensor_scalar_add(dn3[:, :], msq3[:, :], 1.0)
            rc3 = wpool.tile([32, D], f32, tag="rc3")
            nc.vector.reciprocal(rc3[:, :], dn3[:, :])
            f3 = wpool.tile([32, D], f32, tag="f3")
            nc.vector.tensor_mul(f3[:, :], rt3[:, :], rc3[:, :])
            v3 = wpool.tile([32, JD], f32, tag="v3")
            nc.vector.tensor_tensor(
                v3[:, :].rearrange("p (d j) -> p d j", j=J),
                sn3[:, :].rearrange("p (d j) -> p d j", j=J),
                f3[:, :].unsqueeze(2).broadcast_to([32, D, J]),
                ALU.mult,
            )
            nc.sync.dma_start(out_d[:, :], v3[:, :])

    nc.finalize()
    return nc


def _f32_blob():
    blob = np.zeros((128, F32_COLS), np.float32)
    blob[:, 0:128] = np.kron(np.eye(16, dtype=np.float32), np.ones((8, 8), np.float32)) / B
    blob[0:J, 128:256] = 1.0
    blob[0:J, 256:256 + JD] = np.tile(np.eye(J, dtype=np.float32), (1, D))
    return blob


def _prep_in_maps(x, W):
    x = np.asarray(x, np.float32)
    W = np.asarray(W, np.float32)
    Wm = W[0]
    f32_blob = _f32_blob()
    in_maps = []
    for c in range(NCORES):
        sl = slice(c * IL, (c + 1) * IL)
        xs = x[:, :, sl]                                            # (B, U, IL)
        xt = np.ascontiguousarray(xs.transpose(2, 1, 0).reshape(KL, B))
        xb = xt.T
        w = Wm[sl].transpose(0, 3, 2, 1).reshape(KL, JD)            # cols = (d, j)
        wt = w.reshape(KT, 128, JD).transpose(1, 0, 2)              # (128, KT, JD)
        xtt = xt.reshape(KT, 128, B).transpose(1, 0, 2)             # (128, KT, B)
        bf = np.zeros((128, BF_COLS), np.float32)
        o = 0
        for g, gt in enumerate(GROUPS):
            g0 = GOFF[g]
            bf[:, o:o + gt * JD] = wt[:, g0:g0 + gt].reshape(128, gt * JD)
            o += gt * JD
            bf[:, o:o + gt * B] = xtt[:, g0:g0 + gt].reshape(128, gt * B)
            o += gt * B
        bf[:, o:o + KL] = xb[0:128]; o += KL
        bf[:, o:o + KL] = xb[128:256]; o += KL
        bf[:, o] = 0.125; o += 1
        bf[0:J, o:o + 128] = 1.0; o += 128
        bf[0:J, o:o + JD] = np.tile(np.eye(J, dtype=np.float32), (1, D)); o += JD
        assert o == BF_COLS
        in_maps.append({
            "bfin": bf.astype(ml_dtypes.bfloat16),
            "f32in": f32_blob,
        })
    return in_maps


def run(x, W, trace=False):
    from concourse.bass_utils import run_bass_kernel_spmd

    if "nc" not in _CACHE:
        _CACHE["nc"] = _build_module()
    nc = _CACHE["nc"]
    in_maps = _prep_in_maps(x, W)
    res = run_bass_kernel_spmd(
        nc, in_maps, core_ids=list(range(NCORES)), trace=trace
    )
    v = np.concatenate(
        [np.asarray(res.results[c]["out"], np.float32) for c in range(NCORES)],
        axis=0,
    )                                                               # (B, (d,j))
    out = v.reshape(B, D, J).transpose(0, 2, 1)[..., None]
    return np.ascontiguousarray(out.astype(np.float32)), res


def kernel(x, W):
    out, _ = run(x, W, trace=False)
    return out


# revision 56
# speedup vs baseline: 1.0750x; 1.0024x over previous
"""CapsuleLayer dynamic-routing kernel for 8 TRN2 NeuronCores.

Sharding: in_size (i) is split 8 ways (144 rows/core); every core holds the
full batch.  u_hat (B,1152,10,16 = 189MB) is never materialized: both the
c-weighted sum (s_j) and the agreement update factor through x and W:

    s_un[b, (d,j)]   = sum_{(i,u)} x[b,u,i] * (e[i,j] * W[i,j,d,u])
    A[(i,u), (d,j)]  = sum_b x[b,u,i] * v[b,j,d]
    u_vj1[i,j]       = (1/B) sum_{u,d} W[i,j,d,u] * A[(i,u),(d,j)]

with e unnormalized; the softmax denominator z_j = sum_i e[i,j] rides inside
the per-iteration collective (the only cross-core traffic): AllReduce for
routing iterations 1-2, ReduceScatter for the final one (each core then
squashes and emits only its own 32-batch output shard, gathered host-side).

Structural choices vs the straightforward version:
  * iteration 0 has b=1 (uniform softmax), so s0 is a plain matmul on raw W
    with a compile-time softmax denominator z0=1152 -- no exp, no gating, no
    z column in the first collective;
  * b_ij is never materialized: e is tracked multiplicatively,
    e_{k+1} = e_k * exp(u_vj1), with iteration 1's e = exp(1 + u_vj1)
    produced by a single fused activation (Exp with bias=1);
  * inputs stream in three (W,x) k-groups so the first matmuls start while
    the rest of the load is in flight;
  * the agreement block is pipelined per k-group across four engines
    (PE matmul -> ACT drain -> gate -> DVE d-reduce -> PE u-fold -> ACT exp
    -> DVE gate -> PE s-matmul);
  * collective payloads are [128, W]-shaped so DMAs move 128 fat descriptors
    instead of 256 thin ones.
All matmuls run in bf16 with fp32 PSUM accumulation; exp/ln/copy live in one
ACT function table so only one table load is ever issued.
"""

import os
import sys

import numpy as np

for _p in ("/opt/trn_rl_repo",):
    if _p not in sys.path and os.path.isdir(_p):
        sys.path.insert(0, _p)

import ml_dtypes

NCORES = 8
B, U, I = 256, 8, 1152
J, D = 10, 16
IL = I // NCORES        # 144 in_size rows per core
KL = IL * U             # 1152 local contraction length (i,u)
KT = KL // 128          # 9 partition tiles
GROUPS = [3, 3, 3]      # k-tile groups for loads / e / Wc / s-matmuls
GOFF = [0, 3, 6]        # cumulative k-tile offsets
NG = len(GROUPS)
JD = J * D              # 160
BF_COLS = KT * (JD + B) + 2 * KL + 1 + 128 + JD     # groups | xb | ones8 | ones10b | sel10b
F32_COLS = 256 + JD                                 # m8 | ones10 | sel10

_CACHE = {}


def _build_module(a_gate_pool=True):
    import concourse.bacc as bacc
    import concourse.mybir as mybir
    import concourse.tile as tile

    f32 = mybir.dt.float32
    bf16 = mybir.dt.bfloat16
    fp8 = mybir.dt.float8e4
    CSC = 1.0 / 16.0            # fp8 pre-scale for the AllReduce payload
    CC0 = 1.0 / (I * CSC)       # iteration-0 softmax const with CSC folded
    AF = mybir.ActivationFunctionType
    ALU = mybir.AluOpType
    AX = mybir.AxisListType

    # Force the act-table pass's first-match lookup to land every function
    # we use (Exp, Ln, Copy) on the one table that covers them all, so only
    # a single LoadActFuncSet is ever emitted.  Table *ids* are positional,
    # so we only hide functions from other tables, never reorder.
    if not hasattr(bacc, "_orig_get_activation_tables"):
        bacc._orig_get_activation_tables = bacc.get_activation_tables

        def _patched_tables(arch):
            tabs = bacc._orig_get_activation_tables(arch)
            AF_ = mybir.ActivationFunctionType
            ours = {AF_.Exp, AF_.Ln, AF_.Copy, AF_.Square, AF_.Identity}
            out = {}
            for name, s in tabs.items():
                if name == "natural_log_exp_and_others":
                    out[name] = s
                else:
                    out[name] = s - ours
            return out

        bacc.get_activation_tables = _patched_tables

    nc = bacc.Bacc(
        "TRN2", target_bir_lowering=False, debug=False, num_devices=NCORES
    )

    bf_d = nc.declare_dram_parameter("bfin", [128, BF_COLS], bf16, isOutput=False)
    f32_d = nc.declare_dram_parameter("f32in", [128, F32_COLS], f32, isOutput=False)
    out_d = nc.declare_dram_parameter("out", [B // NCORES, JD], f32, isOutput=True)

    a_gate = None  # set below

    with tile.TileContext(nc) as tc:
        with (
            tc.tile_pool(name="const", bufs=1) as cpool,
            tc.tile_pool(name="work", bufs=2) as wpool,
            tc.tile_pool(name="psum", bufs=1, space="PSUM") as ppool,
            tc.tile_pool(name="apsum", bufs=3, space="PSUM") as apool,
            tc.tile_pool(name="work3", bufs=3) as wpool3,
            tc.tile_pool(name="ework", bufs=2) as epool,
            tc.tile_pool(name="dram", bufs=3, space="DRAM") as dpool,
        ):
            a_gate = nc.gpsimd if a_gate_pool else nc.vector

            # ---- streamed loads: 3 (W | xt) k-groups so matmuls start
            # early, then the A-path / normalization constants ----
            wsb, xt = [], []
            off = 0
            for g, gt in enumerate(GROUPS):
                grp = gt * (JD + B)
                t_ = cpool.tile([128, grp], bf16, tag=f"grp{g}", name=f"grp{g}")
                nc.sync.dma_start(t_[:, :], bf_d[:, off:off + grp])
                off += grp
                wsb.append(t_[:, 0:gt * JD].rearrange("p (t n) -> p t n", n=JD))
                xt.append(t_[:, gt * JD:grp].rearrange("p (t b) -> p t b", b=B))
            XB0 = off
            xb_sb = cpool.tile([128, 2 * KL + 1 + 128 + JD], bf16)
            xb0 = xb_sb[:, 0:KL]
            xb1 = xb_sb[:, KL:2 * KL]
            ones8 = xb_sb[:, 2 * KL:2 * KL + 1]
            ones10b = xb_sb[0:J, 2 * KL + 1:2 * KL + 1 + 128]
            sel10b = xb_sb[0:J, 2 * KL + 1 + 128:2 * KL + 1 + 128 + JD]
            f32_sb = cpool.tile([128, F32_COLS], f32)
            m8 = f32_sb[:, 0:128]
            ones10 = f32_sb[0:J, 128:256]      # (10, 128) of ones
            sel10 = f32_sb[0:J, 256:256 + JD]  # sel10[j', d*J+j] = (j==j')

            def s_mms(s_ps, rhs_of, first, last):
                # the two b-halves live in separate PSUM banks: a start=True
                # matmul clears its bank, so interleaved accumulation groups
                # must not share one
                s_ps0, s_ps1 = s_ps
                for g, gt in enumerate(GROUPS):
                    for t_ in range(gt):
                        st = first and g == 0 and t_ == 0
                        sp = last and g == NG - 1 and t_ == gt - 1
                        nc.tensor.matmul(
                            s_ps0[:, :], xt[g][:, t_, 0:128], rhs_of(g, t_),
                            start=st, stop=sp,
                        )
                        nc.tensor.matmul(
                            s_ps1[:, :], xt[g][:, t_, 128:B], rhs_of(g, t_),
                            start=st, stop=sp,
                        )

            def stage_and_collect(s_ps, z_ps, last):
                # stage [s | z] in SBUF; PSUM itself is not DMA-readable.
                # Payload stays [128, W]-shaped (fat rows -> 128 descriptors)
                # for the AllReduce iterations; the final ReduceScatter needs
                # batch on the outer axis so each core receives its 32-row
                # output shard.
                s_ps0, s_ps1 = s_ps
                if not last:
                    # fp8 payload (half the AllReduce bytes): s_un has sigma
                    # ~260 vs e4m3 max 448, so pre-scale by 1/16 on the way
                    # out; the 16 cancels exactly against z (which rides at
                    # z/16) or folds into the iteration-0 squash constants
                    width = 2 * JD + (1 if z_ps is not None else 0)
                    s_sb = wpool.tile([128, width], fp8, tag="s_sb")
                    nc.scalar.mul(s_sb[:, 0:JD], s_ps0[:, :], CSC)
                    nc.vector.tensor_scalar_mul(
                        s_sb[:, JD:2 * JD], s_ps1[:, :], CSC
                    )
                    if z_ps is not None:
                        nc.vector.tensor_scalar_mul(
                            s_sb[0:J, 2 * JD:2 * JD + 1], z_ps[:, :], CSC
                        )
                    cc_in = dpool.tile([128, width], fp8, tag="cc_in")
                    nc.sync.dma_start(cc_in[:, :], s_sb[:, :])
                    cc_out = dpool.tile([128, width], fp8, tag="cc_out", name="ccout")
                    nc.gpsimd.collective_compute(
                        "AllReduce",
                        ALU.add,
                        replica_groups=[list(range(NCORES))],
                        ins=[cc_in.opt()],
                        outs=[cc_out.opt()],
                    )
                    return cc_out
                # final iteration: bf16 payload, feeds the output directly
                s_sb = wpool.tile([128, 2 * (JD + 1)], bf16, tag="s_sb3")
                nc.scalar.copy(s_sb[:, 0:JD], s_ps0[:, :])
                nc.vector.tensor_copy(s_sb[:, JD + 1:2 * JD + 1], s_ps1[:, :])
                for r in range(4):
                    nc.vector.tensor_copy(
                        s_sb[r * 32:r * 32 + J, JD:JD + 1], z_ps[:, :]
                    )
                    nc.vector.tensor_copy(
                        s_sb[r * 32:r * 32 + J, 2 * JD + 1:2 * JD + 2], z_ps[:, :]
                    )
                cc_in = dpool.tile([B, JD + 1], bf16, tag="cc3_in")
                nc.sync.dma_start(
                    cc_in[:, :].rearrange("(c p) n -> p c n", p=128),
                    s_sb[:, :].rearrange("p (c n) -> p c n", n=JD + 1),
                )
                cc_out = dpool.tile([B // NCORES, JD + 1], bf16, tag="cc3_out", name="ccout3")
                nc.gpsimd.collective_compute(
                    "ReduceScatter",
                    ALU.add,
                    replica_groups=[list(range(NCORES))],
                    ins=[cc_in.opt()],
                    outs=[cc_out.opt()],
                )
                return cc_out

            # PE warm-up: the cost model's p-state needs ~3us of continuous
            # matmul activity before full rate; burn it on zeros during the
            # input-load wait so the real matmuls start warm
            warm = cpool.tile([128, 256], bf16, name="warm")
            nc.vector.memset(warm[:, :], 0.0)
            # per-partition bias column for the folded-z0 squash constant
            zb0 = cpool.tile([128, 1], f32, name="zb0")
            nc.vector.memset(zb0[:, :], 2.0 * float(np.log(CC0)))
            # warm_ps shares a PSUM bank with zbc/z (all short-lived, strictly
            # ordered through the tag's WAR chain)
            warm_ps = ppool.tile([128, 256], f32, tag="zbc_ps", name="warm_ps")
            for _ in range(6):
                nc.tensor.matmul(
                    warm_ps[:, :], warm[:, 0:128], warm[:, :],
                    start=True, stop=True,
                )

            # ---- iteration 0 front: b0 == 1 -> uniform softmax: plain
            # matmul on raw W, denominator is the constant 1152 ----
            s_ps = (
                ppool.tile([128, JD], f32, tag="s_ps0", name="s_ps0"),
                ppool.tile([128, JD], f32, tag="s_ps1", name="s_ps1"),
            )
            s_mms(s_ps, lambda g, t_: wsb[g][:, t_, :], True, True)
            # A-path / normalization loads go on the same (SP) queue as the
            # three critical (W | xt) groups: DMA arbitration is arrival
            # order, so another queue's DMA would cut ahead of group data
            nc.sync.dma_start(xb_sb[:, :], bf_d[:, XB0:BF_COLS])
            nc.sync.dma_start(f32_sb[:, :], f32_d[:, :])
            cc_out = stage_and_collect(s_ps, None, last=False)

            e_tiles = [None] * NG
            xp_tiles = [None] * NG
            wc_prev = [None] * NG
            for it in range(2):
                last_cc = it == 1

                # ---- post-AllReduce squash -> v ----
                width = 2 * JD + (1 if it > 0 else 0)
                sgz = wpool.tile([128, width], fp8, tag="sgz")
                nc.sync.dma_start(sgz[:, :], cc_out[:, :])
                sg = sgz[:, 0:2 * JD]

                if it == 0:
                    # z0 = 1152 exactly (uniform softmax over in_size): fold
                    # it into the squash constants instead of scaling s --
                    # the squash then runs directly on the raw AllReduce sum
                    s_n = sg
                else:
                    s_n = wpool.tile([128, 2 * JD], bf16, tag="s_n")
                    # zinv at (d,j) columns on all 128 partitions: recip the
                    # z column, scale sel10 by it, lift via a (K=10) matmul
                    # (bf16: this z only steers routing iteration 2, and a
                    # bf16 matmul is 4x cheaper than fp32)
                    zinv = wpool.tile([J, 1], f32, tag="zinv")
                    nc.vector.reciprocal(zinv[:, :], sgz[0:J, 2 * JD:2 * JD + 1])
                    zsel = wpool.tile([J, JD], bf16, tag="zsel")
                    nc.vector.tensor_scalar_mul(zsel[:, :], sel10b[:, :], zinv[:, 0:1])
                    zbc_ps = ppool.tile([128, JD], f32, tag="zbc_ps")
                    nc.tensor.matmul(
                        zbc_ps[:, :], ones10b[:, :], zsel[:, :], start=True, stop=True
                    )
                    nc.vector.tensor_tensor(
                        s_n[:, :].rearrange("p (c n) -> p c n", n=JD),
                        sg.rearrange("p (c n) -> p c n", n=JD),
                        zbc_ps[:, :].unsqueeze(1).broadcast_to([128, 2, JD]),
                        ALU.mult,
                    )

                # mag_sq[b, d] = sum_j s[b, (d,j)]^2 : square then innermost
                # reduce; F = sqrt(m)/(1+m) with the ACT (ln,exp) pair and
                # the DVE (1+m, recip) pair running in parallel off msq
                sq = wpool.tile([128, 2 * JD], bf16, tag="sq")
                nc.vector.tensor_mul(sq[:, :], s_n[:, :], s_n[:, :])
                msq = wpool.tile([128, 2 * D], bf16, tag="msq")
                with nc.allow_low_precision("10-term mag_sq; routing only"):
                    nc.vector.tensor_reduce(
                        msq[:, :].rearrange("p (c d) -> p c d", d=D),
                        sq[:, :].rearrange("p (c d j) -> p c d j", d=D, j=J),
                        axis=AX.X,
                        op=ALU.add,
                    )
                # it==0 carries the constant z0=1152 inside the squash: with
                # c=1/z0, msq here is z0^2-scaled, so F_eff = c*F(c^2*msq) =
                # exp(0.5*ln(msq) + 2*ln(c)) / (1 + c^2*msq), and v = sg*F_eff
                lnm = wpool.tile([128, 2 * D], f32, tag="lnm")
                nc.scalar.activation(lnm[:, :], msq[:, :], AF.Ln)
                rt = wpool.tile([128, 2 * D], f32, tag="rt")
                nc.scalar.activation(
                    rt[:, :], lnm[:, :], AF.Exp, scale=0.5,
                    bias=(zb0[:, 0:1] if it == 0 else 0.0),
                )
                dn = wpool.tile([128, 2 * D], f32, tag="dn")
                if it == 0:
                    nc.vector.tensor_scalar(
                        dn[:, :], msq[:, :], CC0 * CC0, 1.0,
                        op0=ALU.mult, op1=ALU.add,
                    )
                else:
                    nc.vector.tensor_scalar_add(dn[:, :], msq[:, :], 1.0)
                rc = wpool.tile([128, 2 * D], f32, tag="rc")
                nc.vector.reciprocal(rc[:, :], dn[:, :])
                f_t = wpool.tile([128, 2 * D], f32, tag="f_t")
                nc.vector.tensor_mul(f_t[:, :], rt[:, :], rc[:, :])

                # v = s * F (F broadcast over j); v lands directly in the
                # (b, (d,j)) layout the A-matmul needs -- no transposes
                vt = wpool.tile([128, 2 * JD], bf16, tag="vt")
                for ch, eng in enumerate((nc.vector, nc.gpsimd)):
                    eng.tensor_tensor(
                        vt[:, ch * JD:(ch + 1) * JD].rearrange("p (d j) -> p d j", j=J),
                        s_n[:, ch * JD:(ch + 1) * JD].rearrange("p (d j) -> p d j", j=J),
                        f_t[:, ch * D:(ch + 1) * D].unsqueeze(2).broadcast_to([128, D, J]),
                        ALU.mult,
                    )
                vb0 = vt[:, 0:JD]
                vb1 = vt[:, JD:2 * JD]

                # ---- fused per-group pipeline: A-path group g immediately
                # feeds that group's e-update, Wc gate and s-matmuls ----
                s_ps = (
                    ppool.tile([128, JD], f32, tag="s_ps0", name="s_ps0"),
                    ppool.tile([128, JD], f32, tag="s_ps1", name="s_ps1"),
                )
                z_ps = ppool.tile([J, 1], f32, tag="zbc_ps", name="z_ps")
                # A-path at k-tile granularity: per tile, PE matmul -> ACT
                # psum drain -> DVE gate -> DVE d-reduce, so the waves are
                # small and every engine streams; e/Wc/s-matmuls then fire
                # per 3-tile group.  Wc gating for the early groups runs on
                # the otherwise-idle GpSimd; the last group (on the serial
                # chain into the collective) stays on the faster DVE.
                r_g = [None] * NG
                for t_ in range(KT):
                    g, tt = t_ // 3, t_ % 3
                    a_ps = apool.tile([128, JD], f32, tag="a_ps")
                    nc.tensor.matmul(
                        a_ps[:, :],
                        xb0[:, t_ * 128:(t_ + 1) * 128], vb0,
                        start=True, stop=False,
                    )
                    nc.tensor.matmul(
                        a_ps[:, :],
                        xb1[:, t_ * 128:(t_ + 1) * 128], vb1,
                        start=False, stop=True,
                    )
                    a_sb = wpool3.tile([128, JD], bf16, tag="a_sb")
                    nc.scalar.copy(a_sb[:, :], a_ps[:, :])
                    p_t = wpool3.tile([128, JD], bf16, tag="p_t")
                    nc.vector.tensor_tensor(
                        p_t[:, :], wsb[g][:, tt, :], a_sb[:, :], ALU.mult,
                    )
                    if tt == 0:
                        r_g[g] = wpool3.tile(
                            [128, 3, J], f32, tag="r_t", name=f"r_g{g}"
                        )
                    nc.vector.tensor_reduce(
                        r_g[g][:, tt, :],
                        p_t.rearrange("p (d j) -> p j d", d=D, j=J),
                        axis=AX.X,
                        op=ALU.add,
                    )
                    if tt < 2:
                        continue
                    # ---- group complete: u-fold, e-update, z, Wc, s ----
                    uv_ps = ppool.tile([128, 3 * J], f32, tag="uv_ps")
                    for a in range(3):
                        nc.tensor.matmul(
                            uv_ps[:, a * J:(a + 1) * J], m8[:, :], r_g[g][:, a, :],
                            start=True, stop=True,
                        )
                    # e is exp(cumulative u_vj1): the +1 in b never matters
                    # (softmax is shift-invariant), so no bias anywhere
                    expuv = epool.tile(
                        [128, 3, J], bf16, tag=f"x{g}", name=f"expuv{g}"
                    )
                    nc.scalar.activation(
                        expuv[:, :, :],
                        uv_ps[:, :].rearrange("p (a j) -> p a j", j=J),
                        AF.Exp,
                    )
                    if it == 0:
                        e_tiles[g] = expuv
                    else:
                        # full e needed only for the z column; Wc chains off
                        # the previous iteration's Wc directly (below), so
                        # this multiply sits off the critical path
                        e_new = epool.tile([128, 3, J], bf16, tag=f"e{g}")
                        nc.vector.tensor_tensor(
                            e_new[:, :, :], e_tiles[g][:, :, :], expuv[:, :, :],
                            ALU.mult,
                        )
                        e_tiles[g] = e_new
                    xp_tiles[g] = expuv
                # ---- tail: z, Wc gates and s-matmuls, emitted after the
                # whole per-ktile pipeline so the (in-order) PE stream never
                # stalls behind a slow gate mid-pipeline ----
                for g in range(NG):
                    wc = epool.tile([128, 3, JD], bf16, tag=f"wc{g}")
                    (a_gate if g == 0 else nc.vector).tensor_tensor(
                        wc[:, :, :].rearrange("p t (d j) -> p t d j", j=J),
                        wsb[g][:, :, :].rearrange("p t (d j) -> p t d j", j=J),
                        e_tiles[g][:, :, :].unsqueeze(2).broadcast_to([128, 3, D, J]),
                        ALU.mult,
                    )
                    for a in range(3):
                        st = g == 0 and a == 0
                        sp = g == NG - 1 and a == 2
                        nc.tensor.matmul(
                            s_ps[0][:, :], xt[g][:, a, 0:128], wc[:, a, :],
                            start=st, stop=sp,
                        )
                        nc.tensor.matmul(
                            s_ps[1][:, :], xt[g][:, a, 128:B], wc[:, a, :],
                            start=st, stop=sp,
                        )
                    # z partial: z[j] = sum_i e[i,j] as a (J,1) column
                    for a in range(3):
                        nc.tensor.matmul(
                            z_ps[:, :], e_tiles[g][:, a, :], ones8[:, 0:1],
                            start=(g == 0 and a == 0),
                            stop=(g == NG - 1 and a == 2),
                        )
                cc_out = stage_and_collect(s_ps, z_ps, last=last_cc)

            # ---- post-ReduceScatter shard squash -> out ----
            sg3z = wpool.tile([32, JD + 1], bf16, tag="sg3z")
            nc.sync.dma_start(sg3z[:, :], cc_out[0:32, 0:JD + 1])
            zinv3 = wpool.tile([J, 1], f32, tag="zinv3")
            nc.vector.reciprocal(zinv3[:, :], sg3z[0:J, JD:JD + 1])
            zsel3 = wpool.tile([J, JD], f32, tag="zsel3")
            nc.vector.tensor_scalar_mul(zsel3[:, :], sel10[:, :], zinv3[:, 0:1])
            zbc3 = ppool.tile([32, JD], f32, tag="zbc_ps", name="zbc3")
            nc.tensor.matmul(zbc3[:, :], ones10[:, 0:32], zsel3[:, :], start=True, stop=True)
            sn3 = wpool.tile([32, JD], f32, tag="sn3")
            nc.vector.tensor_mul(sn3[:, :], sg3z[0:32, 0:JD], zbc3[:, :])
            sq3 = wpool.tile([32, JD], bf16, tag="sq3")
            nc.vector.tensor_mul(sq3[:, :], sn3[:, :], sn3[:, :])
            msq3 = wpool.tile([32, D], f32, tag="msq3")
            nc.vector.tensor_reduce(
                msq3[:, :],
                sq3[:, :].rearrange("p (d j) -> p d j", j=J),
                axis=AX.X,
                op=ALU.add,
            )
            ln3 = wpool.tile([32, D], f32, tag="ln3")
            nc.scalar.activation(ln3[:, :], msq3[:, :], AF.Ln)
            rt3 = wpool.tile([32, D], f32, tag="rt3")
            nc.scalar.activation(rt3[:, :], ln3[:, :], AF.Exp, scale=0.5)
            dn3 = wpool.tile([32, D], f32, tag="dn3")
            nc.vector.tensor_scalar_add(dn3[:, :], msq3[:, :], 1.0)
            rc3 = wpool.tile([32, D], f32, tag="rc3")
            nc.vector.reciprocal(rc3[:, :], dn3[:, :])
            f3 = wpool.tile([32, D], f32, tag="f3")
            nc.vector.tensor_mul(f3[:, :], rt3[:, :], rc3[:, :])
            v3 = wpool.tile([32, JD], f32, tag="v3")
            nc.vector.tensor_tensor(
                v3[:, :].rearrange("p (d j) -> p d j", j=J),
                sn3[:, :].rearrange("p (d j) -> p d j", j=J),
                f3[:, :].unsqueeze(2).broadcast_to([32, D, J]),
                ALU.mult,
            )
            nc.sync.dma_start(out_d[:, :], v3[:, :])

    nc.finalize()
    return nc


def _f32_blob():
    blob = np.zeros((128, F32_COLS), np.float32)
    blob[:, 0:128] = np.kron(np.eye(16, dtype=np.float32), np.ones((8, 8), np.float32)) / B
    blob[0:J, 128:256] = 1.0
    blob[0:J, 256:256 + JD] = np.tile(np.eye(J, dtype=np.float32), (1, D))
    return blob


def _prep_in_maps(x, W):
    x = np.asarray(x, np.float32)
    W = np.asarray(W, np.float32)
    Wm = W[0]
    f32_blob = _f32_blob()
    in_maps = []
    for c in range(NCORES):
        sl = slice(c * IL, (c + 1) * IL)
        xs = x[:, :, sl]                                            # (B, U, IL)
        xt = np.ascontiguousarray(xs.transpose(2, 1, 0).reshape(KL, B))
        xb = xt.T
        w = Wm[sl].transpose(0, 3, 2, 1).reshape(KL, JD)            # cols = (d, j)
        wt = w.reshape(KT, 128, JD).transpose(1, 0, 2)              # (128, KT, JD)
        xtt = xt.reshape(KT, 128, B).transpose(1, 0, 2)             # (128, KT, B)
        bf = np.zeros((128, BF_COLS), np.float32)
        o = 0
        for g, gt in enumerate(GROUPS):
            g0 = GOFF[g]
            bf[:, o:o + gt * JD] = wt[:, g0:g0 + gt].reshape(128, gt * JD)
            o += gt * JD
            bf[:, o:o + gt * B] = xtt[:, g0:g0 + gt].reshape(128, gt * B)
            o += gt * B
        bf[:, o:o + KL] = xb[0:128]; o += KL
        bf[:, o:o + KL] = xb[128:256]; o += KL
        bf[:, o] = 0.125; o += 1
        bf[0:J, o:o + 128] = 1.0; o += 128
        bf[0:J, o:o + JD] = np.tile(np.eye(J, dtype=np.float32), (1, D)); o += JD
        assert o == BF_COLS
        in_maps.append({
            "bfin": bf.astype(ml_dtypes.bfloat16),
            "f32in": f32_blob,
        })
    return in_maps


def run(x, W, trace=False):
    from concourse.bass_utils import run_bass_kernel_spmd

    if "nc" not in _CACHE:
        _CACHE["nc"] = _build_module()
    nc = _CACHE["nc"]
    in_maps = _prep_in_maps(x, W)
    res = run_bass_kernel_spmd(
        nc, in_maps, core_ids=list(range(NCORES)), trace=trace
    )
    v = np.concatenate(
        [np.asarray(res.results[c]["out"], np.float32) for c in range(NCORES)],
        axis=0,
    )                                                               # (B, (d,j))
    out = v.reshape(B, D, J).transpose(0, 2, 1)[..., None]
    return np.ascontiguousarray(out.astype(np.float32)), res


def kernel(x, W):
    out, _ = run(x, W, trace=False)
    return out


# revision 57
# speedup vs baseline: 1.0752x; 1.0002x over previous
"""CapsuleLayer dynamic-routing kernel for 8 TRN2 NeuronCores.

Sharding: in_size (i) is split 8 ways (144 rows/core); every core holds the
full batch.  u_hat (B,1152,10,16 = 189MB) is never materialized: both the
c-weighted sum (s_j) and the agreement update factor through x and W:

    s_un[b, (d,j)]   = sum_{(i,u)} x[b,u,i] * (e[i,j] * W[i,j,d,u])
    A[(i,u), (d,j)]  = sum_b x[b,u,i] * v[b,j,d]
    u_vj1[i,j]       = (1/B) sum_{u,d} W[i,j,d,u] * A[(i,u),(d,j)]

with e unnormalized; the softmax denominator z_j = sum_i e[i,j] rides inside
the per-iteration collective (the only cross-core traffic): AllReduce for
routing iterations 1-2, ReduceScatter for the final one (each core then
squashes and emits only its own 32-batch output shard, gathered host-side).

Structural choices vs the straightforward version:
  * iteration 0 has b=1 (uniform softmax), so s0 is a plain matmul on raw W
    with a compile-time softmax denominator z0=1152 -- no exp, no gating, no
    z column in the first collective;
  * b_ij is never materialized: e is tracked multiplicatively,
    e_{k+1} = e_k * exp(u_vj1), with iteration 1's e = exp(1 + u_vj1)
    produced by a single fused activation (Exp with bias=1);
  * inputs stream in three (W,x) k-groups so the first matmuls start while
    the rest of the load is in flight;
  * the agreement block is pipelined per k-group across four engines
    (PE matmul -> ACT drain -> gate -> DVE d-reduce -> PE u-fold -> ACT exp
    -> DVE gate -> PE s-matmul);
  * collective payloads are [128, W]-shaped so DMAs move 128 fat descriptors
    instead of 256 thin ones.
All matmuls run in bf16 with fp32 PSUM accumulation; exp/ln/copy live in one
ACT function table so only one table load is ever issued.
"""

import os
import sys

import numpy as np

for _p in ("/opt/trn_rl_repo",):
    if _p not in sys.path and os.path.isdir(_p):
        sys.path.insert(0, _p)

import ml_dtypes

NCORES = 8
B, U, I = 256, 8, 1152
J, D = 10, 16
IL = I // NCORES        # 144 in_size rows per core
KL = IL * U             # 1152 local contraction length (i,u)
KT = KL // 128          # 9 partition tiles
GROUPS = [3, 3, 3]      # k-tile groups for loads / e / Wc / s-matmuls
GOFF = [0, 3, 6]        # cumulative k-tile offsets
NG = len(GROUPS)
JD = J * D              # 160
BF_COLS = KT * (JD + B) + 2 * KL + 1 + 256 + JD     # groups | xb | ones8 | ones10b | sel10b | m8b
F32_COLS = 256 + JD                                 # m8 | ones10 | sel10

_CACHE = {}


def _build_module(a_gate_pool=True):
    import concourse.bacc as bacc
    import concourse.mybir as mybir
    import concourse.tile as tile

    f32 = mybir.dt.float32
    bf16 = mybir.dt.bfloat16
    fp8 = mybir.dt.float8e4
    CSC = 1.0 / 16.0            # fp8 pre-scale for the AllReduce payload
    CC0 = 1.0 / (I * CSC)       # iteration-0 softmax const with CSC folded
    AF = mybir.ActivationFunctionType
    ALU = mybir.AluOpType
    AX = mybir.AxisListType

    # Force the act-table pass's first-match lookup to land every function
    # we use (Exp, Ln, Copy) on the one table that covers them all, so only
    # a single LoadActFuncSet is ever emitted.  Table *ids* are positional,
    # so we only hide functions from other tables, never reorder.
    if not hasattr(bacc, "_orig_get_activation_tables"):
        bacc._orig_get_activation_tables = bacc.get_activation_tables

        def _patched_tables(arch):
            tabs = bacc._orig_get_activation_tables(arch)
            AF_ = mybir.ActivationFunctionType
            ours = {AF_.Exp, AF_.Ln, AF_.Copy, AF_.Square, AF_.Identity}
            out = {}
            for name, s in tabs.items():
                if name == "natural_log_exp_and_others":
                    out[name] = s
                else:
                    out[name] = s - ours
            return out

        bacc.get_activation_tables = _patched_tables

    nc = bacc.Bacc(
        "TRN2", target_bir_lowering=False, debug=False, num_devices=NCORES
    )

    bf_d = nc.declare_dram_parameter("bfin", [128, BF_COLS], bf16, isOutput=False)
    f32_d = nc.declare_dram_parameter("f32in", [128, F32_COLS], f32, isOutput=False)
    out_d = nc.declare_dram_parameter("out", [B // NCORES, JD], f32, isOutput=True)

    a_gate = None  # set below

    with tile.TileContext(nc) as tc:
        with (
            tc.tile_pool(name="const", bufs=1) as cpool,
            tc.tile_pool(name="work", bufs=2) as wpool,
            tc.tile_pool(name="psum", bufs=1, space="PSUM") as ppool,
            tc.tile_pool(name="apsum", bufs=3, space="PSUM") as apool,
            tc.tile_pool(name="work3", bufs=3) as wpool3,
            tc.tile_pool(name="ework", bufs=2) as epool,
            tc.tile_pool(name="dram", bufs=3, space="DRAM") as dpool,
        ):
            a_gate = nc.gpsimd if a_gate_pool else nc.vector

            # ---- streamed loads: 3 (W | xt) k-groups so matmuls start
            # early, then the A-path / normalization constants ----
            wsb, xt = [], []
            off = 0
            for g, gt in enumerate(GROUPS):
                grp = gt * (JD + B)
                t_ = cpool.tile([128, grp], bf16, tag=f"grp{g}", name=f"grp{g}")
                nc.sync.dma_start(t_[:, :], bf_d[:, off:off + grp])
                off += grp
                wsb.append(t_[:, 0:gt * JD].rearrange("p (t n) -> p t n", n=JD))
                xt.append(t_[:, gt * JD:grp].rearrange("p (t b) -> p t b", b=B))
            XB0 = off
            xb_sb = cpool.tile([128, 2 * KL + 1 + 256 + JD], bf16)
            xb0 = xb_sb[:, 0:KL]
            xb1 = xb_sb[:, KL:2 * KL]
            ones8 = xb_sb[:, 2 * KL:2 * KL + 1]
            ones10b = xb_sb[0:J, 2 * KL + 1:2 * KL + 1 + 128]
            sel10b = xb_sb[0:J, 2 * KL + 1 + 128:2 * KL + 1 + 128 + JD]
            m8b = xb_sb[:, 2 * KL + 1 + 128 + JD:2 * KL + 1 + 256 + JD]
            f32_sb = cpool.tile([128, F32_COLS], f32)
            m8 = f32_sb[:, 0:128]
            ones10 = f32_sb[0:J, 128:256]      # (10, 128) of ones
            sel10 = f32_sb[0:J, 256:256 + JD]  # sel10[j', d*J+j] = (j==j')

            def s_mms(s_ps, rhs_of, first, last):
                # the two b-halves live in separate PSUM banks: a start=True
                # matmul clears its bank, so interleaved accumulation groups
                # must not share one
                s_ps0, s_ps1 = s_ps
                for g, gt in enumerate(GROUPS):
                    for t_ in range(gt):
                        st = first and g == 0 and t_ == 0
                        sp = last and g == NG - 1 and t_ == gt - 1
                        nc.tensor.matmul(
                            s_ps0[:, :], xt[g][:, t_, 0:128], rhs_of(g, t_),
                            start=st, stop=sp,
                        )
                        nc.tensor.matmul(
                            s_ps1[:, :], xt[g][:, t_, 128:B], rhs_of(g, t_),
                            start=st, stop=sp,
                        )

            def stage_and_collect(s_ps, z_ps, last):
                # stage [s | z] in SBUF; PSUM itself is not DMA-readable.
                # Payload stays [128, W]-shaped (fat rows -> 128 descriptors)
                # for the AllReduce iterations; the final ReduceScatter needs
                # batch on the outer axis so each core receives its 32-row
                # output shard.
                s_ps0, s_ps1 = s_ps
                if not last:
                    # fp8 payload (half the AllReduce bytes): s_un has sigma
                    # ~260 vs e4m3 max 448, so pre-scale by 1/16 on the way
                    # out; the 16 cancels exactly against z (which rides at
                    # z/16) or folds into the iteration-0 squash constants
                    width = 2 * JD + (1 if z_ps is not None else 0)
                    s_sb = wpool.tile([128, width], fp8, tag="s_sb")
                    nc.scalar.mul(s_sb[:, 0:JD], s_ps0[:, :], CSC)
                    nc.vector.tensor_scalar_mul(
                        s_sb[:, JD:2 * JD], s_ps1[:, :], CSC
                    )
                    if z_ps is not None:
                        nc.vector.tensor_scalar_mul(
                            s_sb[0:J, 2 * JD:2 * JD + 1], z_ps[:, :], CSC
                        )
                    cc_in = dpool.tile([128, width], fp8, tag="cc_in")
                    nc.sync.dma_start(cc_in[:, :], s_sb[:, :])
                    cc_out = dpool.tile([128, width], fp8, tag="cc_out", name="ccout")
                    nc.gpsimd.collective_compute(
                        "AllReduce",
                        ALU.add,
                        replica_groups=[list(range(NCORES))],
                        ins=[cc_in.opt()],
                        outs=[cc_out.opt()],
                    )
                    return cc_out
                # final iteration: bf16 payload, feeds the output directly
                s_sb = wpool.tile([128, 2 * (JD + 1)], bf16, tag="s_sb3")
                for r in range(4):
                    nc.vector.tensor_copy(
                        s_sb[r * 32:r * 32 + J, JD:JD + 1], z_ps[:, :]
                    )
                    nc.vector.tensor_copy(
                        s_sb[r * 32:r * 32 + J, 2 * JD + 1:2 * JD + 2], z_ps[:, :]
                    )
                nc.scalar.copy(s_sb[:, 0:JD], s_ps0[:, :])
                nc.vector.tensor_copy(s_sb[:, JD + 1:2 * JD + 1], s_ps1[:, :])
                cc_in = dpool.tile([B, JD + 1], bf16, tag="cc3_in")
                nc.sync.dma_start(
                    cc_in[:, :].rearrange("(c p) n -> p c n", p=128),
                    s_sb[:, :].rearrange("p (c n) -> p c n", n=JD + 1),
                )
                cc_out = dpool.tile([B // NCORES, JD + 1], bf16, tag="cc3_out", name="ccout3")
                nc.gpsimd.collective_compute(
                    "ReduceScatter",
                    ALU.add,
                    replica_groups=[list(range(NCORES))],
                    ins=[cc_in.opt()],
                    outs=[cc_out.opt()],
                )
                return cc_out

            # PE warm-up: the cost model's p-state needs ~3us of continuous
            # matmul activity before full rate; burn it on zeros during the
            # input-load wait so the real matmuls start warm
            warm = cpool.tile([128, 256], bf16, name="warm")
            nc.vector.memset(warm[:, :], 0.0)
            # per-partition bias column for the folded-z0 squash constant
            zb0 = cpool.tile([128, 1], f32, name="zb0")
            nc.vector.memset(zb0[:, :], 2.0 * float(np.log(CC0)))
            # warm_ps shares a PSUM bank with zbc/z (all short-lived, strictly
            # ordered through the tag's WAR chain)
            warm_ps = ppool.tile([128, 256], f32, tag="zbc_ps", name="warm_ps")
            for _ in range(6):
                nc.tensor.matmul(
                    warm_ps[:, :], warm[:, 0:128], warm[:, :],
                    start=True, stop=True,
                )

            # ---- iteration 0 front: b0 == 1 -> uniform softmax: plain
            # matmul on raw W, denominator is the constant 1152 ----
            s_ps = (
                ppool.tile([128, JD], f32, tag="s_ps0", name="s_ps0"),
                ppool.tile([128, JD], f32, tag="s_ps1", name="s_ps1"),
            )
            s_mms(s_ps, lambda g, t_: wsb[g][:, t_, :], True, True)
            # A-path / normalization loads go on the same (SP) queue as the
            # three critical (W | xt) groups: DMA arbitration is arrival
            # order, so another queue's DMA would cut ahead of group data
            nc.sync.dma_start(xb_sb[:, :], bf_d[:, XB0:BF_COLS])
            nc.sync.dma_start(f32_sb[:, :], f32_d[:, :])
            cc_out = stage_and_collect(s_ps, None, last=False)

            e_tiles = [None] * NG
            xp_tiles = [None] * NG
            wc_prev = [None] * NG
            for it in range(2):
                last_cc = it == 1

                # ---- post-AllReduce squash -> v ----
                width = 2 * JD + (1 if it > 0 else 0)
                sgz = wpool.tile([128, width], fp8, tag="sgz")
                nc.sync.dma_start(sgz[:, :], cc_out[:, :])
                sg = sgz[:, 0:2 * JD]

                if it == 0:
                    # z0 = 1152 exactly (uniform softmax over in_size): fold
                    # it into the squash constants instead of scaling s --
                    # the squash then runs directly on the raw AllReduce sum
                    s_n = sg
                else:
                    s_n = wpool.tile([128, 2 * JD], bf16, tag="s_n")
                    # zinv at (d,j) columns on all 128 partitions: recip the
                    # z column, scale sel10 by it, lift via a (K=10) matmul
                    # (bf16: this z only steers routing iteration 2, and a
                    # bf16 matmul is 4x cheaper than fp32)
                    zinv = wpool.tile([J, 1], f32, tag="zinv")
                    nc.vector.reciprocal(zinv[:, :], sgz[0:J, 2 * JD:2 * JD + 1])
                    zsel = wpool.tile([J, JD], bf16, tag="zsel")
                    nc.vector.tensor_scalar_mul(zsel[:, :], sel10b[:, :], zinv[:, 0:1])
                    zbc_ps = ppool.tile([128, JD], f32, tag="zbc_ps")
                    nc.tensor.matmul(
                        zbc_ps[:, :], ones10b[:, :], zsel[:, :], start=True, stop=True
                    )
                    nc.vector.tensor_tensor(
                        s_n[:, :].rearrange("p (c n) -> p c n", n=JD),
                        sg.rearrange("p (c n) -> p c n", n=JD),
                        zbc_ps[:, :].unsqueeze(1).broadcast_to([128, 2, JD]),
                        ALU.mult,
                    )

                # mag_sq[b, d] = sum_j s[b, (d,j)]^2 : square then innermost
                # reduce; F = sqrt(m)/(1+m) with the ACT (ln,exp) pair and
                # the DVE (1+m, recip) pair running in parallel off msq
                sq = wpool.tile([128, 2 * JD], bf16, tag="sq")
                nc.vector.tensor_mul(sq[:, :], s_n[:, :], s_n[:, :])
                msq = wpool.tile([128, 2 * D], bf16, tag="msq")
                with nc.allow_low_precision("10-term mag_sq; routing only"):
                    nc.vector.tensor_reduce(
                        msq[:, :].rearrange("p (c d) -> p c d", d=D),
                        sq[:, :].rearrange("p (c d j) -> p c d j", d=D, j=J),
                        axis=AX.X,
                        op=ALU.add,
                    )
                # it==0 carries the constant z0=1152 inside the squash: with
                # c=1/z0, msq here is z0^2-scaled, so F_eff = c*F(c^2*msq) =
                # exp(0.5*ln(msq) + 2*ln(c)) / (1 + c^2*msq), and v = sg*F_eff
                lnm = wpool.tile([128, 2 * D], f32, tag="lnm")
                nc.scalar.activation(lnm[:, :], msq[:, :], AF.Ln)
                rt = wpool.tile([128, 2 * D], f32, tag="rt")
                nc.scalar.activation(
                    rt[:, :], lnm[:, :], AF.Exp, scale=0.5,
                    bias=(zb0[:, 0:1] if it == 0 else 0.0),
                )
                dn = wpool.tile([128, 2 * D], f32, tag="dn")
                if it == 0:
                    nc.vector.tensor_scalar(
                        dn[:, :], msq[:, :], CC0 * CC0, 1.0,
                        op0=ALU.mult, op1=ALU.add,
                    )
                else:
                    nc.vector.tensor_scalar_add(dn[:, :], msq[:, :], 1.0)
                rc = wpool.tile([128, 2 * D], f32, tag="rc")
                nc.vector.reciprocal(rc[:, :], dn[:, :])
                f_t = wpool.tile([128, 2 * D], f32, tag="f_t")
                nc.vector.tensor_mul(f_t[:, :], rt[:, :], rc[:, :])

                # v = s * F (F broadcast over j); v lands directly in the
                # (b, (d,j)) layout the A-matmul needs -- no transposes
                vt = wpool.tile([128, 2 * JD], bf16, tag="vt")
                for ch, eng in enumerate((nc.vector, nc.gpsimd)):
                    eng.tensor_tensor(
                        vt[:, ch * JD:(ch + 1) * JD].rearrange("p (d j) -> p d j", j=J),
                        s_n[:, ch * JD:(ch + 1) * JD].rearrange("p (d j) -> p d j", j=J),
                        f_t[:, ch * D:(ch + 1) * D].unsqueeze(2).broadcast_to([128, D, J]),
                        ALU.mult,
                    )
                vb0 = vt[:, 0:JD]
                vb1 = vt[:, JD:2 * JD]

                # ---- fused per-group pipeline: A-path group g immediately
                # feeds that group's e-update, Wc gate and s-matmuls ----
                s_ps = (
                    ppool.tile([128, JD], f32, tag="s_ps0", name="s_ps0"),
                    ppool.tile([128, JD], f32, tag="s_ps1", name="s_ps1"),
                )
                z_ps = ppool.tile([J, 1], f32, tag="zbc_ps", name="z_ps")
                # A-path at k-tile granularity: per tile, PE matmul -> ACT
                # psum drain -> DVE gate -> DVE d-reduce, so the waves are
                # small and every engine streams; e/Wc/s-matmuls then fire
                # per 3-tile group.  Wc gating for the early groups runs on
                # the otherwise-idle GpSimd; the last group (on the serial
                # chain into the collective) stays on the faster DVE.
                r_g = [None] * NG
                for t_ in range(KT):
                    g, tt = t_ // 3, t_ % 3
                    a_ps = apool.tile([128, JD], f32, tag="a_ps")
                    nc.tensor.matmul(
                        a_ps[:, :],
                        xb0[:, t_ * 128:(t_ + 1) * 128], vb0,
                        start=True, stop=False,
                    )
                    nc.tensor.matmul(
                        a_ps[:, :],
                        xb1[:, t_ * 128:(t_ + 1) * 128], vb1,
                        start=False, stop=True,
                    )
                    a_sb = wpool3.tile([128, JD], bf16, tag="a_sb")
                    nc.scalar.copy(a_sb[:, :], a_ps[:, :])
                    p_t = wpool3.tile([128, JD], bf16, tag="p_t")
                    nc.vector.tensor_tensor(
                        p_t[:, :], wsb[g][:, tt, :], a_sb[:, :], ALU.mult,
                    )
                    if tt == 0:
                        r_g[g] = wpool3.tile(
                            [128, 3, J], bf16, tag="r_t", name=f"r_g{g}"
                        )
                    with nc.allow_low_precision("16-term d-fold; routing only"):
                        nc.vector.tensor_reduce(
                            r_g[g][:, tt, :],
                            p_t.rearrange("p (d j) -> p j d", d=D, j=J),
                            axis=AX.X,
                            op=ALU.add,
                        )
                    if tt < 2:
                        continue
                    # ---- group complete: u-fold, e-update, z, Wc, s ----
                    uv_ps = ppool.tile([128, 3 * J], f32, tag="uv_ps")
                    for a in range(3):
                        nc.tensor.matmul(
                            uv_ps[:, a * J:(a + 1) * J], m8b[:, :], r_g[g][:, a, :],
                            start=True, stop=True,
                        )
                    # e is exp(cumulative u_vj1): the +1 in b never matters
                    # (softmax is shift-invariant), so no bias anywhere
                    expuv = epool.tile(
                        [128, 3, J], bf16, tag=f"x{g}", name=f"expuv{g}"
                    )
                    nc.scalar.activation(
                        expuv[:, :, :],
                        uv_ps[:, :].rearrange("p (a j) -> p a j", j=J),
                        AF.Exp,
                    )
                    if it == 0:
                        e_tiles[g] = expuv
                    else:
                        # full e needed only for the z column; Wc chains off
                        # the previous iteration's Wc directly (below), so
                        # this multiply sits off the critical path
                        e_new = epool.tile([128, 3, J], bf16, tag=f"e{g}")
                        nc.vector.tensor_tensor(
                            e_new[:, :, :], e_tiles[g][:, :, :], expuv[:, :, :],
                            ALU.mult,
                        )
                        e_tiles[g] = e_new
                    xp_tiles[g] = expuv
                # ---- tail: z, Wc gates and s-matmuls, emitted after the
                # whole per-ktile pipeline so the (in-order) PE stream never
                # stalls behind a slow gate mid-pipeline ----
                for g in range(NG):
                    wc = epool.tile([128, 3, JD], bf16, tag=f"wc{g}")
                    (a_gate if g == 0 else nc.vector).tensor_tensor(
                        wc[:, :, :].rearrange("p t (d j) -> p t d j", j=J),
                        wsb[g][:, :, :].rearrange("p t (d j) -> p t d j", j=J),
                        e_tiles[g][:, :, :].unsqueeze(2).broadcast_to([128, 3, D, J]),
                        ALU.mult,
                    )
                    for a in range(3):
                        st = g == 0 and a == 0
                        sp = g == NG - 1 and a == 2
                        nc.tensor.matmul(
                            s_ps[0][:, :], xt[g][:, a, 0:128], wc[:, a, :],
                            start=st, stop=sp,
                        )
                        nc.tensor.matmul(
                            s_ps[1][:, :], xt[g][:, a, 128:B], wc[:, a, :],
                            start=st, stop=sp,
                        )
                    # z partial: z[j] = sum_i e[i,j] as a (J,1) column
                    for a in range(3):
                        nc.tensor.matmul(
                            z_ps[:, :], e_tiles[g][:, a, :], ones8[:, 0:1],
                            start=(g == 0 and a == 0),
                            stop=(g == NG - 1 and a == 2),
                        )
                cc_out = stage_and_collect(s_ps, z_ps, last=last_cc)

            # ---- post-ReduceScatter shard squash -> out ----
            sg3z = wpool.tile([32, JD + 1], bf16, tag="sg3z")
            nc.sync.dma_start(sg3z[:, :], cc_out[0:32, 0:JD + 1])
            zinv3 = wpool.tile([J, 1], f32, tag="zinv3")
            nc.vector.reciprocal(zinv3[:, :], sg3z[0:J, JD:JD + 1])
            zsel3 = wpool.tile([J, JD], f32, tag="zsel3")
            nc.vector.tensor_scalar_mul(zsel3[:, :], sel10[:, :], zinv3[:, 0:1])
            zbc3 = ppool.tile([32, JD], f32, tag="zbc_ps", name="zbc3")
            nc.tensor.matmul(zbc3[:, :], ones10[:, 0:32], zsel3[:, :], start=True, stop=True)
            sn3 = wpool.tile([32, JD], f32, tag="sn3")
            nc.vector.tensor_mul(sn3[:, :], sg3z[0:32, 0:JD], zbc3[:, :])
            sq3 = wpool.tile([32, JD], bf16, tag="sq3")
            nc.vector.tensor_mul(sq3[:, :], sn3[:, :], sn3[:, :])
            msq3 = wpool.tile([32, D], f32, tag="msq3")
            nc.vector.tensor_reduce(
                msq3[:, :],
                sq3[:, :].rearrange("p (d j) -> p d j", j=J),
                axis=AX.X,
                op=ALU.add,
            )
            ln3 = wpool.tile([32, D], f32, tag="ln3")
            nc.scalar.activation(ln3[:, :], msq3[:, :], AF.Ln)
            rt3 = wpool.tile([32, D], f32, tag="rt3")
            nc.scalar.activation(rt3[:, :], ln3[:, :], AF.Exp, scale=0.5)
            dn3 = wpool.tile([32, D], f32, tag="dn3")
            nc.vector.tensor_scalar_add(dn3[:, :], msq3[:, :], 1.0)
            rc3 = wpool.tile([32, D], f32, tag="rc3")
            nc.vector.reciprocal(rc3[:, :], dn3[:, :])
            f3 = wpool.tile([32, D], f32, tag="f3")
            nc.vector.tensor_mul(f3[:, :], rt3[:, :], rc3[:, :])
            v3 = wpool.tile([32, JD], f32, tag="v3")
            nc.vector.tensor_tensor(
                v3[:, :].rearrange("p (d j) -> p d j", j=J),
                sn3[:, :].rearrange("p (d j) -> p d j", j=J),
                f3[:, :].unsqueeze(2).broadcast_to([32, D, J]),
                ALU.mult,
            )
            nc.sync.dma_start(out_d[:, :], v3[:, :])

    nc.finalize()
    return nc


def _f32_blob():
    blob = np.zeros((128, F32_COLS), np.float32)
    blob[:, 0:128] = np.kron(np.eye(16, dtype=np.float32), np.ones((8, 8), np.float32)) / B
    blob[0:J, 128:256] = 1.0
    blob[0:J, 256:256 + JD] = np.tile(np.eye(J, dtype=np.float32), (1, D))
    return blob


def _prep_in_maps(x, W):
    x = np.asarray(x, np.float32)
    W = np.asarray(W, np.float32)
    Wm = W[0]
    f32_blob = _f32_blob()
    in_maps = []
    for c in range(NCORES):
        sl = slice(c * IL, (c + 1) * IL)
        xs = x[:, :, sl]                                            # (B, U, IL)
        xt = np.ascontiguousarray(xs.transpose(2, 1, 0).reshape(KL, B))
        xb = xt.T
        w = Wm[sl].transpose(0, 3, 2, 1).reshape(KL, JD)            # cols = (d, j)
        wt = w.reshape(KT, 128, JD).transpose(1, 0, 2)              # (128, KT, JD)
        xtt = xt.reshape(KT, 128, B).transpose(1, 0, 2)             # (128, KT, B)
        bf = np.zeros((128, BF_COLS), np.float32)
        o = 0
        for g, gt in enumerate(GROUPS):
            g0 = GOFF[g]
            bf[:, o:o + gt * JD] = wt[:, g0:g0 + gt].reshape(128, gt * JD)
            o += gt * JD
            bf[:, o:o + gt * B] = xtt[:, g0:g0 + gt].reshape(128, gt * B)
            o += gt * B
        bf[:, o:o + KL] = xb[0:128]; o += KL
        bf[:, o:o + KL] = xb[128:256]; o += KL
        bf[:, o] = 0.125; o += 1
        bf[0:J, o:o + 128] = 1.0; o += 128
        bf[0:J, o:o + JD] = np.tile(np.eye(J, dtype=np.float32), (1, D)); o += JD
        bf[:, o:o + 128] = np.kron(np.eye(16, dtype=np.float32), np.ones((8, 8), np.float32)) / B; o += 128
        assert o == BF_COLS
        in_maps.append({
            "bfin": bf.astype(ml_dtypes.bfloat16),
            "f32in": f32_blob,
        })
    return in_maps


def run(x, W, trace=False):
    from concourse.bass_utils import run_bass_kernel_spmd

    if "nc" not in _CACHE:
        _CACHE["nc"] = _build_module()
    nc = _CACHE["nc"]
    in_maps = _prep_in_maps(x, W)
    res = run_bass_kernel_spmd(
        nc, in_maps, core_ids=list(range(NCORES)), trace=trace
    )
    v = np.concatenate(
        [np.asarray(res.results[c]["out"], np.float32) for c in range(NCORES)],
        axis=0,
    )                                                               # (B, (d,j))
    out = v.reshape(B, D, J).transpose(0, 2, 1)[..., None]
    return np.ascontiguousarray(out.astype(np.float32)), res


def kernel(x, W):
    out, _ = run(x, W, trace=False)
    return out
